# revision 5
# baseline (speedup 1.0000x reference)
"""Trainium2 Bass kernel for 3x3 (k=2m+1) morphological erosion (sliding-window
min) over [B, C, H, W] fp32, B=8 sharded across 8 NeuronCores (one batch per
core).

v3 scheme (per core, shard = one batch of C=8 channel images, 1024x1024):
  - each partition holds RPP=16 consecutive image rows (ppi=64 partitions per
    image, 2 images per 128-partition stack), processed in CC=256-column
    chunks with a 1-column halo (PW=258).
  - the separable 3x3 min runs in bf16 on DVE at the 2x_1p rate (2-byte
    dtype + unit innermost stride). min never creates new values, so the
    total error is one bf16 rounding of the input (~2^-9 relative), far
    inside the 2e-2 gate.
  - the device kernel STORES bf16 (halves store traffic: 64MB -> 48MB/core
    round trip); kernel() widens to fp32 on the host after the gather.
  - ACT (scalar) engine does the fp32->bf16 input convert (first pass only;
    chained passes read bf16 scratch directly) and drives store DMAs; loads
    and the per-partition-block boundary-row halo copies ride the SP queue;
    PAD memsets go to Pool. Emission is software-pipelined (skew 2) so no
    in-order sequencer blocks a neighbor engine's next tile.
  - V pass: tmp[j] = min(row j, row j+1); interior v rows from tmp + row
    j+2; the 2 per-partition boundary rows take their missing neighbor from
    a small bf16 side tile bt (partition-shifted SBUF->SBUF DMA; image-edge
    partitions: PAD memset at block tops (legal start partitions 0/64),
    own-row duplication DMA at block bottoms).
  - Cost model: DMA engines ~147us, DVE ~141us, ACT ~55us -> balanced
    DMA/DVE at the bf16-store memory roofline.
  - m>1 runs as m chained passes (bf16 DRAM ping-pong) inside one NEFF.
"""

import sys

sys.path.insert(0, "/opt/trn_rl_repo")

import numpy as np

import concourse.bass as bass
import concourse.tile as tile
from concourse import bacc, mybir

PAD = 1.0e9
F32 = mybir.dt.float32
BF16 = mybir.dt.bfloat16
MIN = mybir.AluOpType.min

CC = 256  # column chunk width
RPP = 16  # image rows per partition

_cache = {}


def _emit_pass_f32eo2(nc, pools, x_d, o_d, C, H, W, cc, rpp, in_f32):
    """One erosion pass, fp32 compute, even/odd V pass over halo-loaded rows.

    Each partition loads R+2 rows (its R rows plus one halo row each side,
    overlapping DRAM reads; +2/R load bytes) so the V pass is exactly 3
    row-stepped DVE ops with no boundary fixups:
      p[j] = min(e[2j], e[2j+1])          j = 0..R/2
      v[2j] = min(p[j], e[2j+2])          j = 0..R/2-1
      v[2j+1] = min(e[2j+1], p[j+1])      j = 0..R/2-1
    Image-edge partitions: top halo row = PAD memset (start partitions
    0/64: legal), bottom halo row = own-last-row duplication DMA
    (min-idempotent). All slices keep unit innermost stride (full DVE
    rate). The final H op writes bf16 (free output cast); host widens.
    """
    from concourse.ap import AP as _AP

    ppi = H // rpp
    ips = max(1, 128 // ppi)
    inp, xbp, bnd, vtm, vt, htm, obp = pools
    R = rpp
    E = R + 2  # rows incl halo
    PW = cc + 2
    assert R % 2 == 0

    tiles = [(s0, c0) for s0 in range(0, C, ips) for c0 in range(0, W, cc)]
    front = {}

    def emit_front(i):
        s0, c0 = tiles[i]
        n_img = min(ips, C - s0)
        P = n_img * ppi
        last = c0 + cc == W
        t = inp.tile([128, E, PW], F32, name="t")
        if in_f32:
            ld = t
        else:
            ld = xbp.tile([128, E, PW], BF16, name="xb")
        wlo = max(c0 - 1, 0)
        whi = min(c0 + cc + 1, W)
        nw = whi - wlo
        dlo = 1 if c0 == 0 else 0
        for im in range(n_img):
            img = x_d[s0 + im]  # [H, W]
            p0 = im * ppi
            # interior partitions 1..ppi-2: rows rpp*p-1 .. rpp*p+rpp
            src = _AP(
                tensor=img.tensor,
                offset=img.offset + (rpp - 1) * W + wlo,
                ap=[[rpp * W, ppi - 2], [W, E], [1, nw]],
            )
            nc.sync.dma_start(
                ld[p0 + 1 : p0 + ppi - 1, :, dlo : dlo + nw], src
            )
            # top partition: rows 0..rpp into slots 1..E-1; slot 0 = PAD
            nc.sync.dma_start(
                ld[p0 : p0 + 1, 1:E, dlo : dlo + nw],
                img[0 : rpp + 1, wlo:whi].rearrange("(p r) w -> p r w", p=1),
            )
            nc.gpsimd.memset(ld[p0 : p0 + 1, 0:1, :], PAD)
            # bottom partition: rows H-rpp-1..H-1 into slots 0..E-2;
            # slot E-1 = duplicate of the image's last row (min-idempotent)
            pe = p0 + ppi - 1
            nc.sync.dma_start(
                ld[pe : pe + 1, 0 : E - 1, dlo : dlo + nw],
                img[H - rpp - 1 : H, wlo:whi].rearrange("(p r) w -> p r w", p=1),
            )
            nc.sync.dma_start(
                ld[pe : pe + 1, E - 1 : E, dlo : dlo + nw],
                img[H - 1 : H, wlo:whi].rearrange("(p r) w -> p r w", p=1),
            )
        if c0 == 0:
            nc.gpsimd.memset(ld[0:P, :, 0:1], PAD)
        if last:
            nc.gpsimd.memset(ld[0:P, :, PW - 1 : PW], PAD)
        if not in_f32:
            nc.scalar.copy(t[0:P], ld[0:P])
        front[i] = (t, n_img, P, s0, c0)

    def emit_back(i):
        t, n_img, P, s0, c0 = front.pop(i)
        Rh = R // 2
        v = vt.tile([128, R, PW], F32, name="v")
        p = vtm.tile([128, Rh + 1, PW], F32, name="p")
        nc.vector.tensor_tensor(
            out=p[0:P], in0=t[0:P, 0:E:2, :], in1=t[0:P, 1:E:2, :], op=MIN
        )
        nc.vector.tensor_tensor(
            out=v[0:P, 0:R:2, :],
            in0=p[0:P, 0:Rh, :],
            in1=t[0:P, 2:E:2, :],
            op=MIN,
        )
        nc.vector.tensor_tensor(
            out=v[0:P, 1:R:2, :],
            in0=t[0:P, 1 : E - 2 : 2, :],
            in1=p[0:P, 1 : Rh + 1, :],
            op=MIN,
        )

        h = htm.tile([128, R, cc + 1], F32, name="h")
        nc.vector.tensor_tensor(
            out=h[0:P], in0=v[0:P, :, 0 : cc + 1], in1=v[0:P, :, 1 : cc + 2],
            op=MIN,
        )
        ob = obp.tile([128, R, cc], BF16, name="ob")
        nc.vector.tensor_tensor(
            out=ob[0:P], in0=h[0:P, :, 0:cc], in1=v[0:P, :, 2 : cc + 2], op=MIN
        )

        for im in range(n_img):
            dst = o_d[s0 + im, :, c0 : c0 + cc].rearrange(
                "(p r) w -> p r w", p=ppi
            )
            p0 = im * ppi
            nc.scalar.dma_start(dst, ob[p0 : p0 + ppi, :, :])

    skew = 2
    for i in range(len(tiles) + skew):
        if i < len(tiles):
            emit_front(i)
        if i >= skew:
            emit_back(i - skew)


def _emit_pass_f32eo(nc, pools, x_d, o_d, C, H, W, cc, rpp, in_f32):
    """One erosion pass, fp32 compute with even/odd-shared V pass.

    x_d fp32 (first pass) or bf16 (chained); o_d bf16. All mins on DVE in
    fp32 (bf16 ALU is slower on real HW); V pass uses the pairwise-sharing
    decomposition (1.5 ops/elem): p[j] = min(r[2j], r[2j+1]);
    v[2j] = min(r[2j-1], p[j]); v[2j+1] = min(p[j], r[2j+2]). Row-stepped
    slices keep unit innermost stride (full DVE rate). The final H op
    writes bf16 (free output cast).
    """
    ppi = H // rpp
    ips = max(1, 128 // ppi)
    inp, xbp, bnd, vtm, vt, htm, obp = pools
    R = rpp
    PW = cc + 2
    assert R % 2 == 0

    tiles = [(s0, c0) for s0 in range(0, C, ips) for c0 in range(0, W, cc)]
    front = {}

    def emit_front(i):
        s0, c0 = tiles[i]
        n_img = min(ips, C - s0)
        P = n_img * ppi
        last = c0 + cc == W
        # load tile: fp32 on first pass, bf16 on chained passes; compute
        # stays fp32 either way (bf16 load is upconverted by ACT)
        t = inp.tile([128, R, PW], F32, name="t")
        if in_f32:
            ld = t
        else:
            ld = xbp.tile([128, R, PW], BF16, name="xb")
        wlo = max(c0 - 1, 0)
        whi = min(c0 + cc + 1, W)
        dlo = 1 if c0 == 0 else 0
        for im in range(n_img):
            src = x_d[s0 + im, :, wlo:whi].rearrange("(p r) w -> p r w", p=ppi)
            p0 = im * ppi
            nc.sync.dma_start(ld[p0 : p0 + ppi, :, dlo : dlo + (whi - wlo)], src)
        if c0 == 0:
            nc.gpsimd.memset(ld[0:P, :, 0:1], PAD)
        if last:
            nc.gpsimd.memset(ld[0:P, :, PW - 1 : PW], PAD)
        if not in_f32:
            nc.scalar.copy(t[0:P], ld[0:P])
        front[i] = (t, n_img, P, s0, c0)

    def emit_back(i):
        t, n_img, P, s0, c0 = front.pop(i)
        bt = bnd.tile([128, 2, PW], F32, name="bt")
        for im in range(n_img):
            p0 = im * ppi
            pe = p0 + ppi - 1
            nc.sync.dma_start(bt[p0:pe, 0:1, :], t[p0 + 1 : pe + 1, 0:1, :])
            nc.sync.dma_start(
                bt[p0 + 1 : pe + 1, 1:2, :], t[p0:pe, R - 1 : R, :]
            )
            nc.sync.dma_start(
                bt[pe : pe + 1, 0:1, :], t[pe : pe + 1, R - 1 : R, :]
            )
            nc.gpsimd.memset(bt[p0 : p0 + 1, 1:2, :], PAD)

        # ---- V pass, even/odd shared (fp32) ----
        v = vt.tile([128, R, PW], F32, name="v")
        p = vtm.tile([128, R // 2, PW], F32, name="p")
        Rh = R // 2
        nc.vector.tensor_tensor(
            out=p[0:P], in0=t[0:P, 0:R:2, :], in1=t[0:P, 1:R:2, :], op=MIN
        )
        # v[2j] = min(r[2j-1], p[j]), j=1..Rh-1 (j=0 uses bt row-above)
        nc.vector.tensor_tensor(
            out=v[0:P, 2:R:2, :],
            in0=t[0:P, 1 : R - 2 : 2, :],
            in1=p[0:P, 1:Rh, :],
            op=MIN,
        )
        nc.vector.tensor_tensor(
            out=v[0:P, 0:1, :], in0=p[0:P, 0:1, :], in1=bt[0:P, 1:2, :], op=MIN
        )
        # v[2j+1] = min(p[j], r[2j+2]), j=0..Rh-2 (j=Rh-1 uses bt row-below)
        nc.vector.tensor_tensor(
            out=v[0:P, 1 : R - 1 : 2, :],
            in0=p[0:P, 0 : Rh - 1, :],
            in1=t[0:P, 2:R:2, :],
            op=MIN,
        )
        nc.vector.tensor_tensor(
            out=v[0:P, R - 1 : R, :],
            in0=p[0:P, Rh - 1 : Rh, :],
            in1=bt[0:P, 0:1, :],
            op=MIN,
        )

        # ---- H pass (fp32, last op casts to bf16 on output) ----
        h = htm.tile([128, R, cc + 1], F32, name="h")
        nc.vector.tensor_tensor(
            out=h[0:P], in0=v[0:P, :, 0 : cc + 1], in1=v[0:P, :, 1 : cc + 2],
            op=MIN,
        )
        ob = obp.tile([128, R, cc], BF16, name="ob")
        nc.vector.tensor_tensor(
            out=ob[0:P], in0=h[0:P, :, 0:cc], in1=v[0:P, :, 2 : cc + 2], op=MIN
        )

        for im in range(n_img):
            dst = o_d[s0 + im, :, c0 : c0 + cc].rearrange(
                "(p r) w -> p r w", p=ppi
            )
            p0 = im * ppi
            nc.scalar.dma_start(dst, ob[p0 : p0 + ppi, :, :])

    skew = 2
    for i in range(len(tiles) + skew):
        if i < len(tiles):
            emit_front(i)
        if i >= skew:
            emit_back(i - skew)


def _emit_pass_bf16a(nc, pools, x_d, o_d, C, H, W, cc, rpp, in_f32):
    """One erosion pass, all-bf16 DVE compute with every tensor_tensor
    operand 4B-aligned so the DVE runs at the 2x_1p rate throughout.

    The 2x_1p perf mode needs 16-bit dtype, unit innermost stride AND a
    4-byte-aligned start address for every operand. The H pass combines
    columns of both parities, so one op would always have a 2-byte-offset
    (odd) operand and silently drop to 1x. Fix: the otherwise-idle ACT
    engine makes a one-column-shifted copy vs[c] = v[c+1] (ACT is 1x at
    1.2 GHz regardless of alignment), and the DVE computes
        g[c]  = min(v[c], v[c+2])   (both even offsets, 2x)
        ob[c] = min(g[c], vs[c])    (both aligned, 2x)
    V pass is the even/odd-shared decomposition (1.5 ops/elem, all row
    slices start at even column 0 -> aligned, 2x). Per-partition boundary
    rows come from a small side tile bt filled by partition-shifted
    SBUF->SBUF DMAs; image edges use own-row duplication (min-idempotent)
    so no memsets on partition starts other than 0 are needed.

    DVE ~1.75 cyc/elem (vs 3.5 at 1x fp32), ACT 1 copy/elem (+1 convert
    on the first pass), DMA loads R rows/partition (no halo rows).
    """
    ppi = H // rpp
    ips = max(1, 128 // ppi)
    t32p, tp, bnd, pp, vt, vsp, gp, obp = pools
    R = rpp
    Rh = R // 2
    PW = cc + 2
    assert R % 2 == 0 and cc % 2 == 0 and PW % 2 == 0

    tiles = [(s0, c0) for s0 in range(0, C, ips) for c0 in range(0, W, cc)]
    front = {}
    mid = {}

    def emit_front(i):
        s0, c0 = tiles[i]
        n_img = min(ips, C - s0)
        P = n_img * ppi
        last = c0 + cc == W
        t = tp.tile([128, R, PW], BF16, name="t")
        ld = t32p.tile([128, R, PW], F32, name="t32") if in_f32 else t
        wlo = max(c0 - 1, 0)
        whi = min(c0 + cc + 1, W)
        dlo = 1 if c0 == 0 else 0
        for im in range(n_img):
            src = x_d[s0 + im, :, wlo:whi].rearrange("(p r) w -> p r w", p=ppi)
            p0 = im * ppi
            nc.sync.dma_start(ld[p0 : p0 + ppi, :, dlo : dlo + (whi - wlo)], src)
        if c0 == 0:
            nc.gpsimd.memset(ld[0:P, :, 0:1], PAD)
        if last:
            nc.gpsimd.memset(ld[0:P, :, PW - 1 : PW], PAD)
        if in_f32:
            nc.scalar.copy(t[0:P], ld[0:P])
        front[i] = (t, n_img, P, s0, c0)

    def emit_mid(i):
        t, n_img, P, s0, c0 = front.pop(i)
        # bt[p,0] = row below the block (next partition's row 0),
        # bt[p,1] = row above (prev partition's row R-1); image edges
        # duplicate the block's own edge row (min-idempotent).
        bt = bnd.tile([128, 2, PW], BF16, name="bt")
        for im in range(n_img):
            p0 = im * ppi
            pe = p0 + ppi - 1
            nc.sync.dma_start(bt[p0:pe, 0:1, :], t[p0 + 1 : pe + 1, 0:1, :])
            nc.sync.dma_start(bt[pe : pe + 1, 0:1, :], t[pe : pe + 1, R - 1 : R, :])
            nc.sync.dma_start(bt[p0 + 1 : pe + 1, 1:2, :], t[p0:pe, R - 1 : R, :])
            nc.sync.dma_start(bt[p0 : p0 + 1, 1:2, :], t[p0 : p0 + 1, 0:1, :])

        # ---- V pass (bf16, all operands 4B-aligned -> 2x) ----
        p = pp.tile([128, Rh, PW], BF16, name="p")
        v = vt.tile([128, R, PW], BF16, name="v")
        nc.vector.tensor_tensor(
            out=p[0:P], in0=t[0:P, 0:R:2, :], in1=t[0:P, 1:R:2, :], op=MIN
        )
        if Rh > 1:
            nc.vector.tensor_tensor(
                out=v[0:P, 2:R:2, :],
                in0=t[0:P, 1 : R - 2 : 2, :],
                in1=p[0:P, 1:Rh, :],
                op=MIN,
            )
            nc.vector.tensor_tensor(
                out=v[0:P, 1 : R - 1 : 2, :],
                in0=p[0:P, 0 : Rh - 1, :],
                in1=t[0:P, 2:R:2, :],
                op=MIN,
            )
        nc.vector.tensor_tensor(
            out=v[0:P, 0:1, :], in0=p[0:P, 0:1, :], in1=bt[0:P, 1:2, :], op=MIN
        )
        nc.vector.tensor_tensor(
            out=v[0:P, R - 1 : R, :],
            in0=p[0:P, Rh - 1 : Rh, :],
            in1=bt[0:P, 0:1, :],
            op=MIN,
        )
        # ACT: parity-fixing shifted copy (1x, alignment-agnostic)
        vs = vsp.tile([128, R, cc], BF16, name="vs")
        nc.scalar.copy(vs[0:P], v[0:P, :, 1 : cc + 1])
        mid[i] = (v, vs, n_img, P, s0, c0)

    def emit_tail(i):
        v, vs, n_img, P, s0, c0 = mid.pop(i)
        g = gp.tile([128, R, cc], BF16, name="g")
        nc.vector.tensor_tensor(
            out=g[0:P], in0=v[0:P, :, 0:cc], in1=v[0:P, :, 2 : cc + 2], op=MIN
        )
        ob = obp.tile([128, R, cc], BF16, name="ob")
        nc.vector.tensor_tensor(out=ob[0:P], in0=g[0:P], in1=vs[0:P], op=MIN)
        for im in range(n_img):
            dst = o_d[s0 + im, :, c0 : c0 + cc].rearrange(
                "(p r) w -> p r w", p=ppi
            )
            p0 = im * ppi
            nc.scalar.dma_start(dst, ob[p0 : p0 + ppi, :, :])

    # 3-stage software pipeline: tail(i) is emitted after mid(i+1), so the
    # DVE's ob(i) lands behind V(i+1) in its in-order stream -- by then the
    # ACT's vs(i) is long done and the DVE never stalls on the ACT.
    n = len(tiles)
    for i in range(n + 2):
        if i < n:
            emit_front(i)
        if 1 <= i <= n:
            emit_mid(i - 1)
        if i >= 2:
            emit_tail(i - 2)


def _emit_pass(nc, pools, x_d, o_d, C, H, W, cc, rpp, in_f32):
    """Emit one full erosion pass x_d -> o_d into the open TileContext.

    in_f32: x_d is fp32 and must be converted to bf16 on ACT; otherwise
    x_d is bf16 and is used directly. o_d is always bf16.
    """
    ppi = H // rpp  # partitions per image
    ips = max(1, 128 // ppi)  # images per partition-stack
    inp, xbp, bnd, vtm, vt, htm, obp = pools
    R = rpp
    PW = cc + 2  # padded tile width

    tiles = [(s0, c0) for s0 in range(0, C, ips) for c0 in range(0, W, cc)]
    front = {}

    def emit_front(i):
        s0, c0 = tiles[i]
        n_img = min(ips, C - s0)
        P = n_img * ppi
        last = c0 + cc == W
        t = inp.tile([128, R, PW], F32, name="t") if in_f32 else None
        xb = xbp.tile([128, R, PW], BF16, name="xb")
        ld = t if in_f32 else xb
        wlo = max(c0 - 1, 0)
        whi = min(c0 + cc + 1, W)
        dlo = 1 if c0 == 0 else 0
        for im in range(n_img):
            src = x_d[s0 + im, :, wlo:whi].rearrange("(p r) w -> p r w", p=ppi)
            p0 = im * ppi
            nc.sync.dma_start(ld[p0 : p0 + ppi, :, dlo : dlo + (whi - wlo)], src)
        if c0 == 0:
            nc.gpsimd.memset(ld[0:P, :, 0:1], PAD)
        if last:
            nc.gpsimd.memset(ld[0:P, :, PW - 1 : PW], PAD)
        if in_f32:
            nc.scalar.copy(xb[0:P], t[0:P])
        front[i] = (xb, n_img, P, s0, c0)

    def emit_back(i):
        xb, n_img, P, s0, c0 = front.pop(i)
        # boundary-row side tile (bf16): bt[p,0] = first row of the block
        # below (xb[p+1] row 0), bt[p,1] = last row of the block above
        # (xb[p-1] row R-1); image-edge partitions: top -> PAD memset
        # (start partition p0 is 0 mod ppi>=32: legal), bottom -> own-row
        # duplication DMA (min-idempotent; DMA has no start-partition rule).
        bt = bnd.tile([128, 2, PW], BF16)
        for im in range(n_img):
            p0 = im * ppi
            pe = p0 + ppi - 1  # last partition of this image
            nc.sync.dma_start(bt[p0:pe, 0:1, :], xb[p0 + 1 : pe + 1, 0:1, :])
            nc.sync.dma_start(
                bt[p0 + 1 : pe + 1, 1:2, :], xb[p0:pe, R - 1 : R, :]
            )
            nc.sync.dma_start(
                bt[pe : pe + 1, 0:1, :], xb[pe : pe + 1, R - 1 : R, :]
            )
            nc.gpsimd.memset(bt[p0 : p0 + 1, 1:2, :], PAD)

        # ---- V pass (bf16, 2x DVE): v[r] = min(row r-1, r, r+1) ----
        v = vt.tile([128, R, PW], BF16)
        tmp = vtm.tile([128, R - 1, PW], BF16)
        nc.vector.tensor_tensor(
            out=tmp[0:P], in0=xb[0:P, 0 : R - 1, :], in1=xb[0:P, 1:R, :], op=MIN
        )
        nc.vector.tensor_tensor(
            out=v[0:P, 1 : R - 1, :],
            in0=tmp[0:P, 0 : R - 2, :],
            in1=xb[0:P, 2:R, :],
            op=MIN,
        )
        nc.vector.tensor_tensor(
            out=v[0:P, 0:1, :], in0=tmp[0:P, 0:1, :], in1=bt[0:P, 1:2, :], op=MIN
        )
        nc.vector.tensor_tensor(
            out=v[0:P, R - 1 : R, :],
            in0=tmp[0:P, R - 2 : R - 1, :],
            in1=bt[0:P, 0:1, :],
            op=MIN,
        )

        # ---- H pass (bf16, 2x DVE): o[c] = min(v[c], v[c+1], v[c+2]) ----
        h = htm.tile([128, R, cc + 1], BF16)
        nc.vector.tensor_tensor(
            out=h[0:P], in0=v[0:P, :, 0 : cc + 1], in1=v[0:P, :, 1 : cc + 2],
            op=MIN,
        )
        ob = obp.tile([128, R, cc], BF16)
        nc.vector.tensor_tensor(
            out=ob[0:P], in0=h[0:P, :, 0:cc], in1=v[0:P, :, 2 : cc + 2], op=MIN
        )

        # store bf16 from ACT's queue (host widens to fp32 after gather)
        for im in range(n_img):
            dst = o_d[s0 + im, :, c0 : c0 + cc].rearrange(
                "(p r) w -> p r w", p=ppi
            )
            p0 = im * ppi
            nc.scalar.dma_start(dst, ob[p0 : p0 + ppi, :, :])

    # software-pipelined emission: tile i+skew's load/convert lands in
    # every queue before tile i's compute/store, so ACT's in-order
    # sequencer never delays DVE's next tile.
    skew = 2
    for i in range(len(tiles) + skew):
        if i < len(tiles):
            emit_front(i)
        if i >= skew:
            emit_back(i - skew)


def build_erosion(C, H, W, cc=None, rpp=RPP, reps=1, bufs=None, mode="f32eo"):
    """Per-core Bass program: x [C,H,W] f32 -> o [C,H,W] bf16, erosion^reps."""
    if cc is None:
        # f32eo's fp32 tiles need cc=512 to amortize per-op overhead while
        # fitting SBUF; chained (reps>1) builds add a bf16 load pool, so
        # drop to 256 columns.
        cc = 512 if (mode.startswith("f32eo") and reps == 1) else 256
    assert H % rpp == 0
    ppi = H // rpp
    assert ppi <= 128 and W % cc == 0

    nc = bacc.Bacc("TRN2", target_bir_lowering=False, debug=False, num_devices=1)
    x_d = nc.dram_tensor("x", [C, H, W], F32, kind="ExternalInput").ap()
    o_d = nc.dram_tensor("o", [C, H, W], BF16, kind="ExternalOutput").ap()
    # ping-pong DRAM scratch (bf16) for chained passes
    s_d = [
        nc.dram_tensor(f"scratch{i}", [C, H, W], BF16, kind="Internal").ap()
        for i in range(min(2, max(0, reps - 1)))
    ]

    def stage(i):
        src = x_d if i == 0 else s_d[(i - 1) % 2]
        dst = o_d if i == reps - 1 else s_d[i % 2]
        return src, dst

    if mode.startswith("f32eo"):
        bf = {"inp": 2, "xb": 2, "bnd": 2, "vtm": 1, "vt": 1, "htm": 1, "ob": 2}
    else:
        bf = {"inp": 4, "xb": 3, "bnd": 2, "vtm": 1, "vt": 1, "htm": 1, "ob": 3}
    if bufs:
        bf.update(bufs)
    emit = {
        "f32eo": _emit_pass_f32eo,
        "f32eo2": _emit_pass_f32eo2,
        "bf16": _emit_pass,
    }[mode]
    with tile.TileContext(nc) as tc:
        with (
            tc.tile_pool(name="inp", bufs=bf["inp"]) as inp,
            tc.tile_pool(name="xb", bufs=bf["xb"]) as xbp,
            tc.tile_pool(name="bnd", bufs=bf["bnd"]) as bnd,
            tc.tile_pool(name="vtm", bufs=bf["vtm"]) as vtm,
            tc.tile_pool(name="vt", bufs=bf["vt"]) as vt,
            tc.tile_pool(name="htm", bufs=bf["htm"]) as htm,
            tc.tile_pool(name="ob", bufs=bf["ob"]) as obp,
        ):
            pools = (inp, xbp, bnd, vtm, vt, htm, obp)
            for i in range(reps):
                src, dst = stage(i)
                emit(nc, pools, src, dst, C, H, W, cc, rpp, in_f32=(i == 0))
    nc.compile()
    return nc


def build_erosion_bf16a(
    C, H, W, cc_first=256, cc_chain=256, rpp=RPP, reps=1, bufs=None
):
    """bf16a-mode program: aligned bf16 DVE compute + ACT parity-fix copy.

    The fp32 first pass uses cc_first columns (the fp32 load tile is the
    SBUF hog); chained bf16 passes use cc_chain. Tile pools size slots to
    the max across passes. cc=256 keeps first+chained pools within the
    ~208 KiB/partition SBUF budget (~137 KiB used).
    """
    assert H % rpp == 0
    ppi = H // rpp
    assert ppi <= 128 and W % cc_first == 0 and W % cc_chain == 0

    nc = bacc.Bacc("TRN2", target_bir_lowering=False, debug=False, num_devices=1)
    x_d = nc.dram_tensor("x", [C, H, W], F32, kind="ExternalInput").ap()
    o_d = nc.dram_tensor("o", [C, H, W], BF16, kind="ExternalOutput").ap()
    s_d = [
        nc.dram_tensor(f"scratch{i}", [C, H, W], BF16, kind="Internal").ap()
        for i in range(min(2, max(0, reps - 1)))
    ]

    def stage(i):
        src = x_d if i == 0 else s_d[(i - 1) % 2]
        dst = o_d if i == reps - 1 else s_d[i % 2]
        return src, dst

    bf = {"t32": 3, "t": 2, "bnd": 2, "p": 1, "v": 2, "vs": 2, "g": 1, "ob": 3}
    if bufs:
        bf.update(bufs)
    with tile.TileContext(nc) as tc:
        with (
            tc.tile_pool(name="t32", bufs=bf["t32"]) as t32p,
            tc.tile_pool(name="t", bufs=bf["t"]) as tp,
            tc.tile_pool(name="bnd", bufs=bf["bnd"]) as bnd,
            tc.tile_pool(name="p", bufs=bf["p"]) as pp,
            tc.tile_pool(name="v", bufs=bf["v"]) as vt,
            tc.tile_pool(name="vs", bufs=bf["vs"]) as vsp,
            tc.tile_pool(name="g", bufs=bf["g"]) as gp,
            tc.tile_pool(name="ob", bufs=bf["ob"]) as obp,
        ):
            pools = (t32p, tp, bnd, pp, vt, vsp, gp, obp)
            for i in range(reps):
                src, dst = stage(i)
                cc = cc_first if i == 0 else cc_chain
                _emit_pass_bf16a(
                    nc, pools, src, dst, C, H, W, cc, rpp, in_f32=(i == 0)
                )
    nc.compile()
    return nc


def _get_program(C, H, W, reps=1, mode="bf16a"):
    key = (C, H, W, reps, mode)
    if key not in _cache:
        if mode == "bf16a":
            _cache[key] = build_erosion_bf16a(C, H, W, reps=reps)
        else:
            _cache[key] = build_erosion(C, H, W, reps=reps, mode=mode)
    return _cache[key]


def kernel(x, m):
    from concourse.bass_utils import run_bass_kernel_spmd

    m = int(np.asarray(m))
    x = np.ascontiguousarray(np.asarray(x), dtype=np.float32)
    B, C, H, W = x.shape
    if m <= 0:
        return x.copy()
    # erosion by a (2m+1)-square = m chained 3x3 erosion passes in one NEFF
    nc = _get_program(C, H, W, reps=m)
    n_cores = 8
    assert B == n_cores, f"expected batch {n_cores}, got {B}"
    in_maps = [{"x": x[b]} for b in range(n_cores)]
    res = run_bass_kernel_spmd(nc, in_maps, core_ids=list(range(n_cores)))
    # device output is bf16; widen to fp32 on the host
    return np.stack(
        [np.asarray(r["o"]).astype(np.float32) for r in res.results], axis=0
    )


if __name__ == "__main__":
    # small-scale CoreSim correctness check (no hardware needed)
    import ml_dtypes

    from concourse.bass_interp import CoreSim

    rng = np.random.default_rng(0)
    C, H, W = 2, 128, 64
    x = rng.standard_normal((C, H, W)).astype(np.float32)

    def bf16r(a):
        return a.astype(ml_dtypes.bfloat16).astype(np.float32)

    for reps, mode in ((1, "bf16a"), (2, "bf16a"), (3, "bf16a")):
        if mode == "bf16a":
            nc = build_erosion_bf16a(
                C, H, W, cc_first=16, cc_chain=32, rpp=4, reps=reps
            )
        else:
            nc = build_erosion(C, H, W, cc=32, rpp=4, reps=reps, mode=mode)
        sim = CoreSim(nc)
        sim.tensor("x")[:] = x
        sim.simulate(check_with_hw=False)
        got = np.asarray(sim.tensor("o")).astype(np.float32)

        # bf16a converts the input to bf16 before the first pass
        exp = bf16r(x) if mode in ("bf16", "bf16a") else x
        for _ in range(reps):
            xp = np.pad(exp, ((0, 0), (1, 1), (1, 1)), constant_values=PAD)
            nxt = np.empty_like(exp)
            for i in range(H):
                for j in range(W):
                    nxt[:, i, j] = xp[:, i : i + 3, j : j + 3].min(axis=(1, 2))
            exp = bf16r(nxt)  # device stores bf16 each pass
        ok = np.array_equal(got, exp)
        rel = np.max(np.abs(got - exp) / np.maximum(np.abs(exp), 1e-6))
        print(f"CoreSim reps={reps} mode={mode} exact: {ok} rel={rel:.2e}")



# revision 19
# speedup vs baseline: 3.2763x; 3.2763x over previous
"""Trainium2 Bass kernel for 3x3 (k=2m+1) morphological erosion (sliding-window
min) over [B, C, H, W] fp32, B=8 sharded across 8 NeuronCores (one batch per
core).

v3 scheme (per core, shard = one batch of C=8 channel images, 1024x1024):
  - each partition holds RPP=16 consecutive image rows (ppi=64 partitions per
    image, 2 images per 128-partition stack), processed in CC=256-column
    chunks with a 1-column halo (PW=258).
  - the separable 3x3 min runs in bf16 on DVE at the 2x_1p rate (2-byte
    dtype + unit innermost stride). min never creates new values, so the
    total error is one bf16 rounding of the input (~2^-9 relative), far
    inside the 2e-2 gate.
  - the device kernel STORES bf16 (halves store traffic: 64MB -> 48MB/core
    round trip); kernel() widens to fp32 on the host after the gather.
  - ACT (scalar) engine does the fp32->bf16 input convert (first pass only;
    chained passes read bf16 scratch directly) and drives store DMAs; loads
    and the per-partition-block boundary-row halo copies ride the SP queue;
    PAD memsets go to Pool. Emission is software-pipelined (skew 2) so no
    in-order sequencer blocks a neighbor engine's next tile.
  - V pass: tmp[j] = min(row j, row j+1); interior v rows from tmp + row
    j+2; the 2 per-partition boundary rows take their missing neighbor from
    a small bf16 side tile bt (partition-shifted SBUF->SBUF DMA; image-edge
    partitions: PAD memset at block tops (legal start partitions 0/64),
    own-row duplication DMA at block bottoms).
  - Cost model: DMA engines ~147us, DVE ~141us, ACT ~55us -> balanced
    DMA/DVE at the bf16-store memory roofline.
  - m>1 runs as m chained passes (bf16 DRAM ping-pong) inside one NEFF.
"""

import sys

sys.path.insert(0, "/opt/trn_rl_repo")

import numpy as np

import concourse.bass as bass
import concourse.tile as tile
from concourse import bacc, mybir

PAD = 1.0e9
F32 = mybir.dt.float32
BF16 = mybir.dt.bfloat16
MIN = mybir.AluOpType.min

CC = 256  # column chunk width
RPP = 16  # image rows per partition

_cache = {}


def _emit_pass_f32eo2(nc, pools, x_d, o_d, C, H, W, cc, rpp, in_f32):
    """One erosion pass, fp32 compute, even/odd V pass over halo-loaded rows.

    Each partition loads R+2 rows (its R rows plus one halo row each side,
    overlapping DRAM reads; +2/R load bytes) so the V pass is exactly 3
    row-stepped DVE ops with no boundary fixups:
      p[j] = min(e[2j], e[2j+1])          j = 0..R/2
      v[2j] = min(p[j], e[2j+2])          j = 0..R/2-1
      v[2j+1] = min(e[2j+1], p[j+1])      j = 0..R/2-1
    Image-edge partitions: top halo row = PAD memset (start partitions
    0/64: legal), bottom halo row = own-last-row duplication DMA
    (min-idempotent). All slices keep unit innermost stride (full DVE
    rate). The final H op writes bf16 (free output cast); host widens.
    """
    from concourse.ap import AP as _AP

    ppi = H // rpp
    ips = max(1, 128 // ppi)
    inp, xbp, bnd, vtm, vt, htm, obp = pools
    R = rpp
    E = R + 2  # rows incl halo
    PW = cc + 2
    assert R % 2 == 0

    tiles = [(s0, c0) for s0 in range(0, C, ips) for c0 in range(0, W, cc)]
    front = {}

    def emit_front(i):
        s0, c0 = tiles[i]
        n_img = min(ips, C - s0)
        P = n_img * ppi
        last = c0 + cc == W
        t = inp.tile([128, E, PW], F32, name="t")
        if in_f32:
            ld = t
        else:
            ld = xbp.tile([128, E, PW], BF16, name="xb")
        wlo = max(c0 - 1, 0)
        whi = min(c0 + cc + 1, W)
        nw = whi - wlo
        dlo = 1 if c0 == 0 else 0
        for im in range(n_img):
            img = x_d[s0 + im]  # [H, W]
            p0 = im * ppi
            # interior partitions 1..ppi-2: rows rpp*p-1 .. rpp*p+rpp
            src = _AP(
                tensor=img.tensor,
                offset=img.offset + (rpp - 1) * W + wlo,
                ap=[[rpp * W, ppi - 2], [W, E], [1, nw]],
            )
            nc.sync.dma_start(
                ld[p0 + 1 : p0 + ppi - 1, :, dlo : dlo + nw], src
            )
            # top partition: rows 0..rpp into slots 1..E-1; slot 0 = PAD
            nc.sync.dma_start(
                ld[p0 : p0 + 1, 1:E, dlo : dlo + nw],
                img[0 : rpp + 1, wlo:whi].rearrange("(p r) w -> p r w", p=1),
            )
            nc.gpsimd.memset(ld[p0 : p0 + 1, 0:1, :], PAD)
            # bottom partition: rows H-rpp-1..H-1 into slots 0..E-2;
            # slot E-1 = duplicate of the image's last row (min-idempotent)
            pe = p0 + ppi - 1
            nc.sync.dma_start(
                ld[pe : pe + 1, 0 : E - 1, dlo : dlo + nw],
                img[H - rpp - 1 : H, wlo:whi].rearrange("(p r) w -> p r w", p=1),
            )
            nc.sync.dma_start(
                ld[pe : pe + 1, E - 1 : E, dlo : dlo + nw],
                img[H - 1 : H, wlo:whi].rearrange("(p r) w -> p r w", p=1),
            )
        if c0 == 0:
            nc.gpsimd.memset(ld[0:P, :, 0:1], PAD)
        if last:
            nc.gpsimd.memset(ld[0:P, :, PW - 1 : PW], PAD)
        if not in_f32:
            nc.scalar.copy(t[0:P], ld[0:P])
        front[i] = (t, n_img, P, s0, c0)

    def emit_back(i):
        t, n_img, P, s0, c0 = front.pop(i)
        Rh = R // 2
        v = vt.tile([128, R, PW], F32, name="v")
        p = vtm.tile([128, Rh + 1, PW], F32, name="p")
        nc.vector.tensor_tensor(
            out=p[0:P], in0=t[0:P, 0:E:2, :], in1=t[0:P, 1:E:2, :], op=MIN
        )
        nc.vector.tensor_tensor(
            out=v[0:P, 0:R:2, :],
            in0=p[0:P, 0:Rh, :],
            in1=t[0:P, 2:E:2, :],
            op=MIN,
        )
        nc.vector.tensor_tensor(
            out=v[0:P, 1:R:2, :],
            in0=t[0:P, 1 : E - 2 : 2, :],
            in1=p[0:P, 1 : Rh + 1, :],
            op=MIN,
        )

        h = htm.tile([128, R, cc + 1], F32, name="h")
        nc.vector.tensor_tensor(
            out=h[0:P], in0=v[0:P, :, 0 : cc + 1], in1=v[0:P, :, 1 : cc + 2],
            op=MIN,
        )
        ob = obp.tile([128, R, cc], BF16, name="ob")
        nc.vector.tensor_tensor(
            out=ob[0:P], in0=h[0:P, :, 0:cc], in1=v[0:P, :, 2 : cc + 2], op=MIN
        )

        for im in range(n_img):
            dst = o_d[s0 + im, :, c0 : c0 + cc].rearrange(
                "(p r) w -> p r w", p=ppi
            )
            p0 = im * ppi
            nc.scalar.dma_start(dst, ob[p0 : p0 + ppi, :, :])

    skew = 2
    for i in range(len(tiles) + skew):
        if i < len(tiles):
            emit_front(i)
        if i >= skew:
            emit_back(i - skew)


def _emit_pass_f32eo(nc, pools, x_d, o_d, C, H, W, cc, rpp, in_f32):
    """One erosion pass, fp32 compute with even/odd-shared V pass.

    x_d fp32 (first pass) or bf16 (chained); o_d bf16. All mins on DVE in
    fp32 (bf16 ALU is slower on real HW); V pass uses the pairwise-sharing
    decomposition (1.5 ops/elem): p[j] = min(r[2j], r[2j+1]);
    v[2j] = min(r[2j-1], p[j]); v[2j+1] = min(p[j], r[2j+2]). Row-stepped
    slices keep unit innermost stride (full DVE rate). The final H op
    writes bf16 (free output cast).
    """
    ppi = H // rpp
    ips = max(1, 128 // ppi)
    inp, xbp, bnd, vtm, vt, htm, obp = pools
    R = rpp
    PW = cc + 2
    assert R % 2 == 0

    tiles = [(s0, c0) for s0 in range(0, C, ips) for c0 in range(0, W, cc)]
    front = {}

    def emit_front(i):
        s0, c0 = tiles[i]
        n_img = min(ips, C - s0)
        P = n_img * ppi
        last = c0 + cc == W
        # load tile: fp32 on first pass, bf16 on chained passes; compute
        # stays fp32 either way (bf16 load is upconverted by ACT)
        t = inp.tile([128, R, PW], F32, name="t")
        if in_f32:
            ld = t
        else:
            ld = xbp.tile([128, R, PW], BF16, name="xb")
        wlo = max(c0 - 1, 0)
        whi = min(c0 + cc + 1, W)
        dlo = 1 if c0 == 0 else 0
        for im in range(n_img):
            src = x_d[s0 + im, :, wlo:whi].rearrange("(p r) w -> p r w", p=ppi)
            p0 = im * ppi
            nc.sync.dma_start(ld[p0 : p0 + ppi, :, dlo : dlo + (whi - wlo)], src)
        if c0 == 0:
            nc.gpsimd.memset(ld[0:P, :, 0:1], PAD)
        if last:
            nc.gpsimd.memset(ld[0:P, :, PW - 1 : PW], PAD)
        if not in_f32:
            nc.scalar.copy(t[0:P], ld[0:P])
        front[i] = (t, n_img, P, s0, c0)

    def emit_back(i):
        t, n_img, P, s0, c0 = front.pop(i)
        bt = bnd.tile([128, 2, PW], F32, name="bt")
        for im in range(n_img):
            p0 = im * ppi
            pe = p0 + ppi - 1
            nc.sync.dma_start(bt[p0:pe, 0:1, :], t[p0 + 1 : pe + 1, 0:1, :])
            nc.sync.dma_start(
                bt[p0 + 1 : pe + 1, 1:2, :], t[p0:pe, R - 1 : R, :]
            )
            nc.sync.dma_start(
                bt[pe : pe + 1, 0:1, :], t[pe : pe + 1, R - 1 : R, :]
            )
            nc.gpsimd.memset(bt[p0 : p0 + 1, 1:2, :], PAD)

        # ---- V pass, even/odd shared (fp32) ----
        v = vt.tile([128, R, PW], F32, name="v")
        p = vtm.tile([128, R // 2, PW], F32, name="p")
        Rh = R // 2
        nc.vector.tensor_tensor(
            out=p[0:P], in0=t[0:P, 0:R:2, :], in1=t[0:P, 1:R:2, :], op=MIN
        )
        # v[2j] = min(r[2j-1], p[j]), j=1..Rh-1 (j=0 uses bt row-above)
        nc.vector.tensor_tensor(
            out=v[0:P, 2:R:2, :],
            in0=t[0:P, 1 : R - 2 : 2, :],
            in1=p[0:P, 1:Rh, :],
            op=MIN,
        )
        nc.vector.tensor_tensor(
            out=v[0:P, 0:1, :], in0=p[0:P, 0:1, :], in1=bt[0:P, 1:2, :], op=MIN
        )
        # v[2j+1] = min(p[j], r[2j+2]), j=0..Rh-2 (j=Rh-1 uses bt row-below)
        nc.vector.tensor_tensor(
            out=v[0:P, 1 : R - 1 : 2, :],
            in0=p[0:P, 0 : Rh - 1, :],
            in1=t[0:P, 2:R:2, :],
            op=MIN,
        )
        nc.vector.tensor_tensor(
            out=v[0:P, R - 1 : R, :],
            in0=p[0:P, Rh - 1 : Rh, :],
            in1=bt[0:P, 0:1, :],
            op=MIN,
        )

        # ---- H pass (fp32, last op casts to bf16 on output) ----
        h = htm.tile([128, R, cc + 1], F32, name="h")
        nc.vector.tensor_tensor(
            out=h[0:P], in0=v[0:P, :, 0 : cc + 1], in1=v[0:P, :, 1 : cc + 2],
            op=MIN,
        )
        ob = obp.tile([128, R, cc], BF16, name="ob")
        nc.vector.tensor_tensor(
            out=ob[0:P], in0=h[0:P, :, 0:cc], in1=v[0:P, :, 2 : cc + 2], op=MIN
        )

        for im in range(n_img):
            dst = o_d[s0 + im, :, c0 : c0 + cc].rearrange(
                "(p r) w -> p r w", p=ppi
            )
            p0 = im * ppi
            nc.scalar.dma_start(dst, ob[p0 : p0 + ppi, :, :])

    skew = 2
    for i in range(len(tiles) + skew):
        if i < len(tiles):
            emit_front(i)
        if i >= skew:
            emit_back(i - skew)


def _emit_pass_cg(nc, pools, x_d, o_d, C, H, W, in_f32):
    """One erosion pass, contiguous-big-DMA layout + bf16 DVE compute.

    HW measurements drove this shape:
      - DVE TT min bf16 = 0.372 ns/elem-pp (0.415 odd-offset) vs fp32 1.097
        -> all mins in bf16.
      - DMA throughput is descriptor-dominated: the old [ppi,R,~1KB-line]
        strided loads measured ~45 GB/s; >=16KB contiguous-per-partition
        descriptors reach ~350-400 GB/s.
    So: one image per 128-partition stack, partition p holds R=H/128 FULL
    rows (contiguous 8KB-32KB span in DRAM and in SBUF) -> ONE descriptor
    per partition per image load/store. Per image per pass: 1 big load
    (sync queue), 2 partition-shifted SBUF->SBUF row copies for the V-pass
    boundary rows (scalar queue), 1 big store (scalar queue). Image-edge
    partitions get their boundary row via tiny 1-partition DVE ops reading
    the tile's own edge row (min-idempotent duplicate) -- no memsets, no
    PAD columns: the H-pass edge columns use dedicated 1-column ops.
    """
    R = H // 128
    Rh = R // 2
    assert H % 128 == 0 and R % 2 == 0
    t32p, tp, bnd, pp, vt, _vsp, hp, obp = pools
    front = {}

    def emit_front(i):
        t = tp.tile([128, R, W], BF16, name="t")
        ld = t32p.tile([128, R, W], F32, name="t32") if in_f32 else t
        src = x_d[i].rearrange("(p r) w -> p (r w)", p=128)
        nc.sync.dma_start(ld.rearrange("p r w -> p (r w)"), src)
        if in_f32:
            nc.scalar.copy(t, ld)
        # bt[p,0] = row above (prev partition's last row),
        # bt[p,1] = row below (next partition's first row)
        bt = bnd.tile([128, 2, W], BF16, name="bt")
        nc.scalar.dma_start(bt[1:128, 0:1, :], t[0:127, R - 1 : R, :])
        nc.scalar.dma_start(bt[0:127, 1:2, :], t[1:128, 0:1, :])
        # image-edge partitions duplicate their own edge row (the DVE
        # rejects partition starts other than 0/32/64/96, so these are DMAs)
        nc.scalar.dma_start(bt[0:1, 0:1, :], t[0:1, 0:1, :])
        nc.scalar.dma_start(bt[127:128, 1:2, :], t[127:128, R - 1 : R, :])
        front[i] = (t, bt)

    def emit_back(i):
        t, bt = front.pop(i)
        # ---- V pass (bf16; all aligned) ----
        p = pp.tile([128, Rh, W], BF16, name="p")
        v = vt.tile([128, R, W], BF16, name="v")
        nc.vector.tensor_tensor(
            out=p, in0=t[:, 0:R:2, :], in1=t[:, 1:R:2, :], op=MIN
        )
        if Rh > 1:
            # v[2j] = min(r[2j-1], p[j]), j=1..Rh-1
            nc.vector.tensor_tensor(
                out=v[:, 2:R:2, :],
                in0=t[:, 1 : R - 2 : 2, :],
                in1=p[:, 1:Rh, :],
                op=MIN,
            )
            # v[2j+1] = min(p[j], r[2j+2]), j=0..Rh-2
            nc.vector.tensor_tensor(
                out=v[:, 1 : R - 1 : 2, :],
                in0=p[:, 0 : Rh - 1, :],
                in1=t[:, 2:R:2, :],
                op=MIN,
            )
        # v[0] = min(p[0], row-above); v[R-1] = min(p[Rh-1], row-below)
        nc.vector.tensor_tensor(
            out=v[:, 0:1, :], in0=p[:, 0:1, :], in1=bt[:, 0:1, :], op=MIN
        )
        nc.vector.tensor_tensor(
            out=v[:, R - 1 : R, :],
            in0=p[:, Rh - 1 : Rh, :],
            in1=bt[:, 1:2, :],
            op=MIN,
        )
        # ---- H pass: h[c] = min(v[c], v[c+1]); ob interior + edge cols ----
        h = hp.tile([128, R, W], BF16, name="h")
        nc.vector.tensor_tensor(
            out=h[:, :, 0 : W - 1],
            in0=v[:, :, 0 : W - 1],
            in1=v[:, :, 1:W],
            op=MIN,
        )
        ob = obp.tile([128, R, W], BF16, name="ob")
        nc.vector.tensor_tensor(
            out=ob[:, :, 1 : W - 1],
            in0=h[:, :, 0 : W - 2],
            in1=v[:, :, 2:W],
            op=MIN,
        )
        nc.vector.tensor_tensor(
            out=ob[:, :, 0:1], in0=h[:, :, 0:1], in1=v[:, :, 1:2], op=MIN
        )
        nc.vector.tensor_copy(
            out=ob[:, :, W - 1 : W], in_=h[:, :, W - 2 : W - 1]
        )
        dst = o_d[i].rearrange("(p r) w -> p (r w)", p=128)
        nc.scalar.dma_start(dst, ob.rearrange("p r w -> p (r w)"))

    skew = 2
    for i in range(C + skew):
        if i < C:
            emit_front(i)
        if i >= skew:
            emit_back(i - skew)


def _emit_pass_bf16b(nc, pools, x_d, o_d, C, H, W, cc, rpp, in_f32):
    """One erosion pass, pure-DVE bf16 compute, halo-row loads (f32eo2
    style), no ACT compute and no SBUF->SBUF boundary DMAs.

    Measured on HW (microbench): TT min bf16 = 0.372 ns/elem-per-partition
    when 4B-aligned and 0.415 when one operand starts at an odd column --
    both ~2.9x faster than fp32 (1.097). So compute everything in bf16 on
    the DVE: V pass = even/odd shared decomposition over the halo-loaded
    rows (all aligned), H pass = h(odd operand) then ob (aligned). The
    only ACT work is the fp32->bf16 convert on the first pass; stores
    ride the ACT queue as plain triggers.
    """
    from concourse.ap import AP as _AP

    ppi = H // rpp
    ips = max(1, 128 // ppi)
    t32p, tp, _bnd, pp, vt, _vsp, hp, obp = pools
    R = rpp
    Rh = R // 2
    E = R + 2
    PW = cc + 2
    assert R % 2 == 0 and cc % 2 == 0

    tiles = [(s0, c0) for s0 in range(0, C, ips) for c0 in range(0, W, cc)]
    front = {}

    def emit_front(i):
        s0, c0 = tiles[i]
        n_img = min(ips, C - s0)
        P = n_img * ppi
        last = c0 + cc == W
        t = tp.tile([128, E, PW], BF16, name="t")
        ld = t32p.tile([128, E, PW], F32, name="t32") if in_f32 else t
        wlo = max(c0 - 1, 0)
        whi = min(c0 + cc + 1, W)
        nw = whi - wlo
        dlo = 1 if c0 == 0 else 0
        for im in range(n_img):
            img = x_d[s0 + im]  # [H, W]
            p0 = im * ppi
            # interior partitions 1..ppi-2: rows rpp*p-1 .. rpp*p+rpp
            src = _AP(
                tensor=img.tensor,
                offset=img.offset + (rpp - 1) * W + wlo,
                ap=[[rpp * W, ppi - 2], [W, E], [1, nw]],
            )
            nc.sync.dma_start(ld[p0 + 1 : p0 + ppi - 1, :, dlo : dlo + nw], src)
            # top partition: rows 0..rpp into slots 1..E-1; slot 0 = row 0
            # again (min-idempotent duplicate stands in for the PAD row)
            nc.sync.dma_start(
                ld[p0 : p0 + 1, 1:E, dlo : dlo + nw],
                img[0 : rpp + 1, wlo:whi].rearrange("(p r) w -> p r w", p=1),
            )
            nc.sync.dma_start(
                ld[p0 : p0 + 1, 0:1, dlo : dlo + nw],
                img[0:1, wlo:whi].rearrange("(p r) w -> p r w", p=1),
            )
            # bottom partition: rows H-rpp-1..H-1 into slots 0..E-2;
            # slot E-1 = duplicate of the image's last row
            pe = p0 + ppi - 1
            nc.sync.dma_start(
                ld[pe : pe + 1, 0 : E - 1, dlo : dlo + nw],
                img[H - rpp - 1 : H, wlo:whi].rearrange("(p r) w -> p r w", p=1),
            )
            nc.sync.dma_start(
                ld[pe : pe + 1, E - 1 : E, dlo : dlo + nw],
                img[H - 1 : H, wlo:whi].rearrange("(p r) w -> p r w", p=1),
            )
        if c0 == 0:
            nc.gpsimd.memset(ld[0:P, :, 0:1], PAD)
        if last:
            nc.gpsimd.memset(ld[0:P, :, PW - 1 : PW], PAD)
        if in_f32:
            nc.scalar.copy(t[0:P], ld[0:P])
        front[i] = (t, n_img, P, s0, c0)

    def emit_back(i):
        t, n_img, P, s0, c0 = front.pop(i)
        # ---- V pass over halo rows (bf16, aligned -> fast) ----
        p = pp.tile([128, Rh + 1, PW], BF16, name="p")
        v = vt.tile([128, R, PW], BF16, name="v")
        nc.vector.tensor_tensor(
            out=p[0:P], in0=t[0:P, 0:E:2, :], in1=t[0:P, 1:E:2, :], op=MIN
        )
        nc.vector.tensor_tensor(
            out=v[0:P, 0:R:2, :],
            in0=p[0:P, 0:Rh, :],
            in1=t[0:P, 2:E:2, :],
            op=MIN,
        )
        nc.vector.tensor_tensor(
            out=v[0:P, 1:R:2, :],
            in0=t[0:P, 1 : E - 2 : 2, :],
            in1=p[0:P, 1 : Rh + 1, :],
            op=MIN,
        )
        # ---- H pass: h has one odd-offset operand (12% slower, fine) ----
        h = hp.tile([128, R, PW], BF16, name="h")
        nc.vector.tensor_tensor(
            out=h[0:P, :, 0 : cc + 1],
            in0=v[0:P, :, 0 : cc + 1],
            in1=v[0:P, :, 1 : cc + 2],
            op=MIN,
        )
        ob = obp.tile([128, R, cc], BF16, name="ob")
        nc.vector.tensor_tensor(
            out=ob[0:P], in0=h[0:P, :, 0:cc], in1=v[0:P, :, 2 : cc + 2], op=MIN
        )
        for im in range(n_img):
            dst = o_d[s0 + im, :, c0 : c0 + cc].rearrange(
                "(p r) w -> p r w", p=ppi
            )
            p0 = im * ppi
            nc.scalar.dma_start(dst, ob[p0 : p0 + ppi, :, :])

    skew = 2
    for i in range(len(tiles) + skew):
        if i < len(tiles):
            emit_front(i)
        if i >= skew:
            emit_back(i - skew)


def _emit_pass_bf16a(nc, pools, x_d, o_d, C, H, W, cc, rpp, in_f32):
    """One erosion pass, all-bf16 DVE compute with every tensor_tensor
    operand 4B-aligned so the DVE runs at the 2x_1p rate throughout.

    The 2x_1p perf mode needs 16-bit dtype, unit innermost stride AND a
    4-byte-aligned start address for every operand. The H pass combines
    columns of both parities, so one op would always have a 2-byte-offset
    (odd) operand and silently drop to 1x. Fix: the otherwise-idle ACT
    engine makes a one-column-shifted copy vs[c] = v[c+1] (ACT is 1x at
    1.2 GHz regardless of alignment), and the DVE computes
        g[c]  = min(v[c], v[c+2])   (both even offsets, 2x)
        ob[c] = min(g[c], vs[c])    (both aligned, 2x)
    V pass is the even/odd-shared decomposition (1.5 ops/elem, all row
    slices start at even column 0 -> aligned, 2x). Per-partition boundary
    rows come from a small side tile bt filled by partition-shifted
    SBUF->SBUF DMAs; image edges use own-row duplication (min-idempotent)
    so no memsets on partition starts other than 0 are needed.

    DVE ~1.75 cyc/elem (vs 3.5 at 1x fp32), ACT 1 copy/elem (+1 convert
    on the first pass), DMA loads R rows/partition (no halo rows).
    """
    ppi = H // rpp
    ips = max(1, 128 // ppi)
    t32p, tp, bnd, pp, vt, vsp, gp, obp = pools
    R = rpp
    Rh = R // 2
    PW = cc + 2
    assert R % 2 == 0 and cc % 2 == 0 and PW % 2 == 0

    tiles = [(s0, c0) for s0 in range(0, C, ips) for c0 in range(0, W, cc)]
    front = {}
    mid = {}

    def emit_front(i):
        s0, c0 = tiles[i]
        n_img = min(ips, C - s0)
        P = n_img * ppi
        last = c0 + cc == W
        t = tp.tile([128, R, PW], BF16, name="t")
        ld = t32p.tile([128, R, PW], F32, name="t32") if in_f32 else t
        wlo = max(c0 - 1, 0)
        whi = min(c0 + cc + 1, W)
        dlo = 1 if c0 == 0 else 0
        for im in range(n_img):
            src = x_d[s0 + im, :, wlo:whi].rearrange("(p r) w -> p r w", p=ppi)
            p0 = im * ppi
            nc.sync.dma_start(ld[p0 : p0 + ppi, :, dlo : dlo + (whi - wlo)], src)
        if c0 == 0:
            nc.gpsimd.memset(ld[0:P, :, 0:1], PAD)
        if last:
            nc.gpsimd.memset(ld[0:P, :, PW - 1 : PW], PAD)
        if in_f32:
            nc.scalar.copy(t[0:P], ld[0:P])
        front[i] = (t, n_img, P, s0, c0)

    def emit_mid(i):
        t, n_img, P, s0, c0 = front.pop(i)
        # bt[p,0] = row below the block (next partition's row 0),
        # bt[p,1] = row above (prev partition's row R-1); image edges
        # duplicate the block's own edge row (min-idempotent).
        bt = bnd.tile([128, 2, PW], BF16, name="bt")
        for im in range(n_img):
            p0 = im * ppi
            pe = p0 + ppi - 1
            nc.sync.dma_start(bt[p0:pe, 0:1, :], t[p0 + 1 : pe + 1, 0:1, :])
            nc.sync.dma_start(bt[pe : pe + 1, 0:1, :], t[pe : pe + 1, R - 1 : R, :])
            nc.sync.dma_start(bt[p0 + 1 : pe + 1, 1:2, :], t[p0:pe, R - 1 : R, :])
            nc.sync.dma_start(bt[p0 : p0 + 1, 1:2, :], t[p0 : p0 + 1, 0:1, :])

        # ---- V pass (bf16, all operands 4B-aligned -> 2x) ----
        p = pp.tile([128, Rh, PW], BF16, name="p")
        v = vt.tile([128, R, PW], BF16, name="v")
        nc.vector.tensor_tensor(
            out=p[0:P], in0=t[0:P, 0:R:2, :], in1=t[0:P, 1:R:2, :], op=MIN
        )
        if Rh > 1:
            nc.vector.tensor_tensor(
                out=v[0:P, 2:R:2, :],
                in0=t[0:P, 1 : R - 2 : 2, :],
                in1=p[0:P, 1:Rh, :],
                op=MIN,
            )
            nc.vector.tensor_tensor(
                out=v[0:P, 1 : R - 1 : 2, :],
                in0=p[0:P, 0 : Rh - 1, :],
                in1=t[0:P, 2:R:2, :],
                op=MIN,
            )
        nc.vector.tensor_tensor(
            out=v[0:P, 0:1, :], in0=p[0:P, 0:1, :], in1=bt[0:P, 1:2, :], op=MIN
        )
        nc.vector.tensor_tensor(
            out=v[0:P, R - 1 : R, :],
            in0=p[0:P, Rh - 1 : Rh, :],
            in1=bt[0:P, 0:1, :],
            op=MIN,
        )
        # ACT: parity-fixing shifted copy (1x, alignment-agnostic)
        vs = vsp.tile([128, R, cc], BF16, name="vs")
        nc.scalar.copy(vs[0:P], v[0:P, :, 1 : cc + 1])
        mid[i] = (v, vs, n_img, P, s0, c0)

    def emit_tail(i):
        v, vs, n_img, P, s0, c0 = mid.pop(i)
        g = gp.tile([128, R, cc], BF16, name="g")
        nc.vector.tensor_tensor(
            out=g[0:P], in0=v[0:P, :, 0:cc], in1=v[0:P, :, 2 : cc + 2], op=MIN
        )
        ob = obp.tile([128, R, cc], BF16, name="ob")
        nc.vector.tensor_tensor(out=ob[0:P], in0=g[0:P], in1=vs[0:P], op=MIN)
        for im in range(n_img):
            dst = o_d[s0 + im, :, c0 : c0 + cc].rearrange(
                "(p r) w -> p r w", p=ppi
            )
            p0 = im * ppi
            nc.scalar.dma_start(dst, ob[p0 : p0 + ppi, :, :])

    # 3-stage software pipeline: tail(i) is emitted after mid(i+1), so the
    # DVE's ob(i) lands behind V(i+1) in its in-order stream -- by then the
    # ACT's vs(i) is long done and the DVE never stalls on the ACT.
    n = len(tiles)
    for i in range(n + 2):
        if i < n:
            emit_front(i)
        if 1 <= i <= n:
            emit_mid(i - 1)
        if i >= 2:
            emit_tail(i - 2)


def _emit_pass(nc, pools, x_d, o_d, C, H, W, cc, rpp, in_f32):
    """Emit one full erosion pass x_d -> o_d into the open TileContext.

    in_f32: x_d is fp32 and must be converted to bf16 on ACT; otherwise
    x_d is bf16 and is used directly. o_d is always bf16.
    """
    ppi = H // rpp  # partitions per image
    ips = max(1, 128 // ppi)  # images per partition-stack
    inp, xbp, bnd, vtm, vt, htm, obp = pools
    R = rpp
    PW = cc + 2  # padded tile width

    tiles = [(s0, c0) for s0 in range(0, C, ips) for c0 in range(0, W, cc)]
    front = {}

    def emit_front(i):
        s0, c0 = tiles[i]
        n_img = min(ips, C - s0)
        P = n_img * ppi
        last = c0 + cc == W
        t = inp.tile([128, R, PW], F32, name="t") if in_f32 else None
        xb = xbp.tile([128, R, PW], BF16, name="xb")
        ld = t if in_f32 else xb
        wlo = max(c0 - 1, 0)
        whi = min(c0 + cc + 1, W)
        dlo = 1 if c0 == 0 else 0
        for im in range(n_img):
            src = x_d[s0 + im, :, wlo:whi].rearrange("(p r) w -> p r w", p=ppi)
            p0 = im * ppi
            nc.sync.dma_start(ld[p0 : p0 + ppi, :, dlo : dlo + (whi - wlo)], src)
        if c0 == 0:
            nc.gpsimd.memset(ld[0:P, :, 0:1], PAD)
        if last:
            nc.gpsimd.memset(ld[0:P, :, PW - 1 : PW], PAD)
        if in_f32:
            nc.scalar.copy(xb[0:P], t[0:P])
        front[i] = (xb, n_img, P, s0, c0)

    def emit_back(i):
        xb, n_img, P, s0, c0 = front.pop(i)
        # boundary-row side tile (bf16): bt[p,0] = first row of the block
        # below (xb[p+1] row 0), bt[p,1] = last row of the block above
        # (xb[p-1] row R-1); image-edge partitions: top -> PAD memset
        # (start partition p0 is 0 mod ppi>=32: legal), bottom -> own-row
        # duplication DMA (min-idempotent; DMA has no start-partition rule).
        bt = bnd.tile([128, 2, PW], BF16)
        for im in range(n_img):
            p0 = im * ppi
            pe = p0 + ppi - 1  # last partition of this image
            nc.sync.dma_start(bt[p0:pe, 0:1, :], xb[p0 + 1 : pe + 1, 0:1, :])
            nc.sync.dma_start(
                bt[p0 + 1 : pe + 1, 1:2, :], xb[p0:pe, R - 1 : R, :]
            )
            nc.sync.dma_start(
                bt[pe : pe + 1, 0:1, :], xb[pe : pe + 1, R - 1 : R, :]
            )
            nc.gpsimd.memset(bt[p0 : p0 + 1, 1:2, :], PAD)

        # ---- V pass (bf16, 2x DVE): v[r] = min(row r-1, r, r+1) ----
        v = vt.tile([128, R, PW], BF16)
        tmp = vtm.tile([128, R - 1, PW], BF16)
        nc.vector.tensor_tensor(
            out=tmp[0:P], in0=xb[0:P, 0 : R - 1, :], in1=xb[0:P, 1:R, :], op=MIN
        )
        nc.vector.tensor_tensor(
            out=v[0:P, 1 : R - 1, :],
            in0=tmp[0:P, 0 : R - 2, :],
            in1=xb[0:P, 2:R, :],
            op=MIN,
        )
        nc.vector.tensor_tensor(
            out=v[0:P, 0:1, :], in0=tmp[0:P, 0:1, :], in1=bt[0:P, 1:2, :], op=MIN
        )
        nc.vector.tensor_tensor(
            out=v[0:P, R - 1 : R, :],
            in0=tmp[0:P, R - 2 : R - 1, :],
            in1=bt[0:P, 0:1, :],
            op=MIN,
        )

        # ---- H pass (bf16, 2x DVE): o[c] = min(v[c], v[c+1], v[c+2]) ----
        h = htm.tile([128, R, cc + 1], BF16)
        nc.vector.tensor_tensor(
            out=h[0:P], in0=v[0:P, :, 0 : cc + 1], in1=v[0:P, :, 1 : cc + 2],
            op=MIN,
        )
        ob = obp.tile([128, R, cc], BF16)
        nc.vector.tensor_tensor(
            out=ob[0:P], in0=h[0:P, :, 0:cc], in1=v[0:P, :, 2 : cc + 2], op=MIN
        )

        # store bf16 from ACT's queue (host widens to fp32 after gather)
        for im in range(n_img):
            dst = o_d[s0 + im, :, c0 : c0 + cc].rearrange(
                "(p r) w -> p r w", p=ppi
            )
            p0 = im * ppi
            nc.scalar.dma_start(dst, ob[p0 : p0 + ppi, :, :])

    # software-pipelined emission: tile i+skew's load/convert lands in
    # every queue before tile i's compute/store, so ACT's in-order
    # sequencer never delays DVE's next tile.
    skew = 2
    for i in range(len(tiles) + skew):
        if i < len(tiles):
            emit_front(i)
        if i >= skew:
            emit_back(i - skew)


def build_erosion(C, H, W, cc=None, rpp=RPP, reps=1, bufs=None, mode="f32eo"):
    """Per-core Bass program: x [C,H,W] f32 -> o [C,H,W] bf16, erosion^reps."""
    if cc is None:
        # f32eo's fp32 tiles need cc=512 to amortize per-op overhead while
        # fitting SBUF; chained (reps>1) builds add a bf16 load pool, so
        # drop to 256 columns.
        cc = 512 if (mode.startswith("f32eo") and reps == 1) else 256
    assert H % rpp == 0
    ppi = H // rpp
    assert ppi <= 128 and W % cc == 0

    nc = bacc.Bacc("TRN2", target_bir_lowering=False, debug=False, num_devices=1)
    x_d = nc.dram_tensor("x", [C, H, W], F32, kind="ExternalInput").ap()
    o_d = nc.dram_tensor("o", [C, H, W], BF16, kind="ExternalOutput").ap()
    # ping-pong DRAM scratch (bf16) for chained passes
    s_d = [
        nc.dram_tensor(f"scratch{i}", [C, H, W], BF16, kind="Internal").ap()
        for i in range(min(2, max(0, reps - 1)))
    ]

    def stage(i):
        src = x_d if i == 0 else s_d[(i - 1) % 2]
        dst = o_d if i == reps - 1 else s_d[i % 2]
        return src, dst

    if mode.startswith("f32eo"):
        bf = {"inp": 2, "xb": 2, "bnd": 2, "vtm": 1, "vt": 1, "htm": 1, "ob": 2}
    else:
        bf = {"inp": 4, "xb": 3, "bnd": 2, "vtm": 1, "vt": 1, "htm": 1, "ob": 3}
    if bufs:
        bf.update(bufs)
    emit = {
        "f32eo": _emit_pass_f32eo,
        "f32eo2": _emit_pass_f32eo2,
        "bf16": _emit_pass,
    }[mode]
    with tile.TileContext(nc) as tc:
        with (
            tc.tile_pool(name="inp", bufs=bf["inp"]) as inp,
            tc.tile_pool(name="xb", bufs=bf["xb"]) as xbp,
            tc.tile_pool(name="bnd", bufs=bf["bnd"]) as bnd,
            tc.tile_pool(name="vtm", bufs=bf["vtm"]) as vtm,
            tc.tile_pool(name="vt", bufs=bf["vt"]) as vt,
            tc.tile_pool(name="htm", bufs=bf["htm"]) as htm,
            tc.tile_pool(name="ob", bufs=bf["ob"]) as obp,
        ):
            pools = (inp, xbp, bnd, vtm, vt, htm, obp)
            for i in range(reps):
                src, dst = stage(i)
                emit(nc, pools, src, dst, C, H, W, cc, rpp, in_f32=(i == 0))
    nc.compile()
    return nc


def build_erosion_bf16a(
    C, H, W, mode="bf16b", cc_first=256, cc_chain=None, rpp=RPP, reps=1,
    bufs=None,
):
    """bf16-mode program builder (modes bf16a / bf16b).

    bf16a: aligned bf16 DVE + ACT parity-fix copy, boundary side-tile.
    bf16b: pure-DVE bf16, halo-row loads, odd-offset H op (measured only
    12% slower than aligned on HW -- no ACT round trip).
    The fp32 first pass uses cc_first columns (the fp32 load tile is the
    SBUF hog); chained bf16 passes use cc_chain. Tile pools size slots to
    the max across passes and unused pools allocate nothing.
    """
    if cc_chain is None:
        cc_chain = 512 if mode == "bf16b" else 256
    if mode == "cg":
        assert H % 256 == 0
    else:
        assert H % rpp == 0
        ppi = H // rpp
        assert ppi <= 128 and W % cc_first == 0 and W % cc_chain == 0

    nc = bacc.Bacc("TRN2", target_bir_lowering=False, debug=False, num_devices=1)
    x_d = nc.dram_tensor("x", [C, H, W], F32, kind="ExternalInput").ap()
    o_d = nc.dram_tensor("o", [C, H, W], BF16, kind="ExternalOutput").ap()
    # 3-deep scratch rotation: pass k's stores WAR-conflict only with pass
    # k-3's loads, so coarse DRAM dep tracking can't stall the pipeline.
    ns = min(3, max(0, reps - 1))
    s_d = [
        nc.dram_tensor(f"scratch{i}", [C, H, W], BF16, kind="Internal").ap()
        for i in range(ns)
    ]

    def stage(i):
        src = x_d if i == 0 else s_d[(i - 1) % ns]
        dst = o_d if i == reps - 1 else s_d[i % ns]
        return src, dst

    if mode == "cg":
        bf = {"t32": 2, "t": 2, "bnd": 2, "p": 1, "v": 1, "vs": 1, "g": 1,
              "ob": 2}
    elif mode == "bf16b":
        bf = {"t32": 2, "t": 2, "bnd": 1, "p": 1, "v": 1, "vs": 1, "g": 1,
              "ob": 3}
    else:
        bf = {"t32": 3, "t": 2, "bnd": 2, "p": 1, "v": 2, "vs": 2, "g": 1,
              "ob": 3}
    if bufs:
        bf.update(bufs)
    emit = {"bf16a": _emit_pass_bf16a, "bf16b": _emit_pass_bf16b}.get(mode)
    with tile.TileContext(nc) as tc:
        with (
            tc.tile_pool(name="t32", bufs=bf["t32"]) as t32p,
            tc.tile_pool(name="t", bufs=bf["t"]) as tp,
            tc.tile_pool(name="bnd", bufs=bf["bnd"]) as bnd,
            tc.tile_pool(name="p", bufs=bf["p"]) as pp,
            tc.tile_pool(name="v", bufs=bf["v"]) as vt,
            tc.tile_pool(name="vs", bufs=bf["vs"]) as vsp,
            tc.tile_pool(name="g", bufs=bf["g"]) as gp,
            tc.tile_pool(name="ob", bufs=bf["ob"]) as obp,
        ):
            pools = (t32p, tp, bnd, pp, vt, vsp, gp, obp)
            for i in range(reps):
                src, dst = stage(i)
                if mode == "cg":
                    _emit_pass_cg(nc, pools, src, dst, C, H, W, in_f32=(i == 0))
                else:
                    cc = cc_first if i == 0 else cc_chain
                    emit(nc, pools, src, dst, C, H, W, cc, rpp, in_f32=(i == 0))
    nc.compile()
    return nc


def _get_program(C, H, W, reps=1, mode="cg"):
    key = (C, H, W, reps, mode)
    if key not in _cache:
        if mode in ("bf16a", "bf16b", "cg"):
            _cache[key] = build_erosion_bf16a(C, H, W, mode=mode, reps=reps)
        else:
            _cache[key] = build_erosion(C, H, W, reps=reps, mode=mode)
    return _cache[key]


def kernel(x, m):
    from concourse.bass_utils import run_bass_kernel_spmd

    m = int(np.asarray(m))
    x = np.ascontiguousarray(np.asarray(x), dtype=np.float32)
    B, C, H, W = x.shape
    if m <= 0:
        return x.copy()
    # erosion by a (2m+1)-square = m chained 3x3 erosion passes in one NEFF
    nc = _get_program(C, H, W, reps=m)
    n_cores = 8
    assert B == n_cores, f"expected batch {n_cores}, got {B}"
    in_maps = [{"x": x[b]} for b in range(n_cores)]
    res = run_bass_kernel_spmd(nc, in_maps, core_ids=list(range(n_cores)))
    # device output is bf16; widen to fp32 on the host
    return np.stack(
        [np.asarray(r["o"]).astype(np.float32) for r in res.results], axis=0
    )


if __name__ == "__main__":
    # small-scale CoreSim correctness check (no hardware needed)
    import ml_dtypes

    from concourse.bass_interp import CoreSim

    rng = np.random.default_rng(0)
    C, H, W = 2, 256, 64
    x = rng.standard_normal((C, H, W)).astype(np.float32)

    def bf16r(a):
        return a.astype(ml_dtypes.bfloat16).astype(np.float32)

    for reps, mode in ((1, "cg"), (2, "cg"), (4, "cg")):
        if mode == "cg":
            nc = build_erosion_bf16a(C, H, W, mode=mode, reps=reps)
        elif mode in ("bf16a", "bf16b"):
            nc = build_erosion_bf16a(
                C, H, W, mode=mode, cc_first=16, cc_chain=32, rpp=4, reps=reps
            )
        else:
            nc = build_erosion(C, H, W, cc=32, rpp=4, reps=reps, mode=mode)
        sim = CoreSim(nc)
        sim.tensor("x")[:] = x
        sim.simulate(check_with_hw=False)
        got = np.asarray(sim.tensor("o")).astype(np.float32)

        # bf16a converts the input to bf16 before the first pass
        exp = bf16r(x) if mode in ("bf16", "bf16a") else x
        for _ in range(reps):
            xp = np.pad(exp, ((0, 0), (1, 1), (1, 1)), constant_values=PAD)
            nxt = np.empty_like(exp)
            for i in range(H):
                for j in range(W):
                    nxt[:, i, j] = xp[:, i : i + 3, j : j + 3].min(axis=(1, 2))
            exp = bf16r(nxt)  # device stores bf16 each pass
        ok = np.array_equal(got, exp)
        rel = np.max(np.abs(got - exp) / np.maximum(np.abs(exp), 1e-6))
        print(f"CoreSim reps={reps} mode={mode} exact: {ok} rel={rel:.2e}")



# revision 34
# speedup vs baseline: 6.5718x; 2.0059x over previous
"""Trainium2 Bass kernel for 3x3 (k=2m+1) morphological erosion (sliding-window
min) over [B, C, H, W] fp32, B=8 sharded across 8 NeuronCores (one batch per
core).

v3 scheme (per core, shard = one batch of C=8 channel images, 1024x1024):
  - each partition holds RPP=16 consecutive image rows (ppi=64 partitions per
    image, 2 images per 128-partition stack), processed in CC=256-column
    chunks with a 1-column halo (PW=258).
  - the separable 3x3 min runs in bf16 on DVE at the 2x_1p rate (2-byte
    dtype + unit innermost stride). min never creates new values, so the
    total error is one bf16 rounding of the input (~2^-9 relative), far
    inside the 2e-2 gate.
  - the device kernel STORES bf16 (halves store traffic: 64MB -> 48MB/core
    round trip); kernel() widens to fp32 on the host after the gather.
  - ACT (scalar) engine does the fp32->bf16 input convert (first pass only;
    chained passes read bf16 scratch directly) and drives store DMAs; loads
    and the per-partition-block boundary-row halo copies ride the SP queue;
    PAD memsets go to Pool. Emission is software-pipelined (skew 2) so no
    in-order sequencer blocks a neighbor engine's next tile.
  - V pass: tmp[j] = min(row j, row j+1); interior v rows from tmp + row
    j+2; the 2 per-partition boundary rows take their missing neighbor from
    a small bf16 side tile bt (partition-shifted SBUF->SBUF DMA; image-edge
    partitions: PAD memset at block tops (legal start partitions 0/64),
    own-row duplication DMA at block bottoms).
  - Cost model: DMA engines ~147us, DVE ~141us, ACT ~55us -> balanced
    DMA/DVE at the bf16-store memory roofline.
  - m>1 runs as m chained passes (bf16 DRAM ping-pong) inside one NEFF.
"""

import sys

sys.path.insert(0, "/opt/trn_rl_repo")

import numpy as np

import concourse.bass as bass
import concourse.tile as tile
from concourse import bacc, mybir

PAD = 1.0e9
F32 = mybir.dt.float32
BF16 = mybir.dt.bfloat16
MIN = mybir.AluOpType.min

CC = 256  # column chunk width
RPP = 16  # image rows per partition

_cache = {}


def _emit_pass_f32eo2(nc, pools, x_d, o_d, C, H, W, cc, rpp, in_f32):
    """One erosion pass, fp32 compute, even/odd V pass over halo-loaded rows.

    Each partition loads R+2 rows (its R rows plus one halo row each side,
    overlapping DRAM reads; +2/R load bytes) so the V pass is exactly 3
    row-stepped DVE ops with no boundary fixups:
      p[j] = min(e[2j], e[2j+1])          j = 0..R/2
      v[2j] = min(p[j], e[2j+2])          j = 0..R/2-1
      v[2j+1] = min(e[2j+1], p[j+1])      j = 0..R/2-1
    Image-edge partitions: top halo row = PAD memset (start partitions
    0/64: legal), bottom halo row = own-last-row duplication DMA
    (min-idempotent). All slices keep unit innermost stride (full DVE
    rate). The final H op writes bf16 (free output cast); host widens.
    """
    from concourse.ap import AP as _AP

    ppi = H // rpp
    ips = max(1, 128 // ppi)
    inp, xbp, bnd, vtm, vt, htm, obp = pools
    R = rpp
    E = R + 2  # rows incl halo
    PW = cc + 2
    assert R % 2 == 0

    tiles = [(s0, c0) for s0 in range(0, C, ips) for c0 in range(0, W, cc)]
    front = {}

    def emit_front(i):
        s0, c0 = tiles[i]
        n_img = min(ips, C - s0)
        P = n_img * ppi
        last = c0 + cc == W
        t = inp.tile([128, E, PW], F32, name="t")
        if in_f32:
            ld = t
        else:
            ld = xbp.tile([128, E, PW], BF16, name="xb")
        wlo = max(c0 - 1, 0)
        whi = min(c0 + cc + 1, W)
        nw = whi - wlo
        dlo = 1 if c0 == 0 else 0
        for im in range(n_img):
            img = x_d[s0 + im]  # [H, W]
            p0 = im * ppi
            # interior partitions 1..ppi-2: rows rpp*p-1 .. rpp*p+rpp
            src = _AP(
                tensor=img.tensor,
                offset=img.offset + (rpp - 1) * W + wlo,
                ap=[[rpp * W, ppi - 2], [W, E], [1, nw]],
            )
            nc.sync.dma_start(
                ld[p0 + 1 : p0 + ppi - 1, :, dlo : dlo + nw], src
            )
            # top partition: rows 0..rpp into slots 1..E-1; slot 0 = PAD
            nc.sync.dma_start(
                ld[p0 : p0 + 1, 1:E, dlo : dlo + nw],
                img[0 : rpp + 1, wlo:whi].rearrange("(p r) w -> p r w", p=1),
            )
            nc.gpsimd.memset(ld[p0 : p0 + 1, 0:1, :], PAD)
            # bottom partition: rows H-rpp-1..H-1 into slots 0..E-2;
            # slot E-1 = duplicate of the image's last row (min-idempotent)
            pe = p0 + ppi - 1
            nc.sync.dma_start(
                ld[pe : pe + 1, 0 : E - 1, dlo : dlo + nw],
                img[H - rpp - 1 : H, wlo:whi].rearrange("(p r) w -> p r w", p=1),
            )
            nc.sync.dma_start(
                ld[pe : pe + 1, E - 1 : E, dlo : dlo + nw],
                img[H - 1 : H, wlo:whi].rearrange("(p r) w -> p r w", p=1),
            )
        if c0 == 0:
            nc.gpsimd.memset(ld[0:P, :, 0:1], PAD)
        if last:
            nc.gpsimd.memset(ld[0:P, :, PW - 1 : PW], PAD)
        if not in_f32:
            nc.scalar.copy(t[0:P], ld[0:P])
        front[i] = (t, n_img, P, s0, c0)

    def emit_back(i):
        t, n_img, P, s0, c0 = front.pop(i)
        Rh = R // 2
        v = vt.tile([128, R, PW], F32, name="v")
        p = vtm.tile([128, Rh + 1, PW], F32, name="p")
        nc.vector.tensor_tensor(
            out=p[0:P], in0=t[0:P, 0:E:2, :], in1=t[0:P, 1:E:2, :], op=MIN
        )
        nc.vector.tensor_tensor(
            out=v[0:P, 0:R:2, :],
            in0=p[0:P, 0:Rh, :],
            in1=t[0:P, 2:E:2, :],
            op=MIN,
        )
        nc.vector.tensor_tensor(
            out=v[0:P, 1:R:2, :],
            in0=t[0:P, 1 : E - 2 : 2, :],
            in1=p[0:P, 1 : Rh + 1, :],
            op=MIN,
        )

        h = htm.tile([128, R, cc + 1], F32, name="h")
        nc.vector.tensor_tensor(
            out=h[0:P], in0=v[0:P, :, 0 : cc + 1], in1=v[0:P, :, 1 : cc + 2],
            op=MIN,
        )
        ob = obp.tile([128, R, cc], BF16, name="ob")
        nc.vector.tensor_tensor(
            out=ob[0:P], in0=h[0:P, :, 0:cc], in1=v[0:P, :, 2 : cc + 2], op=MIN
        )

        for im in range(n_img):
            dst = o_d[s0 + im, :, c0 : c0 + cc].rearrange(
                "(p r) w -> p r w", p=ppi
            )
            p0 = im * ppi
            nc.scalar.dma_start(dst, ob[p0 : p0 + ppi, :, :])

    skew = 2
    for i in range(len(tiles) + skew):
        if i < len(tiles):
            emit_front(i)
        if i >= skew:
            emit_back(i - skew)


def _emit_pass_f32eo(nc, pools, x_d, o_d, C, H, W, cc, rpp, in_f32):
    """One erosion pass, fp32 compute with even/odd-shared V pass.

    x_d fp32 (first pass) or bf16 (chained); o_d bf16. All mins on DVE in
    fp32 (bf16 ALU is slower on real HW); V pass uses the pairwise-sharing
    decomposition (1.5 ops/elem): p[j] = min(r[2j], r[2j+1]);
    v[2j] = min(r[2j-1], p[j]); v[2j+1] = min(p[j], r[2j+2]). Row-stepped
    slices keep unit innermost stride (full DVE rate). The final H op
    writes bf16 (free output cast).
    """
    ppi = H // rpp
    ips = max(1, 128 // ppi)
    inp, xbp, bnd, vtm, vt, htm, obp = pools
    R = rpp
    PW = cc + 2
    assert R % 2 == 0

    tiles = [(s0, c0) for s0 in range(0, C, ips) for c0 in range(0, W, cc)]
    front = {}

    def emit_front(i):
        s0, c0 = tiles[i]
        n_img = min(ips, C - s0)
        P = n_img * ppi
        last = c0 + cc == W
        # load tile: fp32 on first pass, bf16 on chained passes; compute
        # stays fp32 either way (bf16 load is upconverted by ACT)
        t = inp.tile([128, R, PW], F32, name="t")
        if in_f32:
            ld = t
        else:
            ld = xbp.tile([128, R, PW], BF16, name="xb")
        wlo = max(c0 - 1, 0)
        whi = min(c0 + cc + 1, W)
        dlo = 1 if c0 == 0 else 0
        for im in range(n_img):
            src = x_d[s0 + im, :, wlo:whi].rearrange("(p r) w -> p r w", p=ppi)
            p0 = im * ppi
            nc.sync.dma_start(ld[p0 : p0 + ppi, :, dlo : dlo + (whi - wlo)], src)
        if c0 == 0:
            nc.gpsimd.memset(ld[0:P, :, 0:1], PAD)
        if last:
            nc.gpsimd.memset(ld[0:P, :, PW - 1 : PW], PAD)
        if not in_f32:
            nc.scalar.copy(t[0:P], ld[0:P])
        front[i] = (t, n_img, P, s0, c0)

    def emit_back(i):
        t, n_img, P, s0, c0 = front.pop(i)
        bt = bnd.tile([128, 2, PW], F32, name="bt")
        for im in range(n_img):
            p0 = im * ppi
            pe = p0 + ppi - 1
            nc.sync.dma_start(bt[p0:pe, 0:1, :], t[p0 + 1 : pe + 1, 0:1, :])
            nc.sync.dma_start(
                bt[p0 + 1 : pe + 1, 1:2, :], t[p0:pe, R - 1 : R, :]
            )
            nc.sync.dma_start(
                bt[pe : pe + 1, 0:1, :], t[pe : pe + 1, R - 1 : R, :]
            )
            nc.gpsimd.memset(bt[p0 : p0 + 1, 1:2, :], PAD)

        # ---- V pass, even/odd shared (fp32) ----
        v = vt.tile([128, R, PW], F32, name="v")
        p = vtm.tile([128, R // 2, PW], F32, name="p")
        Rh = R // 2
        nc.vector.tensor_tensor(
            out=p[0:P], in0=t[0:P, 0:R:2, :], in1=t[0:P, 1:R:2, :], op=MIN
        )
        # v[2j] = min(r[2j-1], p[j]), j=1..Rh-1 (j=0 uses bt row-above)
        nc.vector.tensor_tensor(
            out=v[0:P, 2:R:2, :],
            in0=t[0:P, 1 : R - 2 : 2, :],
            in1=p[0:P, 1:Rh, :],
            op=MIN,
        )
        nc.vector.tensor_tensor(
            out=v[0:P, 0:1, :], in0=p[0:P, 0:1, :], in1=bt[0:P, 1:2, :], op=MIN
        )
        # v[2j+1] = min(p[j], r[2j+2]), j=0..Rh-2 (j=Rh-1 uses bt row-below)
        nc.vector.tensor_tensor(
            out=v[0:P, 1 : R - 1 : 2, :],
            in0=p[0:P, 0 : Rh - 1, :],
            in1=t[0:P, 2:R:2, :],
            op=MIN,
        )
        nc.vector.tensor_tensor(
            out=v[0:P, R - 1 : R, :],
            in0=p[0:P, Rh - 1 : Rh, :],
            in1=bt[0:P, 0:1, :],
            op=MIN,
        )

        # ---- H pass (fp32, last op casts to bf16 on output) ----
        h = htm.tile([128, R, cc + 1], F32, name="h")
        nc.vector.tensor_tensor(
            out=h[0:P], in0=v[0:P, :, 0 : cc + 1], in1=v[0:P, :, 1 : cc + 2],
            op=MIN,
        )
        ob = obp.tile([128, R, cc], BF16, name="ob")
        nc.vector.tensor_tensor(
            out=ob[0:P], in0=h[0:P, :, 0:cc], in1=v[0:P, :, 2 : cc + 2], op=MIN
        )

        for im in range(n_img):
            dst = o_d[s0 + im, :, c0 : c0 + cc].rearrange(
                "(p r) w -> p r w", p=ppi
            )
            p0 = im * ppi
            nc.scalar.dma_start(dst, ob[p0 : p0 + ppi, :, :])

    skew = 2
    for i in range(len(tiles) + skew):
        if i < len(tiles):
            emit_front(i)
        if i >= skew:
            emit_back(i - skew)


def _emit_pass_cg(nc, pools, x_d, o_d, C, H, W, in_f32, probe="none"):
    """One erosion pass, contiguous-big-DMA layout + bf16 DVE compute.

    HW measurements drove this shape:
      - DVE TT min bf16 = 0.372 ns/elem-pp (0.415 odd-offset) vs fp32 1.097
        -> all mins in bf16.
      - DMA throughput is descriptor-dominated: the old [ppi,R,~1KB-line]
        strided loads measured ~45 GB/s; >=16KB contiguous-per-partition
        descriptors reach ~350-400 GB/s.
    So: one image per 128-partition stack, partition p holds R=H/128 FULL
    rows (contiguous 8KB-32KB span in DRAM and in SBUF) -> ONE descriptor
    per partition per image load/store. Per image per pass: 1 big load
    (sync queue), 2 partition-shifted SBUF->SBUF row copies for the V-pass
    boundary rows (scalar queue), 1 big store (scalar queue). Image-edge
    partitions get their boundary row via tiny 1-partition DVE ops reading
    the tile's own edge row (min-idempotent duplicate) -- no memsets, no
    PAD columns: the H-pass edge columns use dedicated 1-column ops.
    """
    R = H // 128
    Rh = R // 2
    assert H % 128 == 0 and R % 2 == 0
    t32p, tp, bnd, pp, vt, _vsp, hp, obp = pools
    front = {}

    def emit_front(i):
        from concourse.ap import AP as _AP

        t = tp.tile([128, R, W], BF16, name="t")
        if in_f32:
            # fp32 staging in two half-tiles: halves the t32 SBUF slot so
            # the bf16 pipeline pools can run deeper
            Rhalf = R // 2
            img = x_d[i]
            for c in range(2):
                ld = t32p.tile([128, Rhalf, W], F32, name="t32")
                src_c = _AP(
                    tensor=img.tensor,
                    offset=img.offset + c * Rhalf * W,
                    ap=[[R * W, 128], [1, Rhalf * W]],
                )
                nc.sync.dma_start(ld.rearrange("p r w -> p (r w)"), src_c)
                nc.scalar.copy(t[:, c * Rhalf : (c + 1) * Rhalf, :], ld)
        else:
            src = x_d[i].rearrange("(p r) w -> p (r w)", p=128)
            nc.sync.dma_start(t.rearrange("p r w -> p (r w)"), src)
        # bt[p,0] = row above (prev partition's last row),
        # bt[p,1] = row below (next partition's first row)
        if probe == "dma":
            front[i] = (t, None)
            return
        bt = bnd.tile([128, 2, W], BF16, name="bt")
        nc.scalar.dma_start(bt[1:128, 0:1, :], t[0:127, R - 1 : R, :])
        nc.scalar.dma_start(bt[0:127, 1:2, :], t[1:128, 0:1, :])
        # image-edge partitions duplicate their own edge row (the DVE
        # rejects partition starts other than 0/32/64/96, so these are DMAs)
        nc.scalar.dma_start(bt[0:1, 0:1, :], t[0:1, 0:1, :])
        nc.scalar.dma_start(bt[127:128, 1:2, :], t[127:128, R - 1 : R, :])
        front[i] = (t, bt)

    def emit_back(i):
        t, bt = front.pop(i)
        if probe == "dma":  # timing probe: load->store only
            dst = o_d[i].rearrange("(p r) w -> p (r w)", p=128)
            nc.scalar.dma_start(dst, t.rearrange("p r w -> p (r w)"))
            return
        # ---- V pass (bf16; all aligned) ----
        p = pp.tile([128, Rh, W], BF16, name="p")
        v = vt.tile([128, R, W], BF16, name="v")
        nc.vector.tensor_tensor(
            out=p, in0=t[:, 0:R:2, :], in1=t[:, 1:R:2, :], op=MIN
        )
        if Rh > 1:
            # v[2j] = min(r[2j-1], p[j]), j=1..Rh-1
            nc.vector.tensor_tensor(
                out=v[:, 2:R:2, :],
                in0=t[:, 1 : R - 2 : 2, :],
                in1=p[:, 1:Rh, :],
                op=MIN,
            )
            # v[2j+1] = min(p[j], r[2j+2]), j=0..Rh-2
            nc.vector.tensor_tensor(
                out=v[:, 1 : R - 1 : 2, :],
                in0=p[:, 0 : Rh - 1, :],
                in1=t[:, 2:R:2, :],
                op=MIN,
            )
        # v[0] = min(p[0], row-above); v[R-1] = min(p[Rh-1], row-below)
        nc.vector.tensor_tensor(
            out=v[:, 0:1, :], in0=p[:, 0:1, :], in1=bt[:, 0:1, :], op=MIN
        )
        nc.vector.tensor_tensor(
            out=v[:, R - 1 : R, :],
            in0=p[:, Rh - 1 : Rh, :],
            in1=bt[:, 1:2, :],
            op=MIN,
        )
        if probe == "noh":  # timing probe: V pass only, store v
            dst = o_d[i].rearrange("(p r) w -> p (r w)", p=128)
            nc.scalar.dma_start(dst, v.rearrange("p r w -> p (r w)"))
            return
        # ---- H pass: h[c] = min(v[c], v[c+1]); ob interior + edge cols ----
        h = hp.tile([128, R, W], BF16, name="h")
        nc.vector.tensor_tensor(
            out=h[:, :, 0 : W - 1],
            in0=v[:, :, 0 : W - 1],
            in1=v[:, :, 1:W],
            op=MIN,
        )
        ob = obp.tile([128, R, W], BF16, name="ob")
        nc.vector.tensor_tensor(
            out=ob[:, :, 1 : W - 1],
            in0=h[:, :, 0 : W - 2],
            in1=v[:, :, 2:W],
            op=MIN,
        )
        nc.vector.tensor_tensor(
            out=ob[:, :, 0:1], in0=h[:, :, 0:1], in1=v[:, :, 1:2], op=MIN
        )
        nc.vector.tensor_copy(
            out=ob[:, :, W - 1 : W], in_=h[:, :, W - 2 : W - 1]
        )
        dst = o_d[i].rearrange("(p r) w -> p (r w)", p=128)
        nc.scalar.dma_start(dst, ob.rearrange("p r w -> p (r w)"))

    skew = 2
    for i in range(C + skew):
        if i < C:
            emit_front(i)
        if i >= skew:
            emit_back(i - skew)


def build_erosion_rs(C, H, W, reps):
    """SBUF-resident multi-pass erosion: the whole per-core shard (C images
    x H/128 rows x W cols, bf16 = 16 KiB/partition/image) stays in SBUF
    across all passes.

    Load once (gpsimd casting DMA, fp32->bf16 inline), run `reps` erosion
    passes entirely on-chip (DVE mins + 4 small SBUF->SBUF boundary-row
    DMAs per image per pass), store once. Per-pass marginal cost is pure
    DVE (~90 us) -- no HBM traffic per pass. The image pool has C+1
    slots; each pass writes image i's result into the rotating spare slot
    and the old buffer becomes the next spare (Tile's WAR deps make the
    rotation safe).
    """
    R = H // 128
    Rh = R // 2
    assert H % 256 == 0 and R % 2 == 0 and reps >= 1

    nc = bacc.Bacc("TRN2", target_bir_lowering=False, debug=False, num_devices=1)
    x_d = nc.dram_tensor("x", [C, H, W], F32, kind="ExternalInput").ap()
    o_d = nc.dram_tensor("o", [C, H, W], BF16, kind="ExternalOutput").ap()

    with tile.TileContext(nc) as tc:
        with (
            tc.tile_pool(name="img", bufs=C + 1) as imgp,
            tc.tile_pool(name="bnd", bufs=3) as bnd,
            tc.tile_pool(name="p", bufs=1) as pp,
            tc.tile_pool(name="v", bufs=1) as vt,
            tc.tile_pool(name="h", bufs=1) as hp,
        ):
            img_t = []
            for i in range(C):
                t = imgp.tile([128, R, W], BF16, name="img")
                nc.gpsimd.dma_start(
                    t.rearrange("p r w -> p (r w)"),
                    x_d[i].rearrange("(p r) w -> p (r w)", p=128),
                )
                img_t.append(t)

            bts = {}

            def emit_bt(i):
                t = img_t[i]
                bt = bnd.tile([128, 2, W], BF16, name="bt")
                nc.scalar.dma_start(bt[1:128, 0:1, :], t[0:127, R - 1 : R, :])
                nc.scalar.dma_start(bt[0:127, 1:2, :], t[1:128, 0:1, :])
                nc.scalar.dma_start(bt[0:1, 0:1, :], t[0:1, 0:1, :])
                nc.scalar.dma_start(
                    bt[127:128, 1:2, :], t[127:128, R - 1 : R, :]
                )
                bts[i] = bt

            def emit_compute(i):
                t = img_t[i]
                bt = bts.pop(i)
                p = pp.tile([128, Rh, W], BF16, name="p")
                v = vt.tile([128, R, W], BF16, name="v")
                nc.vector.tensor_tensor(
                    out=p, in0=t[:, 0:R:2, :], in1=t[:, 1:R:2, :], op=MIN
                )
                if Rh > 1:
                    nc.vector.tensor_tensor(
                        out=v[:, 2:R:2, :],
                        in0=t[:, 1 : R - 2 : 2, :],
                        in1=p[:, 1:Rh, :],
                        op=MIN,
                    )
                    nc.vector.tensor_tensor(
                        out=v[:, 1 : R - 1 : 2, :],
                        in0=p[:, 0 : Rh - 1, :],
                        in1=t[:, 2:R:2, :],
                        op=MIN,
                    )
                nc.vector.tensor_tensor(
                    out=v[:, 0:1, :], in0=p[:, 0:1, :], in1=bt[:, 0:1, :],
                    op=MIN,
                )
                nc.vector.tensor_tensor(
                    out=v[:, R - 1 : R, :],
                    in0=p[:, Rh - 1 : Rh, :],
                    in1=bt[:, 1:2, :],
                    op=MIN,
                )
                h = hp.tile([128, R, W], BF16, name="h")
                nc.vector.tensor_tensor(
                    out=h[:, :, 0 : W - 1],
                    in0=v[:, :, 0 : W - 1],
                    in1=v[:, :, 1:W],
                    op=MIN,
                )
                nxt = imgp.tile([128, R, W], BF16, name="img")
                nc.vector.tensor_tensor(
                    out=nxt[:, :, 1 : W - 1],
                    in0=h[:, :, 0 : W - 2],
                    in1=v[:, :, 2:W],
                    op=MIN,
                )
                nc.vector.tensor_tensor(
                    out=nxt[:, :, 0:1], in0=h[:, :, 0:1], in1=v[:, :, 1:2],
                    op=MIN,
                )
                nc.vector.tensor_copy(
                    out=nxt[:, :, W - 1 : W], in_=h[:, :, W - 2 : W - 1]
                )
                img_t[i] = nxt

            for k in range(reps):
                for i in range(C + 1):
                    if i < C:
                        emit_bt(i)
                    if i >= 1:
                        emit_compute(i - 1)

            for i in range(C):
                nc.sync.dma_start(
                    o_d[i].rearrange("(p r) w -> p (r w)", p=128),
                    img_t[i].rearrange("p r w -> p (r w)"),
                )
    nc.compile()
    return nc


def _emit_pass_bf16b(nc, pools, x_d, o_d, C, H, W, cc, rpp, in_f32):
    """One erosion pass, pure-DVE bf16 compute, halo-row loads (f32eo2
    style), no ACT compute and no SBUF->SBUF boundary DMAs.

    Measured on HW (microbench): TT min bf16 = 0.372 ns/elem-per-partition
    when 4B-aligned and 0.415 when one operand starts at an odd column --
    both ~2.9x faster than fp32 (1.097). So compute everything in bf16 on
    the DVE: V pass = even/odd shared decomposition over the halo-loaded
    rows (all aligned), H pass = h(odd operand) then ob (aligned). The
    only ACT work is the fp32->bf16 convert on the first pass; stores
    ride the ACT queue as plain triggers.
    """
    from concourse.ap import AP as _AP

    ppi = H // rpp
    ips = max(1, 128 // ppi)
    t32p, tp, _bnd, pp, vt, _vsp, hp, obp = pools
    R = rpp
    Rh = R // 2
    E = R + 2
    PW = cc + 2
    assert R % 2 == 0 and cc % 2 == 0

    tiles = [(s0, c0) for s0 in range(0, C, ips) for c0 in range(0, W, cc)]
    front = {}

    def emit_front(i):
        s0, c0 = tiles[i]
        n_img = min(ips, C - s0)
        P = n_img * ppi
        last = c0 + cc == W
        t = tp.tile([128, E, PW], BF16, name="t")
        ld = t32p.tile([128, E, PW], F32, name="t32") if in_f32 else t
        wlo = max(c0 - 1, 0)
        whi = min(c0 + cc + 1, W)
        nw = whi - wlo
        dlo = 1 if c0 == 0 else 0
        for im in range(n_img):
            img = x_d[s0 + im]  # [H, W]
            p0 = im * ppi
            # interior partitions 1..ppi-2: rows rpp*p-1 .. rpp*p+rpp
            src = _AP(
                tensor=img.tensor,
                offset=img.offset + (rpp - 1) * W + wlo,
                ap=[[rpp * W, ppi - 2], [W, E], [1, nw]],
            )
            nc.sync.dma_start(ld[p0 + 1 : p0 + ppi - 1, :, dlo : dlo + nw], src)
            # top partition: rows 0..rpp into slots 1..E-1; slot 0 = row 0
            # again (min-idempotent duplicate stands in for the PAD row)
            nc.sync.dma_start(
                ld[p0 : p0 + 1, 1:E, dlo : dlo + nw],
                img[0 : rpp + 1, wlo:whi].rearrange("(p r) w -> p r w", p=1),
            )
            nc.sync.dma_start(
                ld[p0 : p0 + 1, 0:1, dlo : dlo + nw],
                img[0:1, wlo:whi].rearrange("(p r) w -> p r w", p=1),
            )
            # bottom partition: rows H-rpp-1..H-1 into slots 0..E-2;
            # slot E-1 = duplicate of the image's last row
            pe = p0 + ppi - 1
            nc.sync.dma_start(
                ld[pe : pe + 1, 0 : E - 1, dlo : dlo + nw],
                img[H - rpp - 1 : H, wlo:whi].rearrange("(p r) w -> p r w", p=1),
            )
            nc.sync.dma_start(
                ld[pe : pe + 1, E - 1 : E, dlo : dlo + nw],
                img[H - 1 : H, wlo:whi].rearrange("(p r) w -> p r w", p=1),
            )
        if c0 == 0:
            nc.gpsimd.memset(ld[0:P, :, 0:1], PAD)
        if last:
            nc.gpsimd.memset(ld[0:P, :, PW - 1 : PW], PAD)
        if in_f32:
            nc.scalar.copy(t[0:P], ld[0:P])
        front[i] = (t, n_img, P, s0, c0)

    def emit_back(i):
        t, n_img, P, s0, c0 = front.pop(i)
        # ---- V pass over halo rows (bf16, aligned -> fast) ----
        p = pp.tile([128, Rh + 1, PW], BF16, name="p")
        v = vt.tile([128, R, PW], BF16, name="v")
        nc.vector.tensor_tensor(
            out=p[0:P], in0=t[0:P, 0:E:2, :], in1=t[0:P, 1:E:2, :], op=MIN
        )
        nc.vector.tensor_tensor(
            out=v[0:P, 0:R:2, :],
            in0=p[0:P, 0:Rh, :],
            in1=t[0:P, 2:E:2, :],
            op=MIN,
        )
        nc.vector.tensor_tensor(
            out=v[0:P, 1:R:2, :],
            in0=t[0:P, 1 : E - 2 : 2, :],
            in1=p[0:P, 1 : Rh + 1, :],
            op=MIN,
        )
        # ---- H pass: h has one odd-offset operand (12% slower, fine) ----
        h = hp.tile([128, R, PW], BF16, name="h")
        nc.vector.tensor_tensor(
            out=h[0:P, :, 0 : cc + 1],
            in0=v[0:P, :, 0 : cc + 1],
            in1=v[0:P, :, 1 : cc + 2],
            op=MIN,
        )
        ob = obp.tile([128, R, cc], BF16, name="ob")
        nc.vector.tensor_tensor(
            out=ob[0:P], in0=h[0:P, :, 0:cc], in1=v[0:P, :, 2 : cc + 2], op=MIN
        )
        for im in range(n_img):
            dst = o_d[s0 + im, :, c0 : c0 + cc].rearrange(
                "(p r) w -> p r w", p=ppi
            )
            p0 = im * ppi
            nc.scalar.dma_start(dst, ob[p0 : p0 + ppi, :, :])

    skew = 2
    for i in range(len(tiles) + skew):
        if i < len(tiles):
            emit_front(i)
        if i >= skew:
            emit_back(i - skew)


def _emit_pass_bf16a(nc, pools, x_d, o_d, C, H, W, cc, rpp, in_f32):
    """One erosion pass, all-bf16 DVE compute with every tensor_tensor
    operand 4B-aligned so the DVE runs at the 2x_1p rate throughout.

    The 2x_1p perf mode needs 16-bit dtype, unit innermost stride AND a
    4-byte-aligned start address for every operand. The H pass combines
    columns of both parities, so one op would always have a 2-byte-offset
    (odd) operand and silently drop to 1x. Fix: the otherwise-idle ACT
    engine makes a one-column-shifted copy vs[c] = v[c+1] (ACT is 1x at
    1.2 GHz regardless of alignment), and the DVE computes
        g[c]  = min(v[c], v[c+2])   (both even offsets, 2x)
        ob[c] = min(g[c], vs[c])    (both aligned, 2x)
    V pass is the even/odd-shared decomposition (1.5 ops/elem, all row
    slices start at even column 0 -> aligned, 2x). Per-partition boundary
    rows come from a small side tile bt filled by partition-shifted
    SBUF->SBUF DMAs; image edges use own-row duplication (min-idempotent)
    so no memsets on partition starts other than 0 are needed.

    DVE ~1.75 cyc/elem (vs 3.5 at 1x fp32), ACT 1 copy/elem (+1 convert
    on the first pass), DMA loads R rows/partition (no halo rows).
    """
    ppi = H // rpp
    ips = max(1, 128 // ppi)
    t32p, tp, bnd, pp, vt, vsp, gp, obp = pools
    R = rpp
    Rh = R // 2
    PW = cc + 2
    assert R % 2 == 0 and cc % 2 == 0 and PW % 2 == 0

    tiles = [(s0, c0) for s0 in range(0, C, ips) for c0 in range(0, W, cc)]
    front = {}
    mid = {}

    def emit_front(i):
        s0, c0 = tiles[i]
        n_img = min(ips, C - s0)
        P = n_img * ppi
        last = c0 + cc == W
        t = tp.tile([128, R, PW], BF16, name="t")
        ld = t32p.tile([128, R, PW], F32, name="t32") if in_f32 else t
        wlo = max(c0 - 1, 0)
        whi = min(c0 + cc + 1, W)
        dlo = 1 if c0 == 0 else 0
        for im in range(n_img):
            src = x_d[s0 + im, :, wlo:whi].rearrange("(p r) w -> p r w", p=ppi)
            p0 = im * ppi
            nc.sync.dma_start(ld[p0 : p0 + ppi, :, dlo : dlo + (whi - wlo)], src)
        if c0 == 0:
            nc.gpsimd.memset(ld[0:P, :, 0:1], PAD)
        if last:
            nc.gpsimd.memset(ld[0:P, :, PW - 1 : PW], PAD)
        if in_f32:
            nc.scalar.copy(t[0:P], ld[0:P])
        front[i] = (t, n_img, P, s0, c0)

    def emit_mid(i):
        t, n_img, P, s0, c0 = front.pop(i)
        # bt[p,0] = row below the block (next partition's row 0),
        # bt[p,1] = row above (prev partition's row R-1); image edges
        # duplicate the block's own edge row (min-idempotent).
        bt = bnd.tile([128, 2, PW], BF16, name="bt")
        for im in range(n_img):
            p0 = im * ppi
            pe = p0 + ppi - 1
            nc.sync.dma_start(bt[p0:pe, 0:1, :], t[p0 + 1 : pe + 1, 0:1, :])
            nc.sync.dma_start(bt[pe : pe + 1, 0:1, :], t[pe : pe + 1, R - 1 : R, :])
            nc.sync.dma_start(bt[p0 + 1 : pe + 1, 1:2, :], t[p0:pe, R - 1 : R, :])
            nc.sync.dma_start(bt[p0 : p0 + 1, 1:2, :], t[p0 : p0 + 1, 0:1, :])

        # ---- V pass (bf16, all operands 4B-aligned -> 2x) ----
        p = pp.tile([128, Rh, PW], BF16, name="p")
        v = vt.tile([128, R, PW], BF16, name="v")
        nc.vector.tensor_tensor(
            out=p[0:P], in0=t[0:P, 0:R:2, :], in1=t[0:P, 1:R:2, :], op=MIN
        )
        if Rh > 1:
            nc.vector.tensor_tensor(
                out=v[0:P, 2:R:2, :],
                in0=t[0:P, 1 : R - 2 : 2, :],
                in1=p[0:P, 1:Rh, :],
                op=MIN,
            )
            nc.vector.tensor_tensor(
                out=v[0:P, 1 : R - 1 : 2, :],
                in0=p[0:P, 0 : Rh - 1, :],
                in1=t[0:P, 2:R:2, :],
                op=MIN,
            )
        nc.vector.tensor_tensor(
            out=v[0:P, 0:1, :], in0=p[0:P, 0:1, :], in1=bt[0:P, 1:2, :], op=MIN
        )
        nc.vector.tensor_tensor(
            out=v[0:P, R - 1 : R, :],
            in0=p[0:P, Rh - 1 : Rh, :],
            in1=bt[0:P, 0:1, :],
            op=MIN,
        )
        # ACT: parity-fixing shifted copy (1x, alignment-agnostic)
        vs = vsp.tile([128, R, cc], BF16, name="vs")
        nc.scalar.copy(vs[0:P], v[0:P, :, 1 : cc + 1])
        mid[i] = (v, vs, n_img, P, s0, c0)

    def emit_tail(i):
        v, vs, n_img, P, s0, c0 = mid.pop(i)
        g = gp.tile([128, R, cc], BF16, name="g")
        nc.vector.tensor_tensor(
            out=g[0:P], in0=v[0:P, :, 0:cc], in1=v[0:P, :, 2 : cc + 2], op=MIN
        )
        ob = obp.tile([128, R, cc], BF16, name="ob")
        nc.vector.tensor_tensor(out=ob[0:P], in0=g[0:P], in1=vs[0:P], op=MIN)
        for im in range(n_img):
            dst = o_d[s0 + im, :, c0 : c0 + cc].rearrange(
                "(p r) w -> p r w", p=ppi
            )
            p0 = im * ppi
            nc.scalar.dma_start(dst, ob[p0 : p0 + ppi, :, :])

    # 3-stage software pipeline: tail(i) is emitted after mid(i+1), so the
    # DVE's ob(i) lands behind V(i+1) in its in-order stream -- by then the
    # ACT's vs(i) is long done and the DVE never stalls on the ACT.
    n = len(tiles)
    for i in range(n + 2):
        if i < n:
            emit_front(i)
        if 1 <= i <= n:
            emit_mid(i - 1)
        if i >= 2:
            emit_tail(i - 2)


def _emit_pass(nc, pools, x_d, o_d, C, H, W, cc, rpp, in_f32):
    """Emit one full erosion pass x_d -> o_d into the open TileContext.

    in_f32: x_d is fp32 and must be converted to bf16 on ACT; otherwise
    x_d is bf16 and is used directly. o_d is always bf16.
    """
    ppi = H // rpp  # partitions per image
    ips = max(1, 128 // ppi)  # images per partition-stack
    inp, xbp, bnd, vtm, vt, htm, obp = pools
    R = rpp
    PW = cc + 2  # padded tile width

    tiles = [(s0, c0) for s0 in range(0, C, ips) for c0 in range(0, W, cc)]
    front = {}

    def emit_front(i):
        s0, c0 = tiles[i]
        n_img = min(ips, C - s0)
        P = n_img * ppi
        last = c0 + cc == W
        t = inp.tile([128, R, PW], F32, name="t") if in_f32 else None
        xb = xbp.tile([128, R, PW], BF16, name="xb")
        ld = t if in_f32 else xb
        wlo = max(c0 - 1, 0)
        whi = min(c0 + cc + 1, W)
        dlo = 1 if c0 == 0 else 0
        for im in range(n_img):
            src = x_d[s0 + im, :, wlo:whi].rearrange("(p r) w -> p r w", p=ppi)
            p0 = im * ppi
            nc.sync.dma_start(ld[p0 : p0 + ppi, :, dlo : dlo + (whi - wlo)], src)
        if c0 == 0:
            nc.gpsimd.memset(ld[0:P, :, 0:1], PAD)
        if last:
            nc.gpsimd.memset(ld[0:P, :, PW - 1 : PW], PAD)
        if in_f32:
            nc.scalar.copy(xb[0:P], t[0:P])
        front[i] = (xb, n_img, P, s0, c0)

    def emit_back(i):
        xb, n_img, P, s0, c0 = front.pop(i)
        # boundary-row side tile (bf16): bt[p,0] = first row of the block
        # below (xb[p+1] row 0), bt[p,1] = last row of the block above
        # (xb[p-1] row R-1); image-edge partitions: top -> PAD memset
        # (start partition p0 is 0 mod ppi>=32: legal), bottom -> own-row
        # duplication DMA (min-idempotent; DMA has no start-partition rule).
        bt = bnd.tile([128, 2, PW], BF16)
        for im in range(n_img):
            p0 = im * ppi
            pe = p0 + ppi - 1  # last partition of this image
            nc.sync.dma_start(bt[p0:pe, 0:1, :], xb[p0 + 1 : pe + 1, 0:1, :])
            nc.sync.dma_start(
                bt[p0 + 1 : pe + 1, 1:2, :], xb[p0:pe, R - 1 : R, :]
            )
            nc.sync.dma_start(
                bt[pe : pe + 1, 0:1, :], xb[pe : pe + 1, R - 1 : R, :]
            )
            nc.gpsimd.memset(bt[p0 : p0 + 1, 1:2, :], PAD)

        # ---- V pass (bf16, 2x DVE): v[r] = min(row r-1, r, r+1) ----
        v = vt.tile([128, R, PW], BF16)
        tmp = vtm.tile([128, R - 1, PW], BF16)
        nc.vector.tensor_tensor(
            out=tmp[0:P], in0=xb[0:P, 0 : R - 1, :], in1=xb[0:P, 1:R, :], op=MIN
        )
        nc.vector.tensor_tensor(
            out=v[0:P, 1 : R - 1, :],
            in0=tmp[0:P, 0 : R - 2, :],
            in1=xb[0:P, 2:R, :],
            op=MIN,
        )
        nc.vector.tensor_tensor(
            out=v[0:P, 0:1, :], in0=tmp[0:P, 0:1, :], in1=bt[0:P, 1:2, :], op=MIN
        )
        nc.vector.tensor_tensor(
            out=v[0:P, R - 1 : R, :],
            in0=tmp[0:P, R - 2 : R - 1, :],
            in1=bt[0:P, 0:1, :],
            op=MIN,
        )

        # ---- H pass (bf16, 2x DVE): o[c] = min(v[c], v[c+1], v[c+2]) ----
        h = htm.tile([128, R, cc + 1], BF16)
        nc.vector.tensor_tensor(
            out=h[0:P], in0=v[0:P, :, 0 : cc + 1], in1=v[0:P, :, 1 : cc + 2],
            op=MIN,
        )
        ob = obp.tile([128, R, cc], BF16)
        nc.vector.tensor_tensor(
            out=ob[0:P], in0=h[0:P, :, 0:cc], in1=v[0:P, :, 2 : cc + 2], op=MIN
        )

        # store bf16 from ACT's queue (host widens to fp32 after gather)
        for im in range(n_img):
            dst = o_d[s0 + im, :, c0 : c0 + cc].rearrange(
                "(p r) w -> p r w", p=ppi
            )
            p0 = im * ppi
            nc.scalar.dma_start(dst, ob[p0 : p0 + ppi, :, :])

    # software-pipelined emission: tile i+skew's load/convert lands in
    # every queue before tile i's compute/store, so ACT's in-order
    # sequencer never delays DVE's next tile.
    skew = 2
    for i in range(len(tiles) + skew):
        if i < len(tiles):
            emit_front(i)
        if i >= skew:
            emit_back(i - skew)


def build_erosion(C, H, W, cc=None, rpp=RPP, reps=1, bufs=None, mode="f32eo"):
    """Per-core Bass program: x [C,H,W] f32 -> o [C,H,W] bf16, erosion^reps."""
    if cc is None:
        # f32eo's fp32 tiles need cc=512 to amortize per-op overhead while
        # fitting SBUF; chained (reps>1) builds add a bf16 load pool, so
        # drop to 256 columns.
        cc = 512 if (mode.startswith("f32eo") and reps == 1) else 256
    assert H % rpp == 0
    ppi = H // rpp
    assert ppi <= 128 and W % cc == 0

    nc = bacc.Bacc("TRN2", target_bir_lowering=False, debug=False, num_devices=1)
    x_d = nc.dram_tensor("x", [C, H, W], F32, kind="ExternalInput").ap()
    o_d = nc.dram_tensor("o", [C, H, W], BF16, kind="ExternalOutput").ap()
    # ping-pong DRAM scratch (bf16) for chained passes
    s_d = [
        nc.dram_tensor(f"scratch{i}", [C, H, W], BF16, kind="Internal").ap()
        for i in range(min(2, max(0, reps - 1)))
    ]

    def stage(i):
        src = x_d if i == 0 else s_d[(i - 1) % 2]
        dst = o_d if i == reps - 1 else s_d[i % 2]
        return src, dst

    if mode.startswith("f32eo"):
        bf = {"inp": 2, "xb": 2, "bnd": 2, "vtm": 1, "vt": 1, "htm": 1, "ob": 2}
    else:
        bf = {"inp": 4, "xb": 3, "bnd": 2, "vtm": 1, "vt": 1, "htm": 1, "ob": 3}
    if bufs:
        bf.update(bufs)
    emit = {
        "f32eo": _emit_pass_f32eo,
        "f32eo2": _emit_pass_f32eo2,
        "bf16": _emit_pass,
    }[mode]
    with tile.TileContext(nc) as tc:
        with (
            tc.tile_pool(name="inp", bufs=bf["inp"]) as inp,
            tc.tile_pool(name="xb", bufs=bf["xb"]) as xbp,
            tc.tile_pool(name="bnd", bufs=bf["bnd"]) as bnd,
            tc.tile_pool(name="vtm", bufs=bf["vtm"]) as vtm,
            tc.tile_pool(name="vt", bufs=bf["vt"]) as vt,
            tc.tile_pool(name="htm", bufs=bf["htm"]) as htm,
            tc.tile_pool(name="ob", bufs=bf["ob"]) as obp,
        ):
            pools = (inp, xbp, bnd, vtm, vt, htm, obp)
            for i in range(reps):
                src, dst = stage(i)
                emit(nc, pools, src, dst, C, H, W, cc, rpp, in_f32=(i == 0))
    nc.compile()
    return nc


def build_erosion_bf16a(
    C, H, W, mode="bf16b", cc_first=256, cc_chain=None, rpp=RPP, reps=1,
    bufs=None,
):
    """bf16-mode program builder (modes bf16a / bf16b).

    bf16a: aligned bf16 DVE + ACT parity-fix copy, boundary side-tile.
    bf16b: pure-DVE bf16, halo-row loads, odd-offset H op (measured only
    12% slower than aligned on HW -- no ACT round trip).
    The fp32 first pass uses cc_first columns (the fp32 load tile is the
    SBUF hog); chained bf16 passes use cc_chain. Tile pools size slots to
    the max across passes and unused pools allocate nothing.
    """
    if cc_chain is None:
        cc_chain = 512 if mode == "bf16b" else 256
    if mode.startswith("cg"):
        assert H % 256 == 0
    else:
        assert H % rpp == 0
        ppi = H // rpp
        assert ppi <= 128 and W % cc_first == 0 and W % cc_chain == 0

    nc = bacc.Bacc("TRN2", target_bir_lowering=False, debug=False, num_devices=1)
    x_d = nc.dram_tensor("x", [C, H, W], F32, kind="ExternalInput").ap()
    o_d = nc.dram_tensor("o", [C, H, W], BF16, kind="ExternalOutput").ap()
    # 3-deep scratch rotation: pass k's stores WAR-conflict only with pass
    # k-3's loads, so coarse DRAM dep tracking can't stall the pipeline.
    ns = min(3, max(0, reps - 1))
    s_d = [
        nc.dram_tensor(f"scratch{i}", [C, H, W], BF16, kind="Internal").ap()
        for i in range(ns)
    ]

    def stage(i):
        src = x_d if i == 0 else s_d[(i - 1) % ns]
        dst = o_d if i == reps - 1 else s_d[i % ns]
        return src, dst

    if mode.startswith("cg"):
        bf = {"t32": 2, "t": 3, "bnd": 3, "p": 2, "v": 2, "vs": 1, "g": 1,
              "ob": 3}
    elif mode == "bf16b":
        bf = {"t32": 2, "t": 2, "bnd": 1, "p": 1, "v": 1, "vs": 1, "g": 1,
              "ob": 3}
    else:
        bf = {"t32": 3, "t": 2, "bnd": 2, "p": 1, "v": 2, "vs": 2, "g": 1,
              "ob": 3}
    if bufs:
        bf.update(bufs)
    emit = {"bf16a": _emit_pass_bf16a, "bf16b": _emit_pass_bf16b}.get(mode)
    with tile.TileContext(nc) as tc:
        with (
            tc.tile_pool(name="t32", bufs=bf["t32"]) as t32p,
            tc.tile_pool(name="t", bufs=bf["t"]) as tp,
            tc.tile_pool(name="bnd", bufs=bf["bnd"]) as bnd,
            tc.tile_pool(name="p", bufs=bf["p"]) as pp,
            tc.tile_pool(name="v", bufs=bf["v"]) as vt,
            tc.tile_pool(name="vs", bufs=bf["vs"]) as vsp,
            tc.tile_pool(name="g", bufs=bf["g"]) as gp,
            tc.tile_pool(name="ob", bufs=bf["ob"]) as obp,
        ):
            pools = (t32p, tp, bnd, pp, vt, vsp, gp, obp)
            for i in range(reps):
                src, dst = stage(i)
                if mode.startswith("cg"):
                    probe = mode[2:].lstrip("_") or "none"
                    _emit_pass_cg(
                        nc, pools, src, dst, C, H, W, in_f32=(i == 0),
                        probe=probe,
                    )
                else:
                    cc = cc_first if i == 0 else cc_chain
                    emit(nc, pools, src, dst, C, H, W, cc, rpp, in_f32=(i == 0))
    nc.compile()
    return nc


def _get_program(C, H, W, reps=1, mode="auto"):
    if mode == "auto":
        mode = "cg" if reps == 1 else "rs"
    key = (C, H, W, reps, mode)
    if key not in _cache:
        if mode == "rs":
            _cache[key] = build_erosion_rs(C, H, W, reps=reps)
        elif mode in ("bf16a", "bf16b") or mode.startswith("cg"):
            _cache[key] = build_erosion_bf16a(C, H, W, mode=mode, reps=reps)
        else:
            _cache[key] = build_erosion(C, H, W, reps=reps, mode=mode)
    return _cache[key]


def kernel(x, m):
    from concourse.bass_utils import run_bass_kernel_spmd

    m = int(np.asarray(m))
    x = np.ascontiguousarray(np.asarray(x), dtype=np.float32)
    B, C, H, W = x.shape
    if m <= 0:
        return x.copy()
    # erosion by a (2m+1)-square = m chained 3x3 erosion passes in one NEFF
    nc = _get_program(C, H, W, reps=m)
    n_cores = 8
    assert B == n_cores, f"expected batch {n_cores}, got {B}"
    in_maps = [{"x": x[b]} for b in range(n_cores)]
    res = run_bass_kernel_spmd(nc, in_maps, core_ids=list(range(n_cores)))
    # device output is bf16; widen to fp32 on the host
    return np.stack(
        [np.asarray(r["o"]).astype(np.float32) for r in res.results], axis=0
    )


if __name__ == "__main__":
    # small-scale CoreSim correctness check (no hardware needed)
    import ml_dtypes

    from concourse.bass_interp import CoreSim

    rng = np.random.default_rng(0)
    C, H, W = 2, 256, 64
    x = rng.standard_normal((C, H, W)).astype(np.float32)

    def bf16r(a):
        return a.astype(ml_dtypes.bfloat16).astype(np.float32)

    for reps, mode in ((1, "cg"), (2, "cg"), (4, "cg")):
        if mode == "cg":
            nc = build_erosion_bf16a(C, H, W, mode=mode, reps=reps)
        elif mode in ("bf16a", "bf16b"):
            nc = build_erosion_bf16a(
                C, H, W, mode=mode, cc_first=16, cc_chain=32, rpp=4, reps=reps
            )
        else:
            nc = build_erosion(C, H, W, cc=32, rpp=4, reps=reps, mode=mode)
        sim = CoreSim(nc)
        sim.tensor("x")[:] = x
        sim.simulate(check_with_hw=False)
        got = np.asarray(sim.tensor("o")).astype(np.float32)

        # bf16a converts the input to bf16 before the first pass
        exp = bf16r(x) if mode in ("bf16", "bf16a") else x
        for _ in range(reps):
            xp = np.pad(exp, ((0, 0), (1, 1), (1, 1)), constant_values=PAD)
            nxt = np.empty_like(exp)
            for i in range(H):
                for j in range(W):
                    nxt[:, i, j] = xp[:, i : i + 3, j : j + 3].min(axis=(1, 2))
            exp = bf16r(nxt)  # device stores bf16 each pass
        ok = np.array_equal(got, exp)
        rel = np.max(np.abs(got - exp) / np.maximum(np.abs(exp), 1e-6))
        print(f"CoreSim reps={reps} mode={mode} exact: {ok} rel={rel:.2e}")



# revision 36
# speedup vs baseline: 7.1131x; 1.0824x over previous
"""Trainium2 Bass kernel for 3x3 (k=2m+1) morphological erosion (sliding-window
min) over [B, C, H, W] fp32, B=8 sharded across 8 NeuronCores (one batch per
core).

v3 scheme (per core, shard = one batch of C=8 channel images, 1024x1024):
  - each partition holds RPP=16 consecutive image rows (ppi=64 partitions per
    image, 2 images per 128-partition stack), processed in CC=256-column
    chunks with a 1-column halo (PW=258).
  - the separable 3x3 min runs in bf16 on DVE at the 2x_1p rate (2-byte
    dtype + unit innermost stride). min never creates new values, so the
    total error is one bf16 rounding of the input (~2^-9 relative), far
    inside the 2e-2 gate.
  - the device kernel STORES bf16 (halves store traffic: 64MB -> 48MB/core
    round trip); kernel() widens to fp32 on the host after the gather.
  - ACT (scalar) engine does the fp32->bf16 input convert (first pass only;
    chained passes read bf16 scratch directly) and drives store DMAs; loads
    and the per-partition-block boundary-row halo copies ride the SP queue;
    PAD memsets go to Pool. Emission is software-pipelined (skew 2) so no
    in-order sequencer blocks a neighbor engine's next tile.
  - V pass: tmp[j] = min(row j, row j+1); interior v rows from tmp + row
    j+2; the 2 per-partition boundary rows take their missing neighbor from
    a small bf16 side tile bt (partition-shifted SBUF->SBUF DMA; image-edge
    partitions: PAD memset at block tops (legal start partitions 0/64),
    own-row duplication DMA at block bottoms).
  - Cost model: DMA engines ~147us, DVE ~141us, ACT ~55us -> balanced
    DMA/DVE at the bf16-store memory roofline.
  - m>1 runs as m chained passes (bf16 DRAM ping-pong) inside one NEFF.
"""

import sys

sys.path.insert(0, "/opt/trn_rl_repo")

import numpy as np

import concourse.bass as bass
import concourse.tile as tile
from concourse import bacc, mybir

PAD = 1.0e9
F32 = mybir.dt.float32
BF16 = mybir.dt.bfloat16
MIN = mybir.AluOpType.min

CC = 256  # column chunk width
RPP = 16  # image rows per partition

_cache = {}


def _emit_pass_f32eo2(nc, pools, x_d, o_d, C, H, W, cc, rpp, in_f32):
    """One erosion pass, fp32 compute, even/odd V pass over halo-loaded rows.

    Each partition loads R+2 rows (its R rows plus one halo row each side,
    overlapping DRAM reads; +2/R load bytes) so the V pass is exactly 3
    row-stepped DVE ops with no boundary fixups:
      p[j] = min(e[2j], e[2j+1])          j = 0..R/2
      v[2j] = min(p[j], e[2j+2])          j = 0..R/2-1
      v[2j+1] = min(e[2j+1], p[j+1])      j = 0..R/2-1
    Image-edge partitions: top halo row = PAD memset (start partitions
    0/64: legal), bottom halo row = own-last-row duplication DMA
    (min-idempotent). All slices keep unit innermost stride (full DVE
    rate). The final H op writes bf16 (free output cast); host widens.
    """
    from concourse.ap import AP as _AP

    ppi = H // rpp
    ips = max(1, 128 // ppi)
    inp, xbp, bnd, vtm, vt, htm, obp = pools
    R = rpp
    E = R + 2  # rows incl halo
    PW = cc + 2
    assert R % 2 == 0

    tiles = [(s0, c0) for s0 in range(0, C, ips) for c0 in range(0, W, cc)]
    front = {}

    def emit_front(i):
        s0, c0 = tiles[i]
        n_img = min(ips, C - s0)
        P = n_img * ppi
        last = c0 + cc == W
        t = inp.tile([128, E, PW], F32, name="t")
        if in_f32:
            ld = t
        else:
            ld = xbp.tile([128, E, PW], BF16, name="xb")
        wlo = max(c0 - 1, 0)
        whi = min(c0 + cc + 1, W)
        nw = whi - wlo
        dlo = 1 if c0 == 0 else 0
        for im in range(n_img):
            img = x_d[s0 + im]  # [H, W]
            p0 = im * ppi
            # interior partitions 1..ppi-2: rows rpp*p-1 .. rpp*p+rpp
            src = _AP(
                tensor=img.tensor,
                offset=img.offset + (rpp - 1) * W + wlo,
                ap=[[rpp * W, ppi - 2], [W, E], [1, nw]],
            )
            nc.sync.dma_start(
                ld[p0 + 1 : p0 + ppi - 1, :, dlo : dlo + nw], src
            )
            # top partition: rows 0..rpp into slots 1..E-1; slot 0 = PAD
            nc.sync.dma_start(
                ld[p0 : p0 + 1, 1:E, dlo : dlo + nw],
                img[0 : rpp + 1, wlo:whi].rearrange("(p r) w -> p r w", p=1),
            )
            nc.gpsimd.memset(ld[p0 : p0 + 1, 0:1, :], PAD)
            # bottom partition: rows H-rpp-1..H-1 into slots 0..E-2;
            # slot E-1 = duplicate of the image's last row (min-idempotent)
            pe = p0 + ppi - 1
            nc.sync.dma_start(
                ld[pe : pe + 1, 0 : E - 1, dlo : dlo + nw],
                img[H - rpp - 1 : H, wlo:whi].rearrange("(p r) w -> p r w", p=1),
            )
            nc.sync.dma_start(
                ld[pe : pe + 1, E - 1 : E, dlo : dlo + nw],
                img[H - 1 : H, wlo:whi].rearrange("(p r) w -> p r w", p=1),
            )
        if c0 == 0:
            nc.gpsimd.memset(ld[0:P, :, 0:1], PAD)
        if last:
            nc.gpsimd.memset(ld[0:P, :, PW - 1 : PW], PAD)
        if not in_f32:
            nc.scalar.copy(t[0:P], ld[0:P])
        front[i] = (t, n_img, P, s0, c0)

    def emit_back(i):
        t, n_img, P, s0, c0 = front.pop(i)
        Rh = R // 2
        v = vt.tile([128, R, PW], F32, name="v")
        p = vtm.tile([128, Rh + 1, PW], F32, name="p")
        nc.vector.tensor_tensor(
            out=p[0:P], in0=t[0:P, 0:E:2, :], in1=t[0:P, 1:E:2, :], op=MIN
        )
        nc.vector.tensor_tensor(
            out=v[0:P, 0:R:2, :],
            in0=p[0:P, 0:Rh, :],
            in1=t[0:P, 2:E:2, :],
            op=MIN,
        )
        nc.vector.tensor_tensor(
            out=v[0:P, 1:R:2, :],
            in0=t[0:P, 1 : E - 2 : 2, :],
            in1=p[0:P, 1 : Rh + 1, :],
            op=MIN,
        )

        h = htm.tile([128, R, cc + 1], F32, name="h")
        nc.vector.tensor_tensor(
            out=h[0:P], in0=v[0:P, :, 0 : cc + 1], in1=v[0:P, :, 1 : cc + 2],
            op=MIN,
        )
        ob = obp.tile([128, R, cc], BF16, name="ob")
        nc.vector.tensor_tensor(
            out=ob[0:P], in0=h[0:P, :, 0:cc], in1=v[0:P, :, 2 : cc + 2], op=MIN
        )

        for im in range(n_img):
            dst = o_d[s0 + im, :, c0 : c0 + cc].rearrange(
                "(p r) w -> p r w", p=ppi
            )
            p0 = im * ppi
            nc.scalar.dma_start(dst, ob[p0 : p0 + ppi, :, :])

    skew = 2
    for i in range(len(tiles) + skew):
        if i < len(tiles):
            emit_front(i)
        if i >= skew:
            emit_back(i - skew)


def _emit_pass_f32eo(nc, pools, x_d, o_d, C, H, W, cc, rpp, in_f32):
    """One erosion pass, fp32 compute with even/odd-shared V pass.

    x_d fp32 (first pass) or bf16 (chained); o_d bf16. All mins on DVE in
    fp32 (bf16 ALU is slower on real HW); V pass uses the pairwise-sharing
    decomposition (1.5 ops/elem): p[j] = min(r[2j], r[2j+1]);
    v[2j] = min(r[2j-1], p[j]); v[2j+1] = min(p[j], r[2j+2]). Row-stepped
    slices keep unit innermost stride (full DVE rate). The final H op
    writes bf16 (free output cast).
    """
    ppi = H // rpp
    ips = max(1, 128 // ppi)
    inp, xbp, bnd, vtm, vt, htm, obp = pools
    R = rpp
    PW = cc + 2
    assert R % 2 == 0

    tiles = [(s0, c0) for s0 in range(0, C, ips) for c0 in range(0, W, cc)]
    front = {}

    def emit_front(i):
        s0, c0 = tiles[i]
        n_img = min(ips, C - s0)
        P = n_img * ppi
        last = c0 + cc == W
        # load tile: fp32 on first pass, bf16 on chained passes; compute
        # stays fp32 either way (bf16 load is upconverted by ACT)
        t = inp.tile([128, R, PW], F32, name="t")
        if in_f32:
            ld = t
        else:
            ld = xbp.tile([128, R, PW], BF16, name="xb")
        wlo = max(c0 - 1, 0)
        whi = min(c0 + cc + 1, W)
        dlo = 1 if c0 == 0 else 0
        for im in range(n_img):
            src = x_d[s0 + im, :, wlo:whi].rearrange("(p r) w -> p r w", p=ppi)
            p0 = im * ppi
            nc.sync.dma_start(ld[p0 : p0 + ppi, :, dlo : dlo + (whi - wlo)], src)
        if c0 == 0:
            nc.gpsimd.memset(ld[0:P, :, 0:1], PAD)
        if last:
            nc.gpsimd.memset(ld[0:P, :, PW - 1 : PW], PAD)
        if not in_f32:
            nc.scalar.copy(t[0:P], ld[0:P])
        front[i] = (t, n_img, P, s0, c0)

    def emit_back(i):
        t, n_img, P, s0, c0 = front.pop(i)
        bt = bnd.tile([128, 2, PW], F32, name="bt")
        for im in range(n_img):
            p0 = im * ppi
            pe = p0 + ppi - 1
            nc.sync.dma_start(bt[p0:pe, 0:1, :], t[p0 + 1 : pe + 1, 0:1, :])
            nc.sync.dma_start(
                bt[p0 + 1 : pe + 1, 1:2, :], t[p0:pe, R - 1 : R, :]
            )
            nc.sync.dma_start(
                bt[pe : pe + 1, 0:1, :], t[pe : pe + 1, R - 1 : R, :]
            )
            nc.gpsimd.memset(bt[p0 : p0 + 1, 1:2, :], PAD)

        # ---- V pass, even/odd shared (fp32) ----
        v = vt.tile([128, R, PW], F32, name="v")
        p = vtm.tile([128, R // 2, PW], F32, name="p")
        Rh = R // 2
        nc.vector.tensor_tensor(
            out=p[0:P], in0=t[0:P, 0:R:2, :], in1=t[0:P, 1:R:2, :], op=MIN
        )
        # v[2j] = min(r[2j-1], p[j]), j=1..Rh-1 (j=0 uses bt row-above)
        nc.vector.tensor_tensor(
            out=v[0:P, 2:R:2, :],
            in0=t[0:P, 1 : R - 2 : 2, :],
            in1=p[0:P, 1:Rh, :],
            op=MIN,
        )
        nc.vector.tensor_tensor(
            out=v[0:P, 0:1, :], in0=p[0:P, 0:1, :], in1=bt[0:P, 1:2, :], op=MIN
        )
        # v[2j+1] = min(p[j], r[2j+2]), j=0..Rh-2 (j=Rh-1 uses bt row-below)
        nc.vector.tensor_tensor(
            out=v[0:P, 1 : R - 1 : 2, :],
            in0=p[0:P, 0 : Rh - 1, :],
            in1=t[0:P, 2:R:2, :],
            op=MIN,
        )
        nc.vector.tensor_tensor(
            out=v[0:P, R - 1 : R, :],
            in0=p[0:P, Rh - 1 : Rh, :],
            in1=bt[0:P, 0:1, :],
            op=MIN,
        )

        # ---- H pass (fp32, last op casts to bf16 on output) ----
        h = htm.tile([128, R, cc + 1], F32, name="h")
        nc.vector.tensor_tensor(
            out=h[0:P], in0=v[0:P, :, 0 : cc + 1], in1=v[0:P, :, 1 : cc + 2],
            op=MIN,
        )
        ob = obp.tile([128, R, cc], BF16, name="ob")
        nc.vector.tensor_tensor(
            out=ob[0:P], in0=h[0:P, :, 0:cc], in1=v[0:P, :, 2 : cc + 2], op=MIN
        )

        for im in range(n_img):
            dst = o_d[s0 + im, :, c0 : c0 + cc].rearrange(
                "(p r) w -> p r w", p=ppi
            )
            p0 = im * ppi
            nc.scalar.dma_start(dst, ob[p0 : p0 + ppi, :, :])

    skew = 2
    for i in range(len(tiles) + skew):
        if i < len(tiles):
            emit_front(i)
        if i >= skew:
            emit_back(i - skew)


def _emit_pass_cg(nc, pools, x_d, o_d, C, H, W, in_f32, probe="none"):
    """One erosion pass, contiguous-big-DMA layout + bf16 DVE compute.

    HW measurements drove this shape:
      - DVE TT min bf16 = 0.372 ns/elem-pp (0.415 odd-offset) vs fp32 1.097
        -> all mins in bf16.
      - DMA throughput is descriptor-dominated: the old [ppi,R,~1KB-line]
        strided loads measured ~45 GB/s; >=16KB contiguous-per-partition
        descriptors reach ~350-400 GB/s.
    So: one image per 128-partition stack, partition p holds R=H/128 FULL
    rows (contiguous 8KB-32KB span in DRAM and in SBUF) -> ONE descriptor
    per partition per image load/store. Per image per pass: 1 big load
    (sync queue), 2 partition-shifted SBUF->SBUF row copies for the V-pass
    boundary rows (scalar queue), 1 big store (scalar queue). Image-edge
    partitions get their boundary row via tiny 1-partition DVE ops reading
    the tile's own edge row (min-idempotent duplicate) -- no memsets, no
    PAD columns: the H-pass edge columns use dedicated 1-column ops.
    """
    R = H // 128
    Rh = R // 2
    assert H % 128 == 0 and R % 2 == 0
    t32p, tp, bnd, pp, vt, _vsp, hp, obp = pools
    front = {}

    def emit_front(i):
        from concourse.ap import AP as _AP

        t = tp.tile([128, R, W], BF16, name="t")
        if in_f32:
            # fp32 staging in two half-tiles: halves the t32 SBUF slot so
            # the bf16 pipeline pools can run deeper
            Rhalf = R // 2
            img = x_d[i]
            for c in range(2):
                ld = t32p.tile([128, Rhalf, W], F32, name="t32")
                src_c = _AP(
                    tensor=img.tensor,
                    offset=img.offset + c * Rhalf * W,
                    ap=[[R * W, 128], [1, Rhalf * W]],
                )
                nc.sync.dma_start(ld.rearrange("p r w -> p (r w)"), src_c)
                nc.scalar.copy(t[:, c * Rhalf : (c + 1) * Rhalf, :], ld)
        else:
            src = x_d[i].rearrange("(p r) w -> p (r w)", p=128)
            nc.sync.dma_start(t.rearrange("p r w -> p (r w)"), src)
        # bt[p,0] = row above (prev partition's last row),
        # bt[p,1] = row below (next partition's first row)
        if probe == "dma":
            front[i] = (t, None)
            return
        bt = bnd.tile([128, 2, W], BF16, name="bt")
        nc.scalar.dma_start(bt[1:128, 0:1, :], t[0:127, R - 1 : R, :])
        nc.scalar.dma_start(bt[0:127, 1:2, :], t[1:128, 0:1, :])
        # image-edge partitions duplicate their own edge row (the DVE
        # rejects partition starts other than 0/32/64/96, so these are DMAs)
        nc.scalar.dma_start(bt[0:1, 0:1, :], t[0:1, 0:1, :])
        nc.scalar.dma_start(bt[127:128, 1:2, :], t[127:128, R - 1 : R, :])
        front[i] = (t, bt)

    def emit_back(i):
        t, bt = front.pop(i)
        if probe == "dma":  # timing probe: load->store only
            dst = o_d[i].rearrange("(p r) w -> p (r w)", p=128)
            nc.scalar.dma_start(dst, t.rearrange("p r w -> p (r w)"))
            return
        # ---- V pass (bf16; all aligned) ----
        p = pp.tile([128, Rh, W], BF16, name="p")
        v = vt.tile([128, R, W], BF16, name="v")
        nc.vector.tensor_tensor(
            out=p, in0=t[:, 0:R:2, :], in1=t[:, 1:R:2, :], op=MIN
        )
        if Rh > 1:
            # v[2j] = min(r[2j-1], p[j]), j=1..Rh-1
            nc.vector.tensor_tensor(
                out=v[:, 2:R:2, :],
                in0=t[:, 1 : R - 2 : 2, :],
                in1=p[:, 1:Rh, :],
                op=MIN,
            )
            # v[2j+1] = min(p[j], r[2j+2]), j=0..Rh-2
            nc.vector.tensor_tensor(
                out=v[:, 1 : R - 1 : 2, :],
                in0=p[:, 0 : Rh - 1, :],
                in1=t[:, 2:R:2, :],
                op=MIN,
            )
        # v[0] = min(p[0], row-above); v[R-1] = min(p[Rh-1], row-below)
        if Rh >= 2:
            nc.vector.tensor_tensor(
                out=v[:, 0 : R : R - 1, :],
                in0=p[:, 0 : Rh : Rh - 1, :],
                in1=bt[:, 0:2, :],
                op=MIN,
            )
        else:
            nc.vector.tensor_tensor(
                out=v[:, 0:1, :], in0=p[:, 0:1, :], in1=bt[:, 0:1, :], op=MIN
            )
            nc.vector.tensor_tensor(
                out=v[:, R - 1 : R, :],
                in0=p[:, Rh - 1 : Rh, :],
                in1=bt[:, 1:2, :],
                op=MIN,
            )
        if probe == "noh":  # timing probe: V pass only, store v
            dst = o_d[i].rearrange("(p r) w -> p (r w)", p=128)
            nc.scalar.dma_start(dst, v.rearrange("p r w -> p (r w)"))
            return
        # ---- H pass: h[c] = min(v[c], v[c+1]); ob interior + edge cols ----
        h = hp.tile([128, R, W], BF16, name="h")
        nc.vector.tensor_tensor(
            out=h[:, :, 0 : W - 1],
            in0=v[:, :, 0 : W - 1],
            in1=v[:, :, 1:W],
            op=MIN,
        )
        ob = obp.tile([128, R, W], BF16, name="ob")
        nc.vector.tensor_tensor(
            out=ob[:, :, 1 : W - 1],
            in0=h[:, :, 0 : W - 2],
            in1=v[:, :, 2:W],
            op=MIN,
        )
        nc.vector.tensor_tensor(
            out=ob[:, :, 0:1], in0=h[:, :, 0:1], in1=v[:, :, 1:2], op=MIN
        )
        nc.vector.tensor_copy(
            out=ob[:, :, W - 1 : W], in_=h[:, :, W - 2 : W - 1]
        )
        dst = o_d[i].rearrange("(p r) w -> p (r w)", p=128)
        nc.scalar.dma_start(dst, ob.rearrange("p r w -> p (r w)"))

    skew = 2
    for i in range(C + skew):
        if i < C:
            emit_front(i)
        if i >= skew:
            emit_back(i - skew)


def build_erosion_rs(C, H, W, reps):
    """SBUF-resident multi-pass erosion: the whole per-core shard (C images
    x H/128 rows x W cols, bf16 = 16 KiB/partition/image) stays in SBUF
    across all passes.

    Load once (gpsimd casting DMA, fp32->bf16 inline), run `reps` erosion
    passes entirely on-chip (DVE mins + 4 small SBUF->SBUF boundary-row
    DMAs per image per pass), store once. Per-pass marginal cost is pure
    DVE (~90 us) -- no HBM traffic per pass. The image pool has C+1
    slots; each pass writes image i's result into the rotating spare slot
    and the old buffer becomes the next spare (Tile's WAR deps make the
    rotation safe).
    """
    R = H // 128
    Rh = R // 2
    assert H % 256 == 0 and R % 2 == 0 and reps >= 1

    nc = bacc.Bacc("TRN2", target_bir_lowering=False, debug=False, num_devices=1)
    x_d = nc.dram_tensor("x", [C, H, W], F32, kind="ExternalInput").ap()
    o_d = nc.dram_tensor("o", [C, H, W], BF16, kind="ExternalOutput").ap()

    with tile.TileContext(nc) as tc:
        with (
            tc.tile_pool(name="img", bufs=C + 1) as imgp,
            tc.tile_pool(name="bnd", bufs=3) as bnd,
            tc.tile_pool(name="p", bufs=1) as pp,
            tc.tile_pool(name="v", bufs=1) as vt,
            tc.tile_pool(name="h", bufs=1) as hp,
        ):
            img_t = []
            for i in range(C):
                t = imgp.tile([128, R, W], BF16, name="img")
                nc.gpsimd.dma_start(
                    t.rearrange("p r w -> p (r w)"),
                    x_d[i].rearrange("(p r) w -> p (r w)", p=128),
                )
                img_t.append(t)

            bts = {}

            def emit_bt(i):
                t = img_t[i]
                bt = bnd.tile([128, 2, W], BF16, name="bt")
                nc.scalar.dma_start(bt[1:128, 0:1, :], t[0:127, R - 1 : R, :])
                nc.scalar.dma_start(bt[0:127, 1:2, :], t[1:128, 0:1, :])
                nc.scalar.dma_start(bt[0:1, 0:1, :], t[0:1, 0:1, :])
                nc.scalar.dma_start(
                    bt[127:128, 1:2, :], t[127:128, R - 1 : R, :]
                )
                bts[i] = bt

            def emit_compute(i):
                t = img_t[i]
                bt = bts.pop(i)
                p = pp.tile([128, Rh, W], BF16, name="p")
                v = vt.tile([128, R, W], BF16, name="v")
                nc.vector.tensor_tensor(
                    out=p, in0=t[:, 0:R:2, :], in1=t[:, 1:R:2, :], op=MIN
                )
                if Rh > 1:
                    nc.vector.tensor_tensor(
                        out=v[:, 2:R:2, :],
                        in0=t[:, 1 : R - 2 : 2, :],
                        in1=p[:, 1:Rh, :],
                        op=MIN,
                    )
                    nc.vector.tensor_tensor(
                        out=v[:, 1 : R - 1 : 2, :],
                        in0=p[:, 0 : Rh - 1, :],
                        in1=t[:, 2:R:2, :],
                        op=MIN,
                    )
                if Rh >= 2:
                    # fused boundary rows: out rows {0, R-1}, p rows
                    # {0, Rh-1}, bt rows {0, 1} -- one strided op
                    nc.vector.tensor_tensor(
                        out=v[:, 0 : R : R - 1, :],
                        in0=p[:, 0 : Rh : Rh - 1, :],
                        in1=bt[:, 0:2, :],
                        op=MIN,
                    )
                else:
                    nc.vector.tensor_tensor(
                        out=v[:, 0:1, :], in0=p[:, 0:1, :],
                        in1=bt[:, 0:1, :], op=MIN,
                    )
                    nc.vector.tensor_tensor(
                        out=v[:, R - 1 : R, :],
                        in0=p[:, Rh - 1 : Rh, :],
                        in1=bt[:, 1:2, :],
                        op=MIN,
                    )
                h = hp.tile([128, R, W], BF16, name="h")
                nc.vector.tensor_tensor(
                    out=h[:, :, 0 : W - 1],
                    in0=v[:, :, 0 : W - 1],
                    in1=v[:, :, 1:W],
                    op=MIN,
                )
                nxt = imgp.tile([128, R, W], BF16, name="img")
                nc.vector.tensor_tensor(
                    out=nxt[:, :, 1 : W - 1],
                    in0=h[:, :, 0 : W - 2],
                    in1=v[:, :, 2:W],
                    op=MIN,
                )
                nc.vector.tensor_tensor(
                    out=nxt[:, :, 0:1], in0=h[:, :, 0:1], in1=v[:, :, 1:2],
                    op=MIN,
                )
                nc.vector.tensor_copy(
                    out=nxt[:, :, W - 1 : W], in_=h[:, :, W - 2 : W - 1]
                )
                img_t[i] = nxt

            for k in range(reps):
                for i in range(C + 1):
                    if i < C:
                        emit_bt(i)
                    if i >= 1:
                        emit_compute(i - 1)

            for i in range(C):
                nc.sync.dma_start(
                    o_d[i].rearrange("(p r) w -> p (r w)", p=128),
                    img_t[i].rearrange("p r w -> p (r w)"),
                )
    nc.compile()
    return nc


def _emit_pass_bf16b(nc, pools, x_d, o_d, C, H, W, cc, rpp, in_f32):
    """One erosion pass, pure-DVE bf16 compute, halo-row loads (f32eo2
    style), no ACT compute and no SBUF->SBUF boundary DMAs.

    Measured on HW (microbench): TT min bf16 = 0.372 ns/elem-per-partition
    when 4B-aligned and 0.415 when one operand starts at an odd column --
    both ~2.9x faster than fp32 (1.097). So compute everything in bf16 on
    the DVE: V pass = even/odd shared decomposition over the halo-loaded
    rows (all aligned), H pass = h(odd operand) then ob (aligned). The
    only ACT work is the fp32->bf16 convert on the first pass; stores
    ride the ACT queue as plain triggers.
    """
    from concourse.ap import AP as _AP

    ppi = H // rpp
    ips = max(1, 128 // ppi)
    t32p, tp, _bnd, pp, vt, _vsp, hp, obp = pools
    R = rpp
    Rh = R // 2
    E = R + 2
    PW = cc + 2
    assert R % 2 == 0 and cc % 2 == 0

    tiles = [(s0, c0) for s0 in range(0, C, ips) for c0 in range(0, W, cc)]
    front = {}

    def emit_front(i):
        s0, c0 = tiles[i]
        n_img = min(ips, C - s0)
        P = n_img * ppi
        last = c0 + cc == W
        t = tp.tile([128, E, PW], BF16, name="t")
        ld = t32p.tile([128, E, PW], F32, name="t32") if in_f32 else t
        wlo = max(c0 - 1, 0)
        whi = min(c0 + cc + 1, W)
        nw = whi - wlo
        dlo = 1 if c0 == 0 else 0
        for im in range(n_img):
            img = x_d[s0 + im]  # [H, W]
            p0 = im * ppi
            # interior partitions 1..ppi-2: rows rpp*p-1 .. rpp*p+rpp
            src = _AP(
                tensor=img.tensor,
                offset=img.offset + (rpp - 1) * W + wlo,
                ap=[[rpp * W, ppi - 2], [W, E], [1, nw]],
            )
            nc.sync.dma_start(ld[p0 + 1 : p0 + ppi - 1, :, dlo : dlo + nw], src)
            # top partition: rows 0..rpp into slots 1..E-1; slot 0 = row 0
            # again (min-idempotent duplicate stands in for the PAD row)
            nc.sync.dma_start(
                ld[p0 : p0 + 1, 1:E, dlo : dlo + nw],
                img[0 : rpp + 1, wlo:whi].rearrange("(p r) w -> p r w", p=1),
            )
            nc.sync.dma_start(
                ld[p0 : p0 + 1, 0:1, dlo : dlo + nw],
                img[0:1, wlo:whi].rearrange("(p r) w -> p r w", p=1),
            )
            # bottom partition: rows H-rpp-1..H-1 into slots 0..E-2;
            # slot E-1 = duplicate of the image's last row
            pe = p0 + ppi - 1
            nc.sync.dma_start(
                ld[pe : pe + 1, 0 : E - 1, dlo : dlo + nw],
                img[H - rpp - 1 : H, wlo:whi].rearrange("(p r) w -> p r w", p=1),
            )
            nc.sync.dma_start(
                ld[pe : pe + 1, E - 1 : E, dlo : dlo + nw],
                img[H - 1 : H, wlo:whi].rearrange("(p r) w -> p r w", p=1),
            )
        if c0 == 0:
            nc.gpsimd.memset(ld[0:P, :, 0:1], PAD)
        if last:
            nc.gpsimd.memset(ld[0:P, :, PW - 1 : PW], PAD)
        if in_f32:
            nc.scalar.copy(t[0:P], ld[0:P])
        front[i] = (t, n_img, P, s0, c0)

    def emit_back(i):
        t, n_img, P, s0, c0 = front.pop(i)
        # ---- V pass over halo rows (bf16, aligned -> fast) ----
        p = pp.tile([128, Rh + 1, PW], BF16, name="p")
        v = vt.tile([128, R, PW], BF16, name="v")
        nc.vector.tensor_tensor(
            out=p[0:P], in0=t[0:P, 0:E:2, :], in1=t[0:P, 1:E:2, :], op=MIN
        )
        nc.vector.tensor_tensor(
            out=v[0:P, 0:R:2, :],
            in0=p[0:P, 0:Rh, :],
            in1=t[0:P, 2:E:2, :],
            op=MIN,
        )
        nc.vector.tensor_tensor(
            out=v[0:P, 1:R:2, :],
            in0=t[0:P, 1 : E - 2 : 2, :],
            in1=p[0:P, 1 : Rh + 1, :],
            op=MIN,
        )
        # ---- H pass: h has one odd-offset operand (12% slower, fine) ----
        h = hp.tile([128, R, PW], BF16, name="h")
        nc.vector.tensor_tensor(
            out=h[0:P, :, 0 : cc + 1],
            in0=v[0:P, :, 0 : cc + 1],
            in1=v[0:P, :, 1 : cc + 2],
            op=MIN,
        )
        ob = obp.tile([128, R, cc], BF16, name="ob")
        nc.vector.tensor_tensor(
            out=ob[0:P], in0=h[0:P, :, 0:cc], in1=v[0:P, :, 2 : cc + 2], op=MIN
        )
        for im in range(n_img):
            dst = o_d[s0 + im, :, c0 : c0 + cc].rearrange(
                "(p r) w -> p r w", p=ppi
            )
            p0 = im * ppi
            nc.scalar.dma_start(dst, ob[p0 : p0 + ppi, :, :])

    skew = 2
    for i in range(len(tiles) + skew):
        if i < len(tiles):
            emit_front(i)
        if i >= skew:
            emit_back(i - skew)


def _emit_pass_bf16a(nc, pools, x_d, o_d, C, H, W, cc, rpp, in_f32):
    """One erosion pass, all-bf16 DVE compute with every tensor_tensor
    operand 4B-aligned so the DVE runs at the 2x_1p rate throughout.

    The 2x_1p perf mode needs 16-bit dtype, unit innermost stride AND a
    4-byte-aligned start address for every operand. The H pass combines
    columns of both parities, so one op would always have a 2-byte-offset
    (odd) operand and silently drop to 1x. Fix: the otherwise-idle ACT
    engine makes a one-column-shifted copy vs[c] = v[c+1] (ACT is 1x at
    1.2 GHz regardless of alignment), and the DVE computes
        g[c]  = min(v[c], v[c+2])   (both even offsets, 2x)
        ob[c] = min(g[c], vs[c])    (both aligned, 2x)
    V pass is the even/odd-shared decomposition (1.5 ops/elem, all row
    slices start at even column 0 -> aligned, 2x). Per-partition boundary
    rows come from a small side tile bt filled by partition-shifted
    SBUF->SBUF DMAs; image edges use own-row duplication (min-idempotent)
    so no memsets on partition starts other than 0 are needed.

    DVE ~1.75 cyc/elem (vs 3.5 at 1x fp32), ACT 1 copy/elem (+1 convert
    on the first pass), DMA loads R rows/partition (no halo rows).
    """
    ppi = H // rpp
    ips = max(1, 128 // ppi)
    t32p, tp, bnd, pp, vt, vsp, gp, obp = pools
    R = rpp
    Rh = R // 2
    PW = cc + 2
    assert R % 2 == 0 and cc % 2 == 0 and PW % 2 == 0

    tiles = [(s0, c0) for s0 in range(0, C, ips) for c0 in range(0, W, cc)]
    front = {}
    mid = {}

    def emit_front(i):
        s0, c0 = tiles[i]
        n_img = min(ips, C - s0)
        P = n_img * ppi
        last = c0 + cc == W
        t = tp.tile([128, R, PW], BF16, name="t")
        ld = t32p.tile([128, R, PW], F32, name="t32") if in_f32 else t
        wlo = max(c0 - 1, 0)
        whi = min(c0 + cc + 1, W)
        dlo = 1 if c0 == 0 else 0
        for im in range(n_img):
            src = x_d[s0 + im, :, wlo:whi].rearrange("(p r) w -> p r w", p=ppi)
            p0 = im * ppi
            nc.sync.dma_start(ld[p0 : p0 + ppi, :, dlo : dlo + (whi - wlo)], src)
        if c0 == 0:
            nc.gpsimd.memset(ld[0:P, :, 0:1], PAD)
        if last:
            nc.gpsimd.memset(ld[0:P, :, PW - 1 : PW], PAD)
        if in_f32:
            nc.scalar.copy(t[0:P], ld[0:P])
        front[i] = (t, n_img, P, s0, c0)

    def emit_mid(i):
        t, n_img, P, s0, c0 = front.pop(i)
        # bt[p,0] = row below the block (next partition's row 0),
        # bt[p,1] = row above (prev partition's row R-1); image edges
        # duplicate the block's own edge row (min-idempotent).
        bt = bnd.tile([128, 2, PW], BF16, name="bt")
        for im in range(n_img):
            p0 = im * ppi
            pe = p0 + ppi - 1
            nc.sync.dma_start(bt[p0:pe, 0:1, :], t[p0 + 1 : pe + 1, 0:1, :])
            nc.sync.dma_start(bt[pe : pe + 1, 0:1, :], t[pe : pe + 1, R - 1 : R, :])
            nc.sync.dma_start(bt[p0 + 1 : pe + 1, 1:2, :], t[p0:pe, R - 1 : R, :])
            nc.sync.dma_start(bt[p0 : p0 + 1, 1:2, :], t[p0 : p0 + 1, 0:1, :])

        # ---- V pass (bf16, all operands 4B-aligned -> 2x) ----
        p = pp.tile([128, Rh, PW], BF16, name="p")
        v = vt.tile([128, R, PW], BF16, name="v")
        nc.vector.tensor_tensor(
            out=p[0:P], in0=t[0:P, 0:R:2, :], in1=t[0:P, 1:R:2, :], op=MIN
        )
        if Rh > 1:
            nc.vector.tensor_tensor(
                out=v[0:P, 2:R:2, :],
                in0=t[0:P, 1 : R - 2 : 2, :],
                in1=p[0:P, 1:Rh, :],
                op=MIN,
            )
            nc.vector.tensor_tensor(
                out=v[0:P, 1 : R - 1 : 2, :],
                in0=p[0:P, 0 : Rh - 1, :],
                in1=t[0:P, 2:R:2, :],
                op=MIN,
            )
        nc.vector.tensor_tensor(
            out=v[0:P, 0:1, :], in0=p[0:P, 0:1, :], in1=bt[0:P, 1:2, :], op=MIN
        )
        nc.vector.tensor_tensor(
            out=v[0:P, R - 1 : R, :],
            in0=p[0:P, Rh - 1 : Rh, :],
            in1=bt[0:P, 0:1, :],
            op=MIN,
        )
        # ACT: parity-fixing shifted copy (1x, alignment-agnostic)
        vs = vsp.tile([128, R, cc], BF16, name="vs")
        nc.scalar.copy(vs[0:P], v[0:P, :, 1 : cc + 1])
        mid[i] = (v, vs, n_img, P, s0, c0)

    def emit_tail(i):
        v, vs, n_img, P, s0, c0 = mid.pop(i)
        g = gp.tile([128, R, cc], BF16, name="g")
        nc.vector.tensor_tensor(
            out=g[0:P], in0=v[0:P, :, 0:cc], in1=v[0:P, :, 2 : cc + 2], op=MIN
        )
        ob = obp.tile([128, R, cc], BF16, name="ob")
        nc.vector.tensor_tensor(out=ob[0:P], in0=g[0:P], in1=vs[0:P], op=MIN)
        for im in range(n_img):
            dst = o_d[s0 + im, :, c0 : c0 + cc].rearrange(
                "(p r) w -> p r w", p=ppi
            )
            p0 = im * ppi
            nc.scalar.dma_start(dst, ob[p0 : p0 + ppi, :, :])

    # 3-stage software pipeline: tail(i) is emitted after mid(i+1), so the
    # DVE's ob(i) lands behind V(i+1) in its in-order stream -- by then the
    # ACT's vs(i) is long done and the DVE never stalls on the ACT.
    n = len(tiles)
    for i in range(n + 2):
        if i < n:
            emit_front(i)
        if 1 <= i <= n:
            emit_mid(i - 1)
        if i >= 2:
            emit_tail(i - 2)


def _emit_pass(nc, pools, x_d, o_d, C, H, W, cc, rpp, in_f32):
    """Emit one full erosion pass x_d -> o_d into the open TileContext.

    in_f32: x_d is fp32 and must be converted to bf16 on ACT; otherwise
    x_d is bf16 and is used directly. o_d is always bf16.
    """
    ppi = H // rpp  # partitions per image
    ips = max(1, 128 // ppi)  # images per partition-stack
    inp, xbp, bnd, vtm, vt, htm, obp = pools
    R = rpp
    PW = cc + 2  # padded tile width

    tiles = [(s0, c0) for s0 in range(0, C, ips) for c0 in range(0, W, cc)]
    front = {}

    def emit_front(i):
        s0, c0 = tiles[i]
        n_img = min(ips, C - s0)
        P = n_img * ppi
        last = c0 + cc == W
        t = inp.tile([128, R, PW], F32, name="t") if in_f32 else None
        xb = xbp.tile([128, R, PW], BF16, name="xb")
        ld = t if in_f32 else xb
        wlo = max(c0 - 1, 0)
        whi = min(c0 + cc + 1, W)
        dlo = 1 if c0 == 0 else 0
        for im in range(n_img):
            src = x_d[s0 + im, :, wlo:whi].rearrange("(p r) w -> p r w", p=ppi)
            p0 = im * ppi
            nc.sync.dma_start(ld[p0 : p0 + ppi, :, dlo : dlo + (whi - wlo)], src)
        if c0 == 0:
            nc.gpsimd.memset(ld[0:P, :, 0:1], PAD)
        if last:
            nc.gpsimd.memset(ld[0:P, :, PW - 1 : PW], PAD)
        if in_f32:
            nc.scalar.copy(xb[0:P], t[0:P])
        front[i] = (xb, n_img, P, s0, c0)

    def emit_back(i):
        xb, n_img, P, s0, c0 = front.pop(i)
        # boundary-row side tile (bf16): bt[p,0] = first row of the block
        # below (xb[p+1] row 0), bt[p,1] = last row of the block above
        # (xb[p-1] row R-1); image-edge partitions: top -> PAD memset
        # (start partition p0 is 0 mod ppi>=32: legal), bottom -> own-row
        # duplication DMA (min-idempotent; DMA has no start-partition rule).
        bt = bnd.tile([128, 2, PW], BF16)
        for im in range(n_img):
            p0 = im * ppi
            pe = p0 + ppi - 1  # last partition of this image
            nc.sync.dma_start(bt[p0:pe, 0:1, :], xb[p0 + 1 : pe + 1, 0:1, :])
            nc.sync.dma_start(
                bt[p0 + 1 : pe + 1, 1:2, :], xb[p0:pe, R - 1 : R, :]
            )
            nc.sync.dma_start(
                bt[pe : pe + 1, 0:1, :], xb[pe : pe + 1, R - 1 : R, :]
            )
            nc.gpsimd.memset(bt[p0 : p0 + 1, 1:2, :], PAD)

        # ---- V pass (bf16, 2x DVE): v[r] = min(row r-1, r, r+1) ----
        v = vt.tile([128, R, PW], BF16)
        tmp = vtm.tile([128, R - 1, PW], BF16)
        nc.vector.tensor_tensor(
            out=tmp[0:P], in0=xb[0:P, 0 : R - 1, :], in1=xb[0:P, 1:R, :], op=MIN
        )
        nc.vector.tensor_tensor(
            out=v[0:P, 1 : R - 1, :],
            in0=tmp[0:P, 0 : R - 2, :],
            in1=xb[0:P, 2:R, :],
            op=MIN,
        )
        nc.vector.tensor_tensor(
            out=v[0:P, 0:1, :], in0=tmp[0:P, 0:1, :], in1=bt[0:P, 1:2, :], op=MIN
        )
        nc.vector.tensor_tensor(
            out=v[0:P, R - 1 : R, :],
            in0=tmp[0:P, R - 2 : R - 1, :],
            in1=bt[0:P, 0:1, :],
            op=MIN,
        )

        # ---- H pass (bf16, 2x DVE): o[c] = min(v[c], v[c+1], v[c+2]) ----
        h = htm.tile([128, R, cc + 1], BF16)
        nc.vector.tensor_tensor(
            out=h[0:P], in0=v[0:P, :, 0 : cc + 1], in1=v[0:P, :, 1 : cc + 2],
            op=MIN,
        )
        ob = obp.tile([128, R, cc], BF16)
        nc.vector.tensor_tensor(
            out=ob[0:P], in0=h[0:P, :, 0:cc], in1=v[0:P, :, 2 : cc + 2], op=MIN
        )

        # store bf16 from ACT's queue (host widens to fp32 after gather)
        for im in range(n_img):
            dst = o_d[s0 + im, :, c0 : c0 + cc].rearrange(
                "(p r) w -> p r w", p=ppi
            )
            p0 = im * ppi
            nc.scalar.dma_start(dst, ob[p0 : p0 + ppi, :, :])

    # software-pipelined emission: tile i+skew's load/convert lands in
    # every queue before tile i's compute/store, so ACT's in-order
    # sequencer never delays DVE's next tile.
    skew = 2
    for i in range(len(tiles) + skew):
        if i < len(tiles):
            emit_front(i)
        if i >= skew:
            emit_back(i - skew)


def build_erosion(C, H, W, cc=None, rpp=RPP, reps=1, bufs=None, mode="f32eo"):
    """Per-core Bass program: x [C,H,W] f32 -> o [C,H,W] bf16, erosion^reps."""
    if cc is None:
        # f32eo's fp32 tiles need cc=512 to amortize per-op overhead while
        # fitting SBUF; chained (reps>1) builds add a bf16 load pool, so
        # drop to 256 columns.
        cc = 512 if (mode.startswith("f32eo") and reps == 1) else 256
    assert H % rpp == 0
    ppi = H // rpp
    assert ppi <= 128 and W % cc == 0

    nc = bacc.Bacc("TRN2", target_bir_lowering=False, debug=False, num_devices=1)
    x_d = nc.dram_tensor("x", [C, H, W], F32, kind="ExternalInput").ap()
    o_d = nc.dram_tensor("o", [C, H, W], BF16, kind="ExternalOutput").ap()
    # ping-pong DRAM scratch (bf16) for chained passes
    s_d = [
        nc.dram_tensor(f"scratch{i}", [C, H, W], BF16, kind="Internal").ap()
        for i in range(min(2, max(0, reps - 1)))
    ]

    def stage(i):
        src = x_d if i == 0 else s_d[(i - 1) % 2]
        dst = o_d if i == reps - 1 else s_d[i % 2]
        return src, dst

    if mode.startswith("f32eo"):
        bf = {"inp": 2, "xb": 2, "bnd": 2, "vtm": 1, "vt": 1, "htm": 1, "ob": 2}
    else:
        bf = {"inp": 4, "xb": 3, "bnd": 2, "vtm": 1, "vt": 1, "htm": 1, "ob": 3}
    if bufs:
        bf.update(bufs)
    emit = {
        "f32eo": _emit_pass_f32eo,
        "f32eo2": _emit_pass_f32eo2,
        "bf16": _emit_pass,
    }[mode]
    with tile.TileContext(nc) as tc:
        with (
            tc.tile_pool(name="inp", bufs=bf["inp"]) as inp,
            tc.tile_pool(name="xb", bufs=bf["xb"]) as xbp,
            tc.tile_pool(name="bnd", bufs=bf["bnd"]) as bnd,
            tc.tile_pool(name="vtm", bufs=bf["vtm"]) as vtm,
            tc.tile_pool(name="vt", bufs=bf["vt"]) as vt,
            tc.tile_pool(name="htm", bufs=bf["htm"]) as htm,
            tc.tile_pool(name="ob", bufs=bf["ob"]) as obp,
        ):
            pools = (inp, xbp, bnd, vtm, vt, htm, obp)
            for i in range(reps):
                src, dst = stage(i)
                emit(nc, pools, src, dst, C, H, W, cc, rpp, in_f32=(i == 0))
    nc.compile()
    return nc


def build_erosion_bf16a(
    C, H, W, mode="bf16b", cc_first=256, cc_chain=None, rpp=RPP, reps=1,
    bufs=None,
):
    """bf16-mode program builder (modes bf16a / bf16b).

    bf16a: aligned bf16 DVE + ACT parity-fix copy, boundary side-tile.
    bf16b: pure-DVE bf16, halo-row loads, odd-offset H op (measured only
    12% slower than aligned on HW -- no ACT round trip).
    The fp32 first pass uses cc_first columns (the fp32 load tile is the
    SBUF hog); chained bf16 passes use cc_chain. Tile pools size slots to
    the max across passes and unused pools allocate nothing.
    """
    if cc_chain is None:
        cc_chain = 512 if mode == "bf16b" else 256
    if mode.startswith("cg"):
        assert H % 256 == 0
    else:
        assert H % rpp == 0
        ppi = H // rpp
        assert ppi <= 128 and W % cc_first == 0 and W % cc_chain == 0

    nc = bacc.Bacc("TRN2", target_bir_lowering=False, debug=False, num_devices=1)
    x_d = nc.dram_tensor("x", [C, H, W], F32, kind="ExternalInput").ap()
    o_d = nc.dram_tensor("o", [C, H, W], BF16, kind="ExternalOutput").ap()
    # 3-deep scratch rotation: pass k's stores WAR-conflict only with pass
    # k-3's loads, so coarse DRAM dep tracking can't stall the pipeline.
    ns = min(3, max(0, reps - 1))
    s_d = [
        nc.dram_tensor(f"scratch{i}", [C, H, W], BF16, kind="Internal").ap()
        for i in range(ns)
    ]

    def stage(i):
        src = x_d if i == 0 else s_d[(i - 1) % ns]
        dst = o_d if i == reps - 1 else s_d[i % ns]
        return src, dst

    if mode.startswith("cg"):
        bf = {"t32": 2, "t": 3, "bnd": 3, "p": 2, "v": 2, "vs": 1, "g": 1,
              "ob": 3}
    elif mode == "bf16b":
        bf = {"t32": 2, "t": 2, "bnd": 1, "p": 1, "v": 1, "vs": 1, "g": 1,
              "ob": 3}
    else:
        bf = {"t32": 3, "t": 2, "bnd": 2, "p": 1, "v": 2, "vs": 2, "g": 1,
              "ob": 3}
    if bufs:
        bf.update(bufs)
    emit = {"bf16a": _emit_pass_bf16a, "bf16b": _emit_pass_bf16b}.get(mode)
    with tile.TileContext(nc) as tc:
        with (
            tc.tile_pool(name="t32", bufs=bf["t32"]) as t32p,
            tc.tile_pool(name="t", bufs=bf["t"]) as tp,
            tc.tile_pool(name="bnd", bufs=bf["bnd"]) as bnd,
            tc.tile_pool(name="p", bufs=bf["p"]) as pp,
            tc.tile_pool(name="v", bufs=bf["v"]) as vt,
            tc.tile_pool(name="vs", bufs=bf["vs"]) as vsp,
            tc.tile_pool(name="g", bufs=bf["g"]) as gp,
            tc.tile_pool(name="ob", bufs=bf["ob"]) as obp,
        ):
            pools = (t32p, tp, bnd, pp, vt, vsp, gp, obp)
            for i in range(reps):
                src, dst = stage(i)
                if mode.startswith("cg"):
                    probe = mode[2:].lstrip("_") or "none"
                    _emit_pass_cg(
                        nc, pools, src, dst, C, H, W, in_f32=(i == 0),
                        probe=probe,
                    )
                else:
                    cc = cc_first if i == 0 else cc_chain
                    emit(nc, pools, src, dst, C, H, W, cc, rpp, in_f32=(i == 0))
    nc.compile()
    return nc


def _get_program(C, H, W, reps=1, mode="auto"):
    if mode == "auto":
        mode = "cg" if reps == 1 else "rs"
    key = (C, H, W, reps, mode)
    if key not in _cache:
        if mode == "rs":
            _cache[key] = build_erosion_rs(C, H, W, reps=reps)
        elif mode in ("bf16a", "bf16b") or mode.startswith("cg"):
            _cache[key] = build_erosion_bf16a(C, H, W, mode=mode, reps=reps)
        else:
            _cache[key] = build_erosion(C, H, W, reps=reps, mode=mode)
    return _cache[key]


def kernel(x, m):
    from concourse.bass_utils import run_bass_kernel_spmd

    m = int(np.asarray(m))
    x = np.ascontiguousarray(np.asarray(x), dtype=np.float32)
    B, C, H, W = x.shape
    if m <= 0:
        return x.copy()
    # erosion by a (2m+1)-square = m chained 3x3 erosion passes in one NEFF
    nc = _get_program(C, H, W, reps=m)
    n_cores = 8
    assert B == n_cores, f"expected batch {n_cores}, got {B}"
    in_maps = [{"x": x[b]} for b in range(n_cores)]
    res = run_bass_kernel_spmd(nc, in_maps, core_ids=list(range(n_cores)))
    # device output is bf16; widen to fp32 on the host
    return np.stack(
        [np.asarray(r["o"]).astype(np.float32) for r in res.results], axis=0
    )


if __name__ == "__main__":
    # small-scale CoreSim correctness check (no hardware needed)
    import ml_dtypes

    from concourse.bass_interp import CoreSim

    rng = np.random.default_rng(0)
    C, H, W = 2, 256, 64
    x = rng.standard_normal((C, H, W)).astype(np.float32)

    def bf16r(a):
        return a.astype(ml_dtypes.bfloat16).astype(np.float32)

    for reps, mode in ((1, "cg"), (2, "cg"), (4, "cg")):
        if mode == "cg":
            nc = build_erosion_bf16a(C, H, W, mode=mode, reps=reps)
        elif mode in ("bf16a", "bf16b"):
            nc = build_erosion_bf16a(
                C, H, W, mode=mode, cc_first=16, cc_chain=32, rpp=4, reps=reps
            )
        else:
            nc = build_erosion(C, H, W, cc=32, rpp=4, reps=reps, mode=mode)
        sim = CoreSim(nc)
        sim.tensor("x")[:] = x
        sim.simulate(check_with_hw=False)
        got = np.asarray(sim.tensor("o")).astype(np.float32)

        # bf16a converts the input to bf16 before the first pass
        exp = bf16r(x) if mode in ("bf16", "bf16a") else x
        for _ in range(reps):
            xp = np.pad(exp, ((0, 0), (1, 1), (1, 1)), constant_values=PAD)
            nxt = np.empty_like(exp)
            for i in range(H):
                for j in range(W):
                    nxt[:, i, j] = xp[:, i : i + 3, j : j + 3].min(axis=(1, 2))
            exp = bf16r(nxt)  # device stores bf16 each pass
        ok = np.array_equal(got, exp)
        rel = np.max(np.abs(got - exp) / np.maximum(np.abs(exp), 1e-6))
        print(f"CoreSim reps={reps} mode={mode} exact: {ok} rel={rel:.2e}")



# revision 37
# speedup vs baseline: 7.3621x; 1.0350x over previous
"""Trainium2 Bass kernel for 3x3 (applied m times) morphological erosion
(sliding-window min) over [B, C, H, W] fp32, B=8 sharded across 8
NeuronCores (one batch per core).

v5 scheme, driven by direct HW microbenchmarks (see git-less history in
the session notes; measured on the axon-tunneled trn2 cores):
  - DVE tensor_tensor min bf16 = 0.372 ns/elem-per-partition (0.415 with
    one odd-offset operand) vs fp32 = 1.097 -> ALL mins run in bf16. min
    never creates values, so total error = one bf16 rounding (~4e-3 rel),
    far inside the 2e-2 gate. Device stores bf16; host widens to fp32.
  - DMA cost is descriptor-dominated: [64p x 18r x ~1KB] strided loads
    measured ~45 GB/s, while >=16KB-contiguous-per-partition descriptors
    run at ~350 GB/s each way concurrently. So: one image per
    128-partition stack, partition p holds R=H/128 FULL rows -- every
    image load/store is ONE DMA with 128 contiguous 16-32KB descriptors.
  - m == 1 ("cg" mode): streaming passes; fp32 staged in half-tiles,
    ACT converts to bf16, V pass = even/odd-shared decomposition
    (1.5 ops/elem, aligned), boundary rows via 2 partition-shifted
    SBUF->SBUF row copies + 2 tiny edge duplicates, H pass = h/ob with
    one odd-offset op; per-column-edge 1-column ops instead of PAD.
  - m > 1 ("rs" mode): the whole shard (C images x 16KB/partition, bf16)
    stays RESIDENT in SBUF: load once (gpsimd casting DMA fp32->bf16),
    run m passes on-chip (pure DVE + 4 small boundary DMAs per image per
    pass, C+1-slot rotating image pool), store once. Per-pass marginal
    cost ~93 us vs ~300 us for the DRAM ping-pong baseline.
"""

import sys

sys.path.insert(0, "/opt/trn_rl_repo")

import numpy as np

import concourse.bass as bass
import concourse.tile as tile
from concourse import bacc, mybir

PAD = 1.0e9
F32 = mybir.dt.float32
BF16 = mybir.dt.bfloat16
MIN = mybir.AluOpType.min

CC = 256  # column chunk width
RPP = 16  # image rows per partition

_cache = {}


def _emit_pass_f32eo2(nc, pools, x_d, o_d, C, H, W, cc, rpp, in_f32):
    """One erosion pass, fp32 compute, even/odd V pass over halo-loaded rows.

    Each partition loads R+2 rows (its R rows plus one halo row each side,
    overlapping DRAM reads; +2/R load bytes) so the V pass is exactly 3
    row-stepped DVE ops with no boundary fixups:
      p[j] = min(e[2j], e[2j+1])          j = 0..R/2
      v[2j] = min(p[j], e[2j+2])          j = 0..R/2-1
      v[2j+1] = min(e[2j+1], p[j+1])      j = 0..R/2-1
    Image-edge partitions: top halo row = PAD memset (start partitions
    0/64: legal), bottom halo row = own-last-row duplication DMA
    (min-idempotent). All slices keep unit innermost stride (full DVE
    rate). The final H op writes bf16 (free output cast); host widens.
    """
    from concourse.ap import AP as _AP

    ppi = H // rpp
    ips = max(1, 128 // ppi)
    inp, xbp, bnd, vtm, vt, htm, obp = pools
    R = rpp
    E = R + 2  # rows incl halo
    PW = cc + 2
    assert R % 2 == 0

    tiles = [(s0, c0) for s0 in range(0, C, ips) for c0 in range(0, W, cc)]
    front = {}

    def emit_front(i):
        s0, c0 = tiles[i]
        n_img = min(ips, C - s0)
        P = n_img * ppi
        last = c0 + cc == W
        t = inp.tile([128, E, PW], F32, name="t")
        if in_f32:
            ld = t
        else:
            ld = xbp.tile([128, E, PW], BF16, name="xb")
        wlo = max(c0 - 1, 0)
        whi = min(c0 + cc + 1, W)
        nw = whi - wlo
        dlo = 1 if c0 == 0 else 0
        for im in range(n_img):
            img = x_d[s0 + im]  # [H, W]
            p0 = im * ppi
            # interior partitions 1..ppi-2: rows rpp*p-1 .. rpp*p+rpp
            src = _AP(
                tensor=img.tensor,
                offset=img.offset + (rpp - 1) * W + wlo,
                ap=[[rpp * W, ppi - 2], [W, E], [1, nw]],
            )
            nc.sync.dma_start(
                ld[p0 + 1 : p0 + ppi - 1, :, dlo : dlo + nw], src
            )
            # top partition: rows 0..rpp into slots 1..E-1; slot 0 = PAD
            nc.sync.dma_start(
                ld[p0 : p0 + 1, 1:E, dlo : dlo + nw],
                img[0 : rpp + 1, wlo:whi].rearrange("(p r) w -> p r w", p=1),
            )
            nc.gpsimd.memset(ld[p0 : p0 + 1, 0:1, :], PAD)
            # bottom partition: rows H-rpp-1..H-1 into slots 0..E-2;
            # slot E-1 = duplicate of the image's last row (min-idempotent)
            pe = p0 + ppi - 1
            nc.sync.dma_start(
                ld[pe : pe + 1, 0 : E - 1, dlo : dlo + nw],
                img[H - rpp - 1 : H, wlo:whi].rearrange("(p r) w -> p r w", p=1),
            )
            nc.sync.dma_start(
                ld[pe : pe + 1, E - 1 : E, dlo : dlo + nw],
                img[H - 1 : H, wlo:whi].rearrange("(p r) w -> p r w", p=1),
            )
        if c0 == 0:
            nc.gpsimd.memset(ld[0:P, :, 0:1], PAD)
        if last:
            nc.gpsimd.memset(ld[0:P, :, PW - 1 : PW], PAD)
        if not in_f32:
            nc.scalar.copy(t[0:P], ld[0:P])
        front[i] = (t, n_img, P, s0, c0)

    def emit_back(i):
        t, n_img, P, s0, c0 = front.pop(i)
        Rh = R // 2
        v = vt.tile([128, R, PW], F32, name="v")
        p = vtm.tile([128, Rh + 1, PW], F32, name="p")
        nc.vector.tensor_tensor(
            out=p[0:P], in0=t[0:P, 0:E:2, :], in1=t[0:P, 1:E:2, :], op=MIN
        )
        nc.vector.tensor_tensor(
            out=v[0:P, 0:R:2, :],
            in0=p[0:P, 0:Rh, :],
            in1=t[0:P, 2:E:2, :],
            op=MIN,
        )
        nc.vector.tensor_tensor(
            out=v[0:P, 1:R:2, :],
            in0=t[0:P, 1 : E - 2 : 2, :],
            in1=p[0:P, 1 : Rh + 1, :],
            op=MIN,
        )

        h = htm.tile([128, R, cc + 1], F32, name="h")
        nc.vector.tensor_tensor(
            out=h[0:P], in0=v[0:P, :, 0 : cc + 1], in1=v[0:P, :, 1 : cc + 2],
            op=MIN,
        )
        ob = obp.tile([128, R, cc], BF16, name="ob")
        nc.vector.tensor_tensor(
            out=ob[0:P], in0=h[0:P, :, 0:cc], in1=v[0:P, :, 2 : cc + 2], op=MIN
        )

        for im in range(n_img):
            dst = o_d[s0 + im, :, c0 : c0 + cc].rearrange(
                "(p r) w -> p r w", p=ppi
            )
            p0 = im * ppi
            nc.scalar.dma_start(dst, ob[p0 : p0 + ppi, :, :])

    skew = 2
    for i in range(len(tiles) + skew):
        if i < len(tiles):
            emit_front(i)
        if i >= skew:
            emit_back(i - skew)


def _emit_pass_f32eo(nc, pools, x_d, o_d, C, H, W, cc, rpp, in_f32):
    """One erosion pass, fp32 compute with even/odd-shared V pass.

    x_d fp32 (first pass) or bf16 (chained); o_d bf16. All mins on DVE in
    fp32 (bf16 ALU is slower on real HW); V pass uses the pairwise-sharing
    decomposition (1.5 ops/elem): p[j] = min(r[2j], r[2j+1]);
    v[2j] = min(r[2j-1], p[j]); v[2j+1] = min(p[j], r[2j+2]). Row-stepped
    slices keep unit innermost stride (full DVE rate). The final H op
    writes bf16 (free output cast).
    """
    ppi = H // rpp
    ips = max(1, 128 // ppi)
    inp, xbp, bnd, vtm, vt, htm, obp = pools
    R = rpp
    PW = cc + 2
    assert R % 2 == 0

    tiles = [(s0, c0) for s0 in range(0, C, ips) for c0 in range(0, W, cc)]
    front = {}

    def emit_front(i):
        s0, c0 = tiles[i]
        n_img = min(ips, C - s0)
        P = n_img * ppi
        last = c0 + cc == W
        # load tile: fp32 on first pass, bf16 on chained passes; compute
        # stays fp32 either way (bf16 load is upconverted by ACT)
        t = inp.tile([128, R, PW], F32, name="t")
        if in_f32:
            ld = t
        else:
            ld = xbp.tile([128, R, PW], BF16, name="xb")
        wlo = max(c0 - 1, 0)
        whi = min(c0 + cc + 1, W)
        dlo = 1 if c0 == 0 else 0
        for im in range(n_img):
            src = x_d[s0 + im, :, wlo:whi].rearrange("(p r) w -> p r w", p=ppi)
            p0 = im * ppi
            nc.sync.dma_start(ld[p0 : p0 + ppi, :, dlo : dlo + (whi - wlo)], src)
        if c0 == 0:
            nc.gpsimd.memset(ld[0:P, :, 0:1], PAD)
        if last:
            nc.gpsimd.memset(ld[0:P, :, PW - 1 : PW], PAD)
        if not in_f32:
            nc.scalar.copy(t[0:P], ld[0:P])
        front[i] = (t, n_img, P, s0, c0)

    def emit_back(i):
        t, n_img, P, s0, c0 = front.pop(i)
        bt = bnd.tile([128, 2, PW], F32, name="bt")
        for im in range(n_img):
            p0 = im * ppi
            pe = p0 + ppi - 1
            nc.sync.dma_start(bt[p0:pe, 0:1, :], t[p0 + 1 : pe + 1, 0:1, :])
            nc.sync.dma_start(
                bt[p0 + 1 : pe + 1, 1:2, :], t[p0:pe, R - 1 : R, :]
            )
            nc.sync.dma_start(
                bt[pe : pe + 1, 0:1, :], t[pe : pe + 1, R - 1 : R, :]
            )
            nc.gpsimd.memset(bt[p0 : p0 + 1, 1:2, :], PAD)

        # ---- V pass, even/odd shared (fp32) ----
        v = vt.tile([128, R, PW], F32, name="v")
        p = vtm.tile([128, R // 2, PW], F32, name="p")
        Rh = R // 2
        nc.vector.tensor_tensor(
            out=p[0:P], in0=t[0:P, 0:R:2, :], in1=t[0:P, 1:R:2, :], op=MIN
        )
        # v[2j] = min(r[2j-1], p[j]), j=1..Rh-1 (j=0 uses bt row-above)
        nc.vector.tensor_tensor(
            out=v[0:P, 2:R:2, :],
            in0=t[0:P, 1 : R - 2 : 2, :],
            in1=p[0:P, 1:Rh, :],
            op=MIN,
        )
        nc.vector.tensor_tensor(
            out=v[0:P, 0:1, :], in0=p[0:P, 0:1, :], in1=bt[0:P, 1:2, :], op=MIN
        )
        # v[2j+1] = min(p[j], r[2j+2]), j=0..Rh-2 (j=Rh-1 uses bt row-below)
        nc.vector.tensor_tensor(
            out=v[0:P, 1 : R - 1 : 2, :],
            in0=p[0:P, 0 : Rh - 1, :],
            in1=t[0:P, 2:R:2, :],
            op=MIN,
        )
        nc.vector.tensor_tensor(
            out=v[0:P, R - 1 : R, :],
            in0=p[0:P, Rh - 1 : Rh, :],
            in1=bt[0:P, 0:1, :],
            op=MIN,
        )

        # ---- H pass (fp32, last op casts to bf16 on output) ----
        h = htm.tile([128, R, cc + 1], F32, name="h")
        nc.vector.tensor_tensor(
            out=h[0:P], in0=v[0:P, :, 0 : cc + 1], in1=v[0:P, :, 1 : cc + 2],
            op=MIN,
        )
        ob = obp.tile([128, R, cc], BF16, name="ob")
        nc.vector.tensor_tensor(
            out=ob[0:P], in0=h[0:P, :, 0:cc], in1=v[0:P, :, 2 : cc + 2], op=MIN
        )

        for im in range(n_img):
            dst = o_d[s0 + im, :, c0 : c0 + cc].rearrange(
                "(p r) w -> p r w", p=ppi
            )
            p0 = im * ppi
            nc.scalar.dma_start(dst, ob[p0 : p0 + ppi, :, :])

    skew = 2
    for i in range(len(tiles) + skew):
        if i < len(tiles):
            emit_front(i)
        if i >= skew:
            emit_back(i - skew)


def _emit_pass_cg(nc, pools, x_d, o_d, C, H, W, in_f32, probe="none"):
    """One erosion pass, contiguous-big-DMA layout + bf16 DVE compute.

    HW measurements drove this shape:
      - DVE TT min bf16 = 0.372 ns/elem-pp (0.415 odd-offset) vs fp32 1.097
        -> all mins in bf16.
      - DMA throughput is descriptor-dominated: the old [ppi,R,~1KB-line]
        strided loads measured ~45 GB/s; >=16KB contiguous-per-partition
        descriptors reach ~350-400 GB/s.
    So: one image per 128-partition stack, partition p holds R=H/128 FULL
    rows (contiguous 8KB-32KB span in DRAM and in SBUF) -> ONE descriptor
    per partition per image load/store. Per image per pass: 1 big load
    (sync queue), 2 partition-shifted SBUF->SBUF row copies for the V-pass
    boundary rows (scalar queue), 1 big store (scalar queue). Image-edge
    partitions get their boundary row via tiny 1-partition DVE ops reading
    the tile's own edge row (min-idempotent duplicate) -- no memsets, no
    PAD columns: the H-pass edge columns use dedicated 1-column ops.
    """
    R = H // 128
    Rh = R // 2
    assert H % 128 == 0 and R % 2 == 0
    t32p, tp, bnd, pp, vt, _vsp, hp, obp = pools
    front = {}

    def emit_front(i):
        from concourse.ap import AP as _AP

        t = tp.tile([128, R, W], BF16, name="t")
        if in_f32:
            # fp32 staging in two half-tiles: halves the t32 SBUF slot so
            # the bf16 pipeline pools can run deeper
            Rhalf = R // 2
            img = x_d[i]
            for c in range(2):
                ld = t32p.tile([128, Rhalf, W], F32, name="t32")
                src_c = _AP(
                    tensor=img.tensor,
                    offset=img.offset + c * Rhalf * W,
                    ap=[[R * W, 128], [1, Rhalf * W]],
                )
                nc.sync.dma_start(ld.rearrange("p r w -> p (r w)"), src_c)
                nc.scalar.copy(t[:, c * Rhalf : (c + 1) * Rhalf, :], ld)
        else:
            src = x_d[i].rearrange("(p r) w -> p (r w)", p=128)
            nc.sync.dma_start(t.rearrange("p r w -> p (r w)"), src)
        # bt[p,0] = row above (prev partition's last row),
        # bt[p,1] = row below (next partition's first row)
        if probe == "dma":
            front[i] = (t, None)
            return
        bt = bnd.tile([128, 2, W], BF16, name="bt")
        nc.scalar.dma_start(bt[1:128, 0:1, :], t[0:127, R - 1 : R, :])
        nc.scalar.dma_start(bt[0:127, 1:2, :], t[1:128, 0:1, :])
        # image-edge partitions duplicate their own edge row (the DVE
        # rejects partition starts other than 0/32/64/96, so these are DMAs)
        nc.scalar.dma_start(bt[0:1, 0:1, :], t[0:1, 0:1, :])
        nc.scalar.dma_start(bt[127:128, 1:2, :], t[127:128, R - 1 : R, :])
        front[i] = (t, bt)

    def emit_back(i):
        t, bt = front.pop(i)
        if probe == "dma":  # timing probe: load->store only
            dst = o_d[i].rearrange("(p r) w -> p (r w)", p=128)
            nc.scalar.dma_start(dst, t.rearrange("p r w -> p (r w)"))
            return
        # ---- V pass (bf16; all aligned) ----
        p = pp.tile([128, Rh, W], BF16, name="p")
        v = vt.tile([128, R, W], BF16, name="v")
        nc.vector.tensor_tensor(
            out=p, in0=t[:, 0:R:2, :], in1=t[:, 1:R:2, :], op=MIN
        )
        if Rh > 1:
            # v[2j] = min(r[2j-1], p[j]), j=1..Rh-1
            nc.vector.tensor_tensor(
                out=v[:, 2:R:2, :],
                in0=t[:, 1 : R - 2 : 2, :],
                in1=p[:, 1:Rh, :],
                op=MIN,
            )
            # v[2j+1] = min(p[j], r[2j+2]), j=0..Rh-2
            nc.vector.tensor_tensor(
                out=v[:, 1 : R - 1 : 2, :],
                in0=p[:, 0 : Rh - 1, :],
                in1=t[:, 2:R:2, :],
                op=MIN,
            )
        # v[0] = min(p[0], row-above); v[R-1] = min(p[Rh-1], row-below)
        if Rh >= 2:
            nc.vector.tensor_tensor(
                out=v[:, 0 : R : R - 1, :],
                in0=p[:, 0 : Rh : Rh - 1, :],
                in1=bt[:, 0:2, :],
                op=MIN,
            )
        else:
            nc.vector.tensor_tensor(
                out=v[:, 0:1, :], in0=p[:, 0:1, :], in1=bt[:, 0:1, :], op=MIN
            )
            nc.vector.tensor_tensor(
                out=v[:, R - 1 : R, :],
                in0=p[:, Rh - 1 : Rh, :],
                in1=bt[:, 1:2, :],
                op=MIN,
            )
        if probe == "noh":  # timing probe: V pass only, store v
            dst = o_d[i].rearrange("(p r) w -> p (r w)", p=128)
            nc.scalar.dma_start(dst, v.rearrange("p r w -> p (r w)"))
            return
        # ---- H pass: h[c] = min(v[c], v[c+1]); ob interior + edge cols ----
        h = hp.tile([128, R, W], BF16, name="h")
        nc.vector.tensor_tensor(
            out=h[:, :, 0 : W - 1],
            in0=v[:, :, 0 : W - 1],
            in1=v[:, :, 1:W],
            op=MIN,
        )
        ob = obp.tile([128, R, W], BF16, name="ob")
        nc.vector.tensor_tensor(
            out=ob[:, :, 1 : W - 1],
            in0=h[:, :, 0 : W - 2],
            in1=v[:, :, 2:W],
            op=MIN,
        )
        nc.vector.tensor_tensor(
            out=ob[:, :, 0:1], in0=h[:, :, 0:1], in1=v[:, :, 1:2], op=MIN
        )
        nc.vector.tensor_copy(
            out=ob[:, :, W - 1 : W], in_=h[:, :, W - 2 : W - 1]
        )
        dst = o_d[i].rearrange("(p r) w -> p (r w)", p=128)
        nc.scalar.dma_start(dst, ob.rearrange("p r w -> p (r w)"))

    skew = 2
    for i in range(C + skew):
        if i < C:
            emit_front(i)
        if i >= skew:
            emit_back(i - skew)


def build_erosion_rs(C, H, W, reps):
    """SBUF-resident multi-pass erosion: the whole per-core shard (C images
    x H/128 rows x W cols, bf16 = 16 KiB/partition/image) stays in SBUF
    across all passes.

    Load once (gpsimd casting DMA, fp32->bf16 inline), run `reps` erosion
    passes entirely on-chip (DVE mins + 4 small SBUF->SBUF boundary-row
    DMAs per image per pass), store once. Per-pass marginal cost is pure
    DVE (~90 us) -- no HBM traffic per pass. The image pool has C+1
    slots; each pass writes image i's result into the rotating spare slot
    and the old buffer becomes the next spare (Tile's WAR deps make the
    rotation safe).
    """
    R = H // 128
    Rh = R // 2
    assert H % 256 == 0 and R % 2 == 0 and reps >= 1

    nc = bacc.Bacc("TRN2", target_bir_lowering=False, debug=False, num_devices=1)
    x_d = nc.dram_tensor("x", [C, H, W], F32, kind="ExternalInput").ap()
    o_d = nc.dram_tensor("o", [C, H, W], BF16, kind="ExternalOutput").ap()

    with tile.TileContext(nc) as tc:
        with (
            tc.tile_pool(name="img", bufs=C + 1) as imgp,
            tc.tile_pool(name="bnd", bufs=3) as bnd,
            tc.tile_pool(name="p", bufs=1) as pp,
            tc.tile_pool(name="v", bufs=1) as vt,
            tc.tile_pool(name="h", bufs=1) as hp,
        ):
            img_t = []
            for i in range(C):
                t = imgp.tile([128, R, W], BF16, name="img")
                nc.gpsimd.dma_start(
                    t.rearrange("p r w -> p (r w)"),
                    x_d[i].rearrange("(p r) w -> p (r w)", p=128),
                )
                img_t.append(t)

            bts = {}

            def emit_bt(i):
                t = img_t[i]
                bt = bnd.tile([128, 2, W], BF16, name="bt")
                nc.scalar.dma_start(bt[1:128, 0:1, :], t[0:127, R - 1 : R, :])
                nc.scalar.dma_start(bt[0:127, 1:2, :], t[1:128, 0:1, :])
                nc.scalar.dma_start(bt[0:1, 0:1, :], t[0:1, 0:1, :])
                nc.scalar.dma_start(
                    bt[127:128, 1:2, :], t[127:128, R - 1 : R, :]
                )
                bts[i] = bt

            def emit_compute(i):
                t = img_t[i]
                bt = bts.pop(i)
                p = pp.tile([128, Rh, W], BF16, name="p")
                v = vt.tile([128, R, W], BF16, name="v")
                nc.vector.tensor_tensor(
                    out=p, in0=t[:, 0:R:2, :], in1=t[:, 1:R:2, :], op=MIN
                )
                if Rh > 1:
                    nc.vector.tensor_tensor(
                        out=v[:, 2:R:2, :],
                        in0=t[:, 1 : R - 2 : 2, :],
                        in1=p[:, 1:Rh, :],
                        op=MIN,
                    )
                    nc.vector.tensor_tensor(
                        out=v[:, 1 : R - 1 : 2, :],
                        in0=p[:, 0 : Rh - 1, :],
                        in1=t[:, 2:R:2, :],
                        op=MIN,
                    )
                if Rh >= 2:
                    # fused boundary rows: out rows {0, R-1}, p rows
                    # {0, Rh-1}, bt rows {0, 1} -- one strided op
                    nc.vector.tensor_tensor(
                        out=v[:, 0 : R : R - 1, :],
                        in0=p[:, 0 : Rh : Rh - 1, :],
                        in1=bt[:, 0:2, :],
                        op=MIN,
                    )
                else:
                    nc.vector.tensor_tensor(
                        out=v[:, 0:1, :], in0=p[:, 0:1, :],
                        in1=bt[:, 0:1, :], op=MIN,
                    )
                    nc.vector.tensor_tensor(
                        out=v[:, R - 1 : R, :],
                        in0=p[:, Rh - 1 : Rh, :],
                        in1=bt[:, 1:2, :],
                        op=MIN,
                    )
                h = hp.tile([128, R, W], BF16, name="h")
                nc.vector.tensor_tensor(
                    out=h[:, :, 0 : W - 1],
                    in0=v[:, :, 0 : W - 1],
                    in1=v[:, :, 1:W],
                    op=MIN,
                )
                nxt = imgp.tile([128, R, W], BF16, name="img")
                nc.vector.tensor_tensor(
                    out=nxt[:, :, 1 : W - 1],
                    in0=h[:, :, 0 : W - 2],
                    in1=v[:, :, 2:W],
                    op=MIN,
                )
                nc.vector.tensor_tensor(
                    out=nxt[:, :, 0:1], in0=h[:, :, 0:1], in1=v[:, :, 1:2],
                    op=MIN,
                )
                nc.vector.tensor_copy(
                    out=nxt[:, :, W - 1 : W], in_=h[:, :, W - 2 : W - 1]
                )
                img_t[i] = nxt

            for k in range(reps):
                for i in range(C + 1):
                    if i < C:
                        emit_bt(i)
                    if i >= 1:
                        emit_compute(i - 1)

            for i in range(C):
                nc.sync.dma_start(
                    o_d[i].rearrange("(p r) w -> p (r w)", p=128),
                    img_t[i].rearrange("p r w -> p (r w)"),
                )
    nc.compile()
    return nc


def _emit_pass_bf16b(nc, pools, x_d, o_d, C, H, W, cc, rpp, in_f32):
    """One erosion pass, pure-DVE bf16 compute, halo-row loads (f32eo2
    style), no ACT compute and no SBUF->SBUF boundary DMAs.

    Measured on HW (microbench): TT min bf16 = 0.372 ns/elem-per-partition
    when 4B-aligned and 0.415 when one operand starts at an odd column --
    both ~2.9x faster than fp32 (1.097). So compute everything in bf16 on
    the DVE: V pass = even/odd shared decomposition over the halo-loaded
    rows (all aligned), H pass = h(odd operand) then ob (aligned). The
    only ACT work is the fp32->bf16 convert on the first pass; stores
    ride the ACT queue as plain triggers.
    """
    from concourse.ap import AP as _AP

    ppi = H // rpp
    ips = max(1, 128 // ppi)
    t32p, tp, _bnd, pp, vt, _vsp, hp, obp = pools
    R = rpp
    Rh = R // 2
    E = R + 2
    PW = cc + 2
    assert R % 2 == 0 and cc % 2 == 0

    tiles = [(s0, c0) for s0 in range(0, C, ips) for c0 in range(0, W, cc)]
    front = {}

    def emit_front(i):
        s0, c0 = tiles[i]
        n_img = min(ips, C - s0)
        P = n_img * ppi
        last = c0 + cc == W
        t = tp.tile([128, E, PW], BF16, name="t")
        ld = t32p.tile([128, E, PW], F32, name="t32") if in_f32 else t
        wlo = max(c0 - 1, 0)
        whi = min(c0 + cc + 1, W)
        nw = whi - wlo
        dlo = 1 if c0 == 0 else 0
        for im in range(n_img):
            img = x_d[s0 + im]  # [H, W]
            p0 = im * ppi
            # interior partitions 1..ppi-2: rows rpp*p-1 .. rpp*p+rpp
            src = _AP(
                tensor=img.tensor,
                offset=img.offset + (rpp - 1) * W + wlo,
                ap=[[rpp * W, ppi - 2], [W, E], [1, nw]],
            )
            nc.sync.dma_start(ld[p0 + 1 : p0 + ppi - 1, :, dlo : dlo + nw], src)
            # top partition: rows 0..rpp into slots 1..E-1; slot 0 = row 0
            # again (min-idempotent duplicate stands in for the PAD row)
            nc.sync.dma_start(
                ld[p0 : p0 + 1, 1:E, dlo : dlo + nw],
                img[0 : rpp + 1, wlo:whi].rearrange("(p r) w -> p r w", p=1),
            )
            nc.sync.dma_start(
                ld[p0 : p0 + 1, 0:1, dlo : dlo + nw],
                img[0:1, wlo:whi].rearrange("(p r) w -> p r w", p=1),
            )
            # bottom partition: rows H-rpp-1..H-1 into slots 0..E-2;
            # slot E-1 = duplicate of the image's last row
            pe = p0 + ppi - 1
            nc.sync.dma_start(
                ld[pe : pe + 1, 0 : E - 1, dlo : dlo + nw],
                img[H - rpp - 1 : H, wlo:whi].rearrange("(p r) w -> p r w", p=1),
            )
            nc.sync.dma_start(
                ld[pe : pe + 1, E - 1 : E, dlo : dlo + nw],
                img[H - 1 : H, wlo:whi].rearrange("(p r) w -> p r w", p=1),
            )
        if c0 == 0:
            nc.gpsimd.memset(ld[0:P, :, 0:1], PAD)
        if last:
            nc.gpsimd.memset(ld[0:P, :, PW - 1 : PW], PAD)
        if in_f32:
            nc.scalar.copy(t[0:P], ld[0:P])
        front[i] = (t, n_img, P, s0, c0)

    def emit_back(i):
        t, n_img, P, s0, c0 = front.pop(i)
        # ---- V pass over halo rows (bf16, aligned -> fast) ----
        p = pp.tile([128, Rh + 1, PW], BF16, name="p")
        v = vt.tile([128, R, PW], BF16, name="v")
        nc.vector.tensor_tensor(
            out=p[0:P], in0=t[0:P, 0:E:2, :], in1=t[0:P, 1:E:2, :], op=MIN
        )
        nc.vector.tensor_tensor(
            out=v[0:P, 0:R:2, :],
            in0=p[0:P, 0:Rh, :],
            in1=t[0:P, 2:E:2, :],
            op=MIN,
        )
        nc.vector.tensor_tensor(
            out=v[0:P, 1:R:2, :],
            in0=t[0:P, 1 : E - 2 : 2, :],
            in1=p[0:P, 1 : Rh + 1, :],
            op=MIN,
        )
        # ---- H pass: h has one odd-offset operand (12% slower, fine) ----
        h = hp.tile([128, R, PW], BF16, name="h")
        nc.vector.tensor_tensor(
            out=h[0:P, :, 0 : cc + 1],
            in0=v[0:P, :, 0 : cc + 1],
            in1=v[0:P, :, 1 : cc + 2],
            op=MIN,
        )
        ob = obp.tile([128, R, cc], BF16, name="ob")
        nc.vector.tensor_tensor(
            out=ob[0:P], in0=h[0:P, :, 0:cc], in1=v[0:P, :, 2 : cc + 2], op=MIN
        )
        for im in range(n_img):
            dst = o_d[s0 + im, :, c0 : c0 + cc].rearrange(
                "(p r) w -> p r w", p=ppi
            )
            p0 = im * ppi
            nc.scalar.dma_start(dst, ob[p0 : p0 + ppi, :, :])

    skew = 2
    for i in range(len(tiles) + skew):
        if i < len(tiles):
            emit_front(i)
        if i >= skew:
            emit_back(i - skew)


def _emit_pass_bf16a(nc, pools, x_d, o_d, C, H, W, cc, rpp, in_f32):
    """One erosion pass, all-bf16 DVE compute with every tensor_tensor
    operand 4B-aligned so the DVE runs at the 2x_1p rate throughout.

    The 2x_1p perf mode needs 16-bit dtype, unit innermost stride AND a
    4-byte-aligned start address for every operand. The H pass combines
    columns of both parities, so one op would always have a 2-byte-offset
    (odd) operand and silently drop to 1x. Fix: the otherwise-idle ACT
    engine makes a one-column-shifted copy vs[c] = v[c+1] (ACT is 1x at
    1.2 GHz regardless of alignment), and the DVE computes
        g[c]  = min(v[c], v[c+2])   (both even offsets, 2x)
        ob[c] = min(g[c], vs[c])    (both aligned, 2x)
    V pass is the even/odd-shared decomposition (1.5 ops/elem, all row
    slices start at even column 0 -> aligned, 2x). Per-partition boundary
    rows come from a small side tile bt filled by partition-shifted
    SBUF->SBUF DMAs; image edges use own-row duplication (min-idempotent)
    so no memsets on partition starts other than 0 are needed.

    DVE ~1.75 cyc/elem (vs 3.5 at 1x fp32), ACT 1 copy/elem (+1 convert
    on the first pass), DMA loads R rows/partition (no halo rows).
    """
    ppi = H // rpp
    ips = max(1, 128 // ppi)
    t32p, tp, bnd, pp, vt, vsp, gp, obp = pools
    R = rpp
    Rh = R // 2
    PW = cc + 2
    assert R % 2 == 0 and cc % 2 == 0 and PW % 2 == 0

    tiles = [(s0, c0) for s0 in range(0, C, ips) for c0 in range(0, W, cc)]
    front = {}
    mid = {}

    def emit_front(i):
        s0, c0 = tiles[i]
        n_img = min(ips, C - s0)
        P = n_img * ppi
        last = c0 + cc == W
        t = tp.tile([128, R, PW], BF16, name="t")
        ld = t32p.tile([128, R, PW], F32, name="t32") if in_f32 else t
        wlo = max(c0 - 1, 0)
        whi = min(c0 + cc + 1, W)
        dlo = 1 if c0 == 0 else 0
        for im in range(n_img):
            src = x_d[s0 + im, :, wlo:whi].rearrange("(p r) w -> p r w", p=ppi)
            p0 = im * ppi
            nc.sync.dma_start(ld[p0 : p0 + ppi, :, dlo : dlo + (whi - wlo)], src)
        if c0 == 0:
            nc.gpsimd.memset(ld[0:P, :, 0:1], PAD)
        if last:
            nc.gpsimd.memset(ld[0:P, :, PW - 1 : PW], PAD)
        if in_f32:
            nc.scalar.copy(t[0:P], ld[0:P])
        front[i] = (t, n_img, P, s0, c0)

    def emit_mid(i):
        t, n_img, P, s0, c0 = front.pop(i)
        # bt[p,0] = row below the block (next partition's row 0),
        # bt[p,1] = row above (prev partition's row R-1); image edges
        # duplicate the block's own edge row (min-idempotent).
        bt = bnd.tile([128, 2, PW], BF16, name="bt")
        for im in range(n_img):
            p0 = im * ppi
            pe = p0 + ppi - 1
            nc.sync.dma_start(bt[p0:pe, 0:1, :], t[p0 + 1 : pe + 1, 0:1, :])
            nc.sync.dma_start(bt[pe : pe + 1, 0:1, :], t[pe : pe + 1, R - 1 : R, :])
            nc.sync.dma_start(bt[p0 + 1 : pe + 1, 1:2, :], t[p0:pe, R - 1 : R, :])
            nc.sync.dma_start(bt[p0 : p0 + 1, 1:2, :], t[p0 : p0 + 1, 0:1, :])

        # ---- V pass (bf16, all operands 4B-aligned -> 2x) ----
        p = pp.tile([128, Rh, PW], BF16, name="p")
        v = vt.tile([128, R, PW], BF16, name="v")
        nc.vector.tensor_tensor(
            out=p[0:P], in0=t[0:P, 0:R:2, :], in1=t[0:P, 1:R:2, :], op=MIN
        )
        if Rh > 1:
            nc.vector.tensor_tensor(
                out=v[0:P, 2:R:2, :],
                in0=t[0:P, 1 : R - 2 : 2, :],
                in1=p[0:P, 1:Rh, :],
                op=MIN,
            )
            nc.vector.tensor_tensor(
                out=v[0:P, 1 : R - 1 : 2, :],
                in0=p[0:P, 0 : Rh - 1, :],
                in1=t[0:P, 2:R:2, :],
                op=MIN,
            )
        nc.vector.tensor_tensor(
            out=v[0:P, 0:1, :], in0=p[0:P, 0:1, :], in1=bt[0:P, 1:2, :], op=MIN
        )
        nc.vector.tensor_tensor(
            out=v[0:P, R - 1 : R, :],
            in0=p[0:P, Rh - 1 : Rh, :],
            in1=bt[0:P, 0:1, :],
            op=MIN,
        )
        # ACT: parity-fixing shifted copy (1x, alignment-agnostic)
        vs = vsp.tile([128, R, cc], BF16, name="vs")
        nc.scalar.copy(vs[0:P], v[0:P, :, 1 : cc + 1])
        mid[i] = (v, vs, n_img, P, s0, c0)

    def emit_tail(i):
        v, vs, n_img, P, s0, c0 = mid.pop(i)
        g = gp.tile([128, R, cc], BF16, name="g")
        nc.vector.tensor_tensor(
            out=g[0:P], in0=v[0:P, :, 0:cc], in1=v[0:P, :, 2 : cc + 2], op=MIN
        )
        ob = obp.tile([128, R, cc], BF16, name="ob")
        nc.vector.tensor_tensor(out=ob[0:P], in0=g[0:P], in1=vs[0:P], op=MIN)
        for im in range(n_img):
            dst = o_d[s0 + im, :, c0 : c0 + cc].rearrange(
                "(p r) w -> p r w", p=ppi
            )
            p0 = im * ppi
            nc.scalar.dma_start(dst, ob[p0 : p0 + ppi, :, :])

    # 3-stage software pipeline: tail(i) is emitted after mid(i+1), so the
    # DVE's ob(i) lands behind V(i+1) in its in-order stream -- by then the
    # ACT's vs(i) is long done and the DVE never stalls on the ACT.
    n = len(tiles)
    for i in range(n + 2):
        if i < n:
            emit_front(i)
        if 1 <= i <= n:
            emit_mid(i - 1)
        if i >= 2:
            emit_tail(i - 2)


def _emit_pass(nc, pools, x_d, o_d, C, H, W, cc, rpp, in_f32):
    """Emit one full erosion pass x_d -> o_d into the open TileContext.

    in_f32: x_d is fp32 and must be converted to bf16 on ACT; otherwise
    x_d is bf16 and is used directly. o_d is always bf16.
    """
    ppi = H // rpp  # partitions per image
    ips = max(1, 128 // ppi)  # images per partition-stack
    inp, xbp, bnd, vtm, vt, htm, obp = pools
    R = rpp
    PW = cc + 2  # padded tile width

    tiles = [(s0, c0) for s0 in range(0, C, ips) for c0 in range(0, W, cc)]
    front = {}

    def emit_front(i):
        s0, c0 = tiles[i]
        n_img = min(ips, C - s0)
        P = n_img * ppi
        last = c0 + cc == W
        t = inp.tile([128, R, PW], F32, name="t") if in_f32 else None
        xb = xbp.tile([128, R, PW], BF16, name="xb")
        ld = t if in_f32 else xb
        wlo = max(c0 - 1, 0)
        whi = min(c0 + cc + 1, W)
        dlo = 1 if c0 == 0 else 0
        for im in range(n_img):
            src = x_d[s0 + im, :, wlo:whi].rearrange("(p r) w -> p r w", p=ppi)
            p0 = im * ppi
            nc.sync.dma_start(ld[p0 : p0 + ppi, :, dlo : dlo + (whi - wlo)], src)
        if c0 == 0:
            nc.gpsimd.memset(ld[0:P, :, 0:1], PAD)
        if last:
            nc.gpsimd.memset(ld[0:P, :, PW - 1 : PW], PAD)
        if in_f32:
            nc.scalar.copy(xb[0:P], t[0:P])
        front[i] = (xb, n_img, P, s0, c0)

    def emit_back(i):
        xb, n_img, P, s0, c0 = front.pop(i)
        # boundary-row side tile (bf16): bt[p,0] = first row of the block
        # below (xb[p+1] row 0), bt[p,1] = last row of the block above
        # (xb[p-1] row R-1); image-edge partitions: top -> PAD memset
        # (start partition p0 is 0 mod ppi>=32: legal), bottom -> own-row
        # duplication DMA (min-idempotent; DMA has no start-partition rule).
        bt = bnd.tile([128, 2, PW], BF16)
        for im in range(n_img):
            p0 = im * ppi
            pe = p0 + ppi - 1  # last partition of this image
            nc.sync.dma_start(bt[p0:pe, 0:1, :], xb[p0 + 1 : pe + 1, 0:1, :])
            nc.sync.dma_start(
                bt[p0 + 1 : pe + 1, 1:2, :], xb[p0:pe, R - 1 : R, :]
            )
            nc.sync.dma_start(
                bt[pe : pe + 1, 0:1, :], xb[pe : pe + 1, R - 1 : R, :]
            )
            nc.gpsimd.memset(bt[p0 : p0 + 1, 1:2, :], PAD)

        # ---- V pass (bf16, 2x DVE): v[r] = min(row r-1, r, r+1) ----
        v = vt.tile([128, R, PW], BF16)
        tmp = vtm.tile([128, R - 1, PW], BF16)
        nc.vector.tensor_tensor(
            out=tmp[0:P], in0=xb[0:P, 0 : R - 1, :], in1=xb[0:P, 1:R, :], op=MIN
        )
        nc.vector.tensor_tensor(
            out=v[0:P, 1 : R - 1, :],
            in0=tmp[0:P, 0 : R - 2, :],
            in1=xb[0:P, 2:R, :],
            op=MIN,
        )
        nc.vector.tensor_tensor(
            out=v[0:P, 0:1, :], in0=tmp[0:P, 0:1, :], in1=bt[0:P, 1:2, :], op=MIN
        )
        nc.vector.tensor_tensor(
            out=v[0:P, R - 1 : R, :],
            in0=tmp[0:P, R - 2 : R - 1, :],
            in1=bt[0:P, 0:1, :],
            op=MIN,
        )

        # ---- H pass (bf16, 2x DVE): o[c] = min(v[c], v[c+1], v[c+2]) ----
        h = htm.tile([128, R, cc + 1], BF16)
        nc.vector.tensor_tensor(
            out=h[0:P], in0=v[0:P, :, 0 : cc + 1], in1=v[0:P, :, 1 : cc + 2],
            op=MIN,
        )
        ob = obp.tile([128, R, cc], BF16)
        nc.vector.tensor_tensor(
            out=ob[0:P], in0=h[0:P, :, 0:cc], in1=v[0:P, :, 2 : cc + 2], op=MIN
        )

        # store bf16 from ACT's queue (host widens to fp32 after gather)
        for im in range(n_img):
            dst = o_d[s0 + im, :, c0 : c0 + cc].rearrange(
                "(p r) w -> p r w", p=ppi
            )
            p0 = im * ppi
            nc.scalar.dma_start(dst, ob[p0 : p0 + ppi, :, :])

    # software-pipelined emission: tile i+skew's load/convert lands in
    # every queue before tile i's compute/store, so ACT's in-order
    # sequencer never delays DVE's next tile.
    skew = 2
    for i in range(len(tiles) + skew):
        if i < len(tiles):
            emit_front(i)
        if i >= skew:
            emit_back(i - skew)


def build_erosion(C, H, W, cc=None, rpp=RPP, reps=1, bufs=None, mode="f32eo"):
    """Per-core Bass program: x [C,H,W] f32 -> o [C,H,W] bf16, erosion^reps."""
    if cc is None:
        # f32eo's fp32 tiles need cc=512 to amortize per-op overhead while
        # fitting SBUF; chained (reps>1) builds add a bf16 load pool, so
        # drop to 256 columns.
        cc = 512 if (mode.startswith("f32eo") and reps == 1) else 256
    assert H % rpp == 0
    ppi = H // rpp
    assert ppi <= 128 and W % cc == 0

    nc = bacc.Bacc("TRN2", target_bir_lowering=False, debug=False, num_devices=1)
    x_d = nc.dram_tensor("x", [C, H, W], F32, kind="ExternalInput").ap()
    o_d = nc.dram_tensor("o", [C, H, W], BF16, kind="ExternalOutput").ap()
    # ping-pong DRAM scratch (bf16) for chained passes
    s_d = [
        nc.dram_tensor(f"scratch{i}", [C, H, W], BF16, kind="Internal").ap()
        for i in range(min(2, max(0, reps - 1)))
    ]

    def stage(i):
        src = x_d if i == 0 else s_d[(i - 1) % 2]
        dst = o_d if i == reps - 1 else s_d[i % 2]
        return src, dst

    if mode.startswith("f32eo"):
        bf = {"inp": 2, "xb": 2, "bnd": 2, "vtm": 1, "vt": 1, "htm": 1, "ob": 2}
    else:
        bf = {"inp": 4, "xb": 3, "bnd": 2, "vtm": 1, "vt": 1, "htm": 1, "ob": 3}
    if bufs:
        bf.update(bufs)
    emit = {
        "f32eo": _emit_pass_f32eo,
        "f32eo2": _emit_pass_f32eo2,
        "bf16": _emit_pass,
    }[mode]
    with tile.TileContext(nc) as tc:
        with (
            tc.tile_pool(name="inp", bufs=bf["inp"]) as inp,
            tc.tile_pool(name="xb", bufs=bf["xb"]) as xbp,
            tc.tile_pool(name="bnd", bufs=bf["bnd"]) as bnd,
            tc.tile_pool(name="vtm", bufs=bf["vtm"]) as vtm,
            tc.tile_pool(name="vt", bufs=bf["vt"]) as vt,
            tc.tile_pool(name="htm", bufs=bf["htm"]) as htm,
            tc.tile_pool(name="ob", bufs=bf["ob"]) as obp,
        ):
            pools = (inp, xbp, bnd, vtm, vt, htm, obp)
            for i in range(reps):
                src, dst = stage(i)
                emit(nc, pools, src, dst, C, H, W, cc, rpp, in_f32=(i == 0))
    nc.compile()
    return nc


def build_erosion_bf16a(
    C, H, W, mode="bf16b", cc_first=256, cc_chain=None, rpp=RPP, reps=1,
    bufs=None,
):
    """bf16-mode program builder (modes bf16a / bf16b).

    bf16a: aligned bf16 DVE + ACT parity-fix copy, boundary side-tile.
    bf16b: pure-DVE bf16, halo-row loads, odd-offset H op (measured only
    12% slower than aligned on HW -- no ACT round trip).
    The fp32 first pass uses cc_first columns (the fp32 load tile is the
    SBUF hog); chained bf16 passes use cc_chain. Tile pools size slots to
    the max across passes and unused pools allocate nothing.
    """
    if cc_chain is None:
        cc_chain = 512 if mode == "bf16b" else 256
    if mode.startswith("cg"):
        assert H % 256 == 0
    else:
        assert H % rpp == 0
        ppi = H // rpp
        assert ppi <= 128 and W % cc_first == 0 and W % cc_chain == 0

    nc = bacc.Bacc("TRN2", target_bir_lowering=False, debug=False, num_devices=1)
    x_d = nc.dram_tensor("x", [C, H, W], F32, kind="ExternalInput").ap()
    o_d = nc.dram_tensor("o", [C, H, W], BF16, kind="ExternalOutput").ap()
    # 3-deep scratch rotation: pass k's stores WAR-conflict only with pass
    # k-3's loads, so coarse DRAM dep tracking can't stall the pipeline.
    ns = min(3, max(0, reps - 1))
    s_d = [
        nc.dram_tensor(f"scratch{i}", [C, H, W], BF16, kind="Internal").ap()
        for i in range(ns)
    ]

    def stage(i):
        src = x_d if i == 0 else s_d[(i - 1) % ns]
        dst = o_d if i == reps - 1 else s_d[i % ns]
        return src, dst

    if mode.startswith("cg"):
        bf = {"t32": 2, "t": 3, "bnd": 3, "p": 2, "v": 2, "vs": 1, "g": 1,
              "ob": 3}
    elif mode == "bf16b":
        bf = {"t32": 2, "t": 2, "bnd": 1, "p": 1, "v": 1, "vs": 1, "g": 1,
              "ob": 3}
    else:
        bf = {"t32": 3, "t": 2, "bnd": 2, "p": 1, "v": 2, "vs": 2, "g": 1,
              "ob": 3}
    if bufs:
        bf.update(bufs)
    emit = {"bf16a": _emit_pass_bf16a, "bf16b": _emit_pass_bf16b}.get(mode)
    with tile.TileContext(nc) as tc:
        with (
            tc.tile_pool(name="t32", bufs=bf["t32"]) as t32p,
            tc.tile_pool(name="t", bufs=bf["t"]) as tp,
            tc.tile_pool(name="bnd", bufs=bf["bnd"]) as bnd,
            tc.tile_pool(name="p", bufs=bf["p"]) as pp,
            tc.tile_pool(name="v", bufs=bf["v"]) as vt,
            tc.tile_pool(name="vs", bufs=bf["vs"]) as vsp,
            tc.tile_pool(name="g", bufs=bf["g"]) as gp,
            tc.tile_pool(name="ob", bufs=bf["ob"]) as obp,
        ):
            pools = (t32p, tp, bnd, pp, vt, vsp, gp, obp)
            for i in range(reps):
                src, dst = stage(i)
                if mode.startswith("cg"):
                    probe = mode[2:].lstrip("_") or "none"
                    _emit_pass_cg(
                        nc, pools, src, dst, C, H, W, in_f32=(i == 0),
                        probe=probe,
                    )
                else:
                    cc = cc_first if i == 0 else cc_chain
                    emit(nc, pools, src, dst, C, H, W, cc, rpp, in_f32=(i == 0))
    nc.compile()
    return nc


def _get_program(C, H, W, reps=1, mode="auto"):
    if mode == "auto":
        mode = "cg" if reps == 1 else "rs"
    key = (C, H, W, reps, mode)
    if key not in _cache:
        if mode == "rs":
            _cache[key] = build_erosion_rs(C, H, W, reps=reps)
        elif mode in ("bf16a", "bf16b") or mode.startswith("cg"):
            _cache[key] = build_erosion_bf16a(C, H, W, mode=mode, reps=reps)
        else:
            _cache[key] = build_erosion(C, H, W, reps=reps, mode=mode)
    return _cache[key]


def kernel(x, m):
    from concourse.bass_utils import run_bass_kernel_spmd

    m = int(np.asarray(m))
    x = np.ascontiguousarray(np.asarray(x), dtype=np.float32)
    B, C, H, W = x.shape
    if m <= 0:
        return x.copy()
    # erosion by a (2m+1)-square = m chained 3x3 erosion passes in one NEFF
    nc = _get_program(C, H, W, reps=m)
    n_cores = 8
    assert B == n_cores, f"expected batch {n_cores}, got {B}"
    in_maps = [{"x": x[b]} for b in range(n_cores)]
    res = run_bass_kernel_spmd(nc, in_maps, core_ids=list(range(n_cores)))
    # device output is bf16; widen to fp32 on the host
    return np.stack(
        [np.asarray(r["o"]).astype(np.float32) for r in res.results], axis=0
    )


if __name__ == "__main__":
    # small-scale CoreSim correctness check (no hardware needed)
    import ml_dtypes

    from concourse.bass_interp import CoreSim

    rng = np.random.default_rng(0)
    C, H, W = 2, 256, 64
    x = rng.standard_normal((C, H, W)).astype(np.float32)

    def bf16r(a):
        return a.astype(ml_dtypes.bfloat16).astype(np.float32)

    for reps, mode in ((1, "cg"), (2, "cg"), (4, "cg")):
        if mode == "cg":
            nc = build_erosion_bf16a(C, H, W, mode=mode, reps=reps)
        elif mode in ("bf16a", "bf16b"):
            nc = build_erosion_bf16a(
                C, H, W, mode=mode, cc_first=16, cc_chain=32, rpp=4, reps=reps
            )
        else:
            nc = build_erosion(C, H, W, cc=32, rpp=4, reps=reps, mode=mode)
        sim = CoreSim(nc)
        sim.tensor("x")[:] = x
        sim.simulate(check_with_hw=False)
        got = np.asarray(sim.tensor("o")).astype(np.float32)

        # bf16a converts the input to bf16 before the first pass
        exp = bf16r(x) if mode in ("bf16", "bf16a") else x
        for _ in range(reps):
            xp = np.pad(exp, ((0, 0), (1, 1), (1, 1)), constant_values=PAD)
            nxt = np.empty_like(exp)
            for i in range(H):
                for j in range(W):
                    nxt[:, i, j] = xp[:, i : i + 3, j : j + 3].min(axis=(1, 2))
            exp = bf16r(nxt)  # device stores bf16 each pass
        ok = np.array_equal(got, exp)
        rel = np.max(np.abs(got - exp) / np.maximum(np.abs(exp), 1e-6))
        print(f"CoreSim reps={reps} mode={mode} exact: {ok} rel={rel:.2e}")



# revision 41
# speedup vs baseline: 8.3450x; 1.1335x over previous
"""Trainium2 Bass kernel for 3x3 (applied m times) morphological erosion
(sliding-window min) over [B, C, H, W] fp32, B=8 sharded across 8
NeuronCores (one batch per core).

v5 scheme, driven by direct HW microbenchmarks (see git-less history in
the session notes; measured on the axon-tunneled trn2 cores):
  - DVE tensor_tensor min bf16 = 0.372 ns/elem-per-partition (0.415 with
    one odd-offset operand) vs fp32 = 1.097 -> ALL mins run in bf16. min
    never creates values, so total error = one bf16 rounding (~4e-3 rel),
    far inside the 2e-2 gate. Device stores bf16; host widens to fp32.
  - DMA cost is descriptor-dominated: [64p x 18r x ~1KB] strided loads
    measured ~45 GB/s, while >=16KB-contiguous-per-partition descriptors
    run at ~350 GB/s each way concurrently. So: one image per
    128-partition stack, partition p holds R=H/128 FULL rows -- every
    image load/store is ONE DMA with 128 contiguous 16-32KB descriptors.
  - m == 1 ("cg" mode): streaming passes; fp32 staged in half-tiles,
    ACT converts to bf16, V pass = even/odd-shared decomposition
    (1.5 ops/elem, aligned), boundary rows via 2 partition-shifted
    SBUF->SBUF row copies + 2 tiny edge duplicates, H pass = h/ob with
    one odd-offset op; per-column-edge 1-column ops instead of PAD.
  - m > 1 ("rs" mode): the whole shard (C images x 16KB/partition, bf16)
    stays RESIDENT in SBUF: load once (gpsimd casting DMA fp32->bf16),
    run m passes on-chip (pure DVE + 4 small boundary DMAs per image per
    pass, C+1-slot rotating image pool), store once. Per-pass marginal
    cost ~93 us vs ~300 us for the DRAM ping-pong baseline.
"""

import sys

sys.path.insert(0, "/opt/trn_rl_repo")

import numpy as np

import concourse.bass as bass
import concourse.tile as tile
from concourse import bacc, mybir

PAD = 1.0e9
F32 = mybir.dt.float32
BF16 = mybir.dt.bfloat16
MIN = mybir.AluOpType.min

CC = 256  # column chunk width
RPP = 16  # image rows per partition

_cache = {}


def _emit_pass_f32eo2(nc, pools, x_d, o_d, C, H, W, cc, rpp, in_f32):
    """One erosion pass, fp32 compute, even/odd V pass over halo-loaded rows.

    Each partition loads R+2 rows (its R rows plus one halo row each side,
    overlapping DRAM reads; +2/R load bytes) so the V pass is exactly 3
    row-stepped DVE ops with no boundary fixups:
      p[j] = min(e[2j], e[2j+1])          j = 0..R/2
      v[2j] = min(p[j], e[2j+2])          j = 0..R/2-1
      v[2j+1] = min(e[2j+1], p[j+1])      j = 0..R/2-1
    Image-edge partitions: top halo row = PAD memset (start partitions
    0/64: legal), bottom halo row = own-last-row duplication DMA
    (min-idempotent). All slices keep unit innermost stride (full DVE
    rate). The final H op writes bf16 (free output cast); host widens.
    """
    from concourse.ap import AP as _AP

    ppi = H // rpp
    ips = max(1, 128 // ppi)
    inp, xbp, bnd, vtm, vt, htm, obp = pools
    R = rpp
    E = R + 2  # rows incl halo
    PW = cc + 2
    assert R % 2 == 0

    tiles = [(s0, c0) for s0 in range(0, C, ips) for c0 in range(0, W, cc)]
    front = {}

    def emit_front(i):
        s0, c0 = tiles[i]
        n_img = min(ips, C - s0)
        P = n_img * ppi
        last = c0 + cc == W
        t = inp.tile([128, E, PW], F32, name="t")
        if in_f32:
            ld = t
        else:
            ld = xbp.tile([128, E, PW], BF16, name="xb")
        wlo = max(c0 - 1, 0)
        whi = min(c0 + cc + 1, W)
        nw = whi - wlo
        dlo = 1 if c0 == 0 else 0
        for im in range(n_img):
            img = x_d[s0 + im]  # [H, W]
            p0 = im * ppi
            # interior partitions 1..ppi-2: rows rpp*p-1 .. rpp*p+rpp
            src = _AP(
                tensor=img.tensor,
                offset=img.offset + (rpp - 1) * W + wlo,
                ap=[[rpp * W, ppi - 2], [W, E], [1, nw]],
            )
            nc.sync.dma_start(
                ld[p0 + 1 : p0 + ppi - 1, :, dlo : dlo + nw], src
            )
            # top partition: rows 0..rpp into slots 1..E-1; slot 0 = PAD
            nc.sync.dma_start(
                ld[p0 : p0 + 1, 1:E, dlo : dlo + nw],
                img[0 : rpp + 1, wlo:whi].rearrange("(p r) w -> p r w", p=1),
            )
            nc.gpsimd.memset(ld[p0 : p0 + 1, 0:1, :], PAD)
            # bottom partition: rows H-rpp-1..H-1 into slots 0..E-2;
            # slot E-1 = duplicate of the image's last row (min-idempotent)
            pe = p0 + ppi - 1
            nc.sync.dma_start(
                ld[pe : pe + 1, 0 : E - 1, dlo : dlo + nw],
                img[H - rpp - 1 : H, wlo:whi].rearrange("(p r) w -> p r w", p=1),
            )
            nc.sync.dma_start(
                ld[pe : pe + 1, E - 1 : E, dlo : dlo + nw],
                img[H - 1 : H, wlo:whi].rearrange("(p r) w -> p r w", p=1),
            )
        if c0 == 0:
            nc.gpsimd.memset(ld[0:P, :, 0:1], PAD)
        if last:
            nc.gpsimd.memset(ld[0:P, :, PW - 1 : PW], PAD)
        if not in_f32:
            nc.scalar.copy(t[0:P], ld[0:P])
        front[i] = (t, n_img, P, s0, c0)

    def emit_back(i):
        t, n_img, P, s0, c0 = front.pop(i)
        Rh = R // 2
        v = vt.tile([128, R, PW], F32, name="v")
        p = vtm.tile([128, Rh + 1, PW], F32, name="p")
        nc.vector.tensor_tensor(
            out=p[0:P], in0=t[0:P, 0:E:2, :], in1=t[0:P, 1:E:2, :], op=MIN
        )
        nc.vector.tensor_tensor(
            out=v[0:P, 0:R:2, :],
            in0=p[0:P, 0:Rh, :],
            in1=t[0:P, 2:E:2, :],
            op=MIN,
        )
        nc.vector.tensor_tensor(
            out=v[0:P, 1:R:2, :],
            in0=t[0:P, 1 : E - 2 : 2, :],
            in1=p[0:P, 1 : Rh + 1, :],
            op=MIN,
        )

        h = htm.tile([128, R, cc + 1], F32, name="h")
        nc.vector.tensor_tensor(
            out=h[0:P], in0=v[0:P, :, 0 : cc + 1], in1=v[0:P, :, 1 : cc + 2],
            op=MIN,
        )
        ob = obp.tile([128, R, cc], BF16, name="ob")
        nc.vector.tensor_tensor(
            out=ob[0:P], in0=h[0:P, :, 0:cc], in1=v[0:P, :, 2 : cc + 2], op=MIN
        )

        for im in range(n_img):
            dst = o_d[s0 + im, :, c0 : c0 + cc].rearrange(
                "(p r) w -> p r w", p=ppi
            )
            p0 = im * ppi
            nc.scalar.dma_start(dst, ob[p0 : p0 + ppi, :, :])

    skew = 2
    for i in range(len(tiles) + skew):
        if i < len(tiles):
            emit_front(i)
        if i >= skew:
            emit_back(i - skew)


def _emit_pass_f32eo(nc, pools, x_d, o_d, C, H, W, cc, rpp, in_f32):
    """One erosion pass, fp32 compute with even/odd-shared V pass.

    x_d fp32 (first pass) or bf16 (chained); o_d bf16. All mins on DVE in
    fp32 (bf16 ALU is slower on real HW); V pass uses the pairwise-sharing
    decomposition (1.5 ops/elem): p[j] = min(r[2j], r[2j+1]);
    v[2j] = min(r[2j-1], p[j]); v[2j+1] = min(p[j], r[2j+2]). Row-stepped
    slices keep unit innermost stride (full DVE rate). The final H op
    writes bf16 (free output cast).
    """
    ppi = H // rpp
    ips = max(1, 128 // ppi)
    inp, xbp, bnd, vtm, vt, htm, obp = pools
    R = rpp
    PW = cc + 2
    assert R % 2 == 0

    tiles = [(s0, c0) for s0 in range(0, C, ips) for c0 in range(0, W, cc)]
    front = {}

    def emit_front(i):
        s0, c0 = tiles[i]
        n_img = min(ips, C - s0)
        P = n_img * ppi
        last = c0 + cc == W
        # load tile: fp32 on first pass, bf16 on chained passes; compute
        # stays fp32 either way (bf16 load is upconverted by ACT)
        t = inp.tile([128, R, PW], F32, name="t")
        if in_f32:
            ld = t
        else:
            ld = xbp.tile([128, R, PW], BF16, name="xb")
        wlo = max(c0 - 1, 0)
        whi = min(c0 + cc + 1, W)
        dlo = 1 if c0 == 0 else 0
        for im in range(n_img):
            src = x_d[s0 + im, :, wlo:whi].rearrange("(p r) w -> p r w", p=ppi)
            p0 = im * ppi
            nc.sync.dma_start(ld[p0 : p0 + ppi, :, dlo : dlo + (whi - wlo)], src)
        if c0 == 0:
            nc.gpsimd.memset(ld[0:P, :, 0:1], PAD)
        if last:
            nc.gpsimd.memset(ld[0:P, :, PW - 1 : PW], PAD)
        if not in_f32:
            nc.scalar.copy(t[0:P], ld[0:P])
        front[i] = (t, n_img, P, s0, c0)

    def emit_back(i):
        t, n_img, P, s0, c0 = front.pop(i)
        bt = bnd.tile([128, 2, PW], F32, name="bt")
        for im in range(n_img):
            p0 = im * ppi
            pe = p0 + ppi - 1
            nc.sync.dma_start(bt[p0:pe, 0:1, :], t[p0 + 1 : pe + 1, 0:1, :])
            nc.sync.dma_start(
                bt[p0 + 1 : pe + 1, 1:2, :], t[p0:pe, R - 1 : R, :]
            )
            nc.sync.dma_start(
                bt[pe : pe + 1, 0:1, :], t[pe : pe + 1, R - 1 : R, :]
            )
            nc.gpsimd.memset(bt[p0 : p0 + 1, 1:2, :], PAD)

        # ---- V pass, even/odd shared (fp32) ----
        v = vt.tile([128, R, PW], F32, name="v")
        p = vtm.tile([128, R // 2, PW], F32, name="p")
        Rh = R // 2
        nc.vector.tensor_tensor(
            out=p[0:P], in0=t[0:P, 0:R:2, :], in1=t[0:P, 1:R:2, :], op=MIN
        )
        # v[2j] = min(r[2j-1], p[j]), j=1..Rh-1 (j=0 uses bt row-above)
        nc.vector.tensor_tensor(
            out=v[0:P, 2:R:2, :],
            in0=t[0:P, 1 : R - 2 : 2, :],
            in1=p[0:P, 1:Rh, :],
            op=MIN,
        )
        nc.vector.tensor_tensor(
            out=v[0:P, 0:1, :], in0=p[0:P, 0:1, :], in1=bt[0:P, 1:2, :], op=MIN
        )
        # v[2j+1] = min(p[j], r[2j+2]), j=0..Rh-2 (j=Rh-1 uses bt row-below)
        nc.vector.tensor_tensor(
            out=v[0:P, 1 : R - 1 : 2, :],
            in0=p[0:P, 0 : Rh - 1, :],
            in1=t[0:P, 2:R:2, :],
            op=MIN,
        )
        nc.vector.tensor_tensor(
            out=v[0:P, R - 1 : R, :],
            in0=p[0:P, Rh - 1 : Rh, :],
            in1=bt[0:P, 0:1, :],
            op=MIN,
        )

        # ---- H pass (fp32, last op casts to bf16 on output) ----
        h = htm.tile([128, R, cc + 1], F32, name="h")
        nc.vector.tensor_tensor(
            out=h[0:P], in0=v[0:P, :, 0 : cc + 1], in1=v[0:P, :, 1 : cc + 2],
            op=MIN,
        )
        ob = obp.tile([128, R, cc], BF16, name="ob")
        nc.vector.tensor_tensor(
            out=ob[0:P], in0=h[0:P, :, 0:cc], in1=v[0:P, :, 2 : cc + 2], op=MIN
        )

        for im in range(n_img):
            dst = o_d[s0 + im, :, c0 : c0 + cc].rearrange(
                "(p r) w -> p r w", p=ppi
            )
            p0 = im * ppi
            nc.scalar.dma_start(dst, ob[p0 : p0 + ppi, :, :])

    skew = 2
    for i in range(len(tiles) + skew):
        if i < len(tiles):
            emit_front(i)
        if i >= skew:
            emit_back(i - skew)


def _emit_pass_cg(nc, pools, x_d, o_d, C, H, W, in_f32, probe="none"):
    """One erosion pass, contiguous-big-DMA layout + bf16 DVE compute.

    HW measurements drove this shape:
      - DVE TT min bf16 = 0.372 ns/elem-pp (0.415 odd-offset) vs fp32 1.097
        -> all mins in bf16.
      - DMA throughput is descriptor-dominated: the old [ppi,R,~1KB-line]
        strided loads measured ~45 GB/s; >=16KB contiguous-per-partition
        descriptors reach ~350-400 GB/s.
    So: one image per 128-partition stack, partition p holds R=H/128 FULL
    rows (contiguous 8KB-32KB span in DRAM and in SBUF) -> ONE descriptor
    per partition per image load/store. Per image per pass: 1 big load
    (sync queue), 2 partition-shifted SBUF->SBUF row copies for the V-pass
    boundary rows (scalar queue), 1 big store (scalar queue). Image-edge
    partitions get their boundary row via tiny 1-partition DVE ops reading
    the tile's own edge row (min-idempotent duplicate) -- no memsets, no
    PAD columns: the H-pass edge columns use dedicated 1-column ops.
    """
    R = H // 128
    Rh = R // 2
    assert H % 128 == 0 and R % 2 == 0
    t32p, tp, bnd, pp, vt, _vsp, hp, obp = pools
    front = {}

    def emit_front(i):
        from concourse.ap import AP as _AP

        t = tp.tile([128, R, W], BF16, name="t")
        if in_f32:
            # fp32 staging in two half-tiles: halves the t32 SBUF slot so
            # the bf16 pipeline pools can run deeper
            Rhalf = R // 2
            img = x_d[i]
            for c in range(2):
                ld = t32p.tile([128, Rhalf, W], F32, name="t32")
                src_c = _AP(
                    tensor=img.tensor,
                    offset=img.offset + c * Rhalf * W,
                    ap=[[R * W, 128], [1, Rhalf * W]],
                )
                nc.sync.dma_start(ld.rearrange("p r w -> p (r w)"), src_c)
                nc.scalar.copy(t[:, c * Rhalf : (c + 1) * Rhalf, :], ld)
        else:
            src = x_d[i].rearrange("(p r) w -> p (r w)", p=128)
            nc.sync.dma_start(t.rearrange("p r w -> p (r w)"), src)
        # bt[p,0] = row above (prev partition's last row),
        # bt[p,1] = row below (next partition's first row)
        if probe == "dma":
            front[i] = (t, None)
            return
        bt = bnd.tile([128, 2, W], BF16, name="bt")
        nc.scalar.dma_start(bt[1:128, 0:1, :], t[0:127, R - 1 : R, :])
        nc.scalar.dma_start(bt[0:127, 1:2, :], t[1:128, 0:1, :])
        # image-edge partitions duplicate their own edge row (the DVE
        # rejects partition starts other than 0/32/64/96, so these are DMAs)
        nc.scalar.dma_start(bt[0:1, 0:1, :], t[0:1, 0:1, :])
        nc.scalar.dma_start(bt[127:128, 1:2, :], t[127:128, R - 1 : R, :])
        front[i] = (t, bt)

    def emit_back(i):
        t, bt = front.pop(i)
        if probe == "dma":  # timing probe: load->store only
            dst = o_d[i].rearrange("(p r) w -> p (r w)", p=128)
            nc.scalar.dma_start(dst, t.rearrange("p r w -> p (r w)"))
            return
        # ---- V pass (bf16; all aligned) ----
        p = pp.tile([128, Rh, W], BF16, name="p")
        v = vt.tile([128, R, W], BF16, name="v")
        nc.vector.tensor_tensor(
            out=p, in0=t[:, 0:R:2, :], in1=t[:, 1:R:2, :], op=MIN
        )
        if Rh > 1:
            # v[2j] = min(r[2j-1], p[j]), j=1..Rh-1
            nc.vector.tensor_tensor(
                out=v[:, 2:R:2, :],
                in0=t[:, 1 : R - 2 : 2, :],
                in1=p[:, 1:Rh, :],
                op=MIN,
            )
            # v[2j+1] = min(p[j], r[2j+2]), j=0..Rh-2
            nc.vector.tensor_tensor(
                out=v[:, 1 : R - 1 : 2, :],
                in0=p[:, 0 : Rh - 1, :],
                in1=t[:, 2:R:2, :],
                op=MIN,
            )
        # v[0] = min(p[0], row-above); v[R-1] = min(p[Rh-1], row-below)
        if Rh >= 2:
            nc.vector.tensor_tensor(
                out=v[:, 0 : R : R - 1, :],
                in0=p[:, 0 : Rh : Rh - 1, :],
                in1=bt[:, 0:2, :],
                op=MIN,
            )
        else:
            nc.vector.tensor_tensor(
                out=v[:, 0:1, :], in0=p[:, 0:1, :], in1=bt[:, 0:1, :], op=MIN
            )
            nc.vector.tensor_tensor(
                out=v[:, R - 1 : R, :],
                in0=p[:, Rh - 1 : Rh, :],
                in1=bt[:, 1:2, :],
                op=MIN,
            )
        if probe == "noh":  # timing probe: V pass only, store v
            dst = o_d[i].rearrange("(p r) w -> p (r w)", p=128)
            nc.scalar.dma_start(dst, v.rearrange("p r w -> p (r w)"))
            return
        # ---- H pass: h[c] = min(v[c], v[c+1]); ob interior + edge cols ----
        h = hp.tile([128, R, W], BF16, name="h")
        nc.vector.tensor_tensor(
            out=h[:, :, 0 : W - 1],
            in0=v[:, :, 0 : W - 1],
            in1=v[:, :, 1:W],
            op=MIN,
        )
        ob = obp.tile([128, R, W], BF16, name="ob")
        nc.vector.tensor_tensor(
            out=ob[:, :, 1 : W - 1],
            in0=h[:, :, 0 : W - 2],
            in1=v[:, :, 2:W],
            op=MIN,
        )
        nc.vector.tensor_tensor(
            out=ob[:, :, 0:1], in0=h[:, :, 0:1], in1=v[:, :, 1:2], op=MIN
        )
        nc.vector.tensor_copy(
            out=ob[:, :, W - 1 : W], in_=h[:, :, W - 2 : W - 1]
        )
        dst = o_d[i].rearrange("(p r) w -> p (r w)", p=128)
        nc.scalar.dma_start(dst, ob.rearrange("p r w -> p (r w)"))

    skew = 2
    for i in range(C + skew):
        if i < C:
            emit_front(i)
        if i >= skew:
            emit_back(i - skew)


def build_erosion_rs(C, H, W, reps, flat=False):
    """SBUF-resident multi-pass erosion: the whole per-core shard (C images
    x H/128 rows x W cols, bf16 = 16 KiB/partition/image) stays in SBUF
    across all passes.

    Load once (gpsimd casting DMA, fp32->bf16 inline), run `reps` erosion
    passes entirely on-chip (DVE mins + 4 small SBUF->SBUF boundary-row
    DMAs per image per pass), store once. Per-pass marginal cost is pure
    DVE (~90 us) -- no HBM traffic per pass. The image pool has C+1
    slots; each pass writes image i's result into the rotating spare slot
    and the old buffer becomes the next spare (Tile's WAR deps make the
    rotation safe).
    """
    R = H // 128
    Rh = R // 2
    assert H % 256 == 0 and R % 2 == 0 and reps >= 1

    nc = bacc.Bacc("TRN2", target_bir_lowering=False, debug=False, num_devices=1)
    x_d = nc.dram_tensor("x", [C, H, W], F32, kind="ExternalInput").ap()
    o_d = nc.dram_tensor("o", [C, H, W], BF16, kind="ExternalOutput").ap()

    with tile.TileContext(nc) as tc:
        with (
            tc.tile_pool(name="img", bufs=C + 1) as imgp,
            tc.tile_pool(name="bnd", bufs=3) as bnd,
            tc.tile_pool(name="p", bufs=1) as pp,
            tc.tile_pool(name="v", bufs=1) as vt,
            tc.tile_pool(name="h", bufs=1) as hp,
        ):
            img_t = []
            for i in range(C):
                t = imgp.tile([128, R, W], BF16, name="img")
                nc.gpsimd.dma_start(
                    t.rearrange("p r w -> p (r w)"),
                    x_d[i].rearrange("(p r) w -> p (r w)", p=128),
                )
                img_t.append(t)

            bts = {}

            def emit_bt(i):
                t = img_t[i]
                bt = bnd.tile([128, 2, W], BF16, name="bt")
                nc.scalar.dma_start(bt[1:128, 0:1, :], t[0:127, R - 1 : R, :])
                nc.scalar.dma_start(bt[0:127, 1:2, :], t[1:128, 0:1, :])
                nc.scalar.dma_start(bt[0:1, 0:1, :], t[0:1, 0:1, :])
                nc.scalar.dma_start(
                    bt[127:128, 1:2, :], t[127:128, R - 1 : R, :]
                )
                bts[i] = bt

            def emit_compute(i):
                t = img_t[i]
                bt = bts.pop(i)
                p = pp.tile([128, Rh, W], BF16, name="p")
                v = vt.tile([128, R, W], BF16, name="v")
                nc.vector.tensor_tensor(
                    out=p, in0=t[:, 0:R:2, :], in1=t[:, 1:R:2, :], op=MIN
                )
                if Rh > 1:
                    nc.vector.tensor_tensor(
                        out=v[:, 2:R:2, :],
                        in0=t[:, 1 : R - 2 : 2, :],
                        in1=p[:, 1:Rh, :],
                        op=MIN,
                    )
                    nc.vector.tensor_tensor(
                        out=v[:, 1 : R - 1 : 2, :],
                        in0=p[:, 0 : Rh - 1, :],
                        in1=t[:, 2:R:2, :],
                        op=MIN,
                    )
                if Rh >= 2:
                    # fused boundary rows: out rows {0, R-1}, p rows
                    # {0, Rh-1}, bt rows {0, 1} -- one strided op
                    nc.vector.tensor_tensor(
                        out=v[:, 0 : R : R - 1, :],
                        in0=p[:, 0 : Rh : Rh - 1, :],
                        in1=bt[:, 0:2, :],
                        op=MIN,
                    )
                else:
                    nc.vector.tensor_tensor(
                        out=v[:, 0:1, :], in0=p[:, 0:1, :],
                        in1=bt[:, 0:1, :], op=MIN,
                    )
                    nc.vector.tensor_tensor(
                        out=v[:, R - 1 : R, :],
                        in0=p[:, Rh - 1 : Rh, :],
                        in1=bt[:, 1:2, :],
                        op=MIN,
                    )
                h = hp.tile([128, R, W], BF16, name="h")
                nxt = imgp.tile([128, R, W], BF16, name="img")
                if flat:
                    # flat 2D APs over the packed rows; the row-seam
                    # positions compute garbage (cross-row mins) but they
                    # are exactly the positions the edge-column ops below
                    # overwrite.
                    RW = R * W
                    vf = v.rearrange("p r w -> p (r w)")
                    hf = h.rearrange("p r w -> p (r w)")
                    nf = nxt.rearrange("p r w -> p (r w)")
                    nc.vector.tensor_tensor(
                        out=hf[:, 0 : RW - 1],
                        in0=vf[:, 0 : RW - 1],
                        in1=vf[:, 1:RW],
                        op=MIN,
                    )
                    nc.vector.tensor_tensor(
                        out=nf[:, 1 : RW - 1],
                        in0=hf[:, 0 : RW - 2],
                        in1=vf[:, 2:RW],
                        op=MIN,
                    )
                else:
                    nc.vector.tensor_tensor(
                        out=h[:, :, 0 : W - 1],
                        in0=v[:, :, 0 : W - 1],
                        in1=v[:, :, 1:W],
                        op=MIN,
                    )
                    nc.vector.tensor_tensor(
                        out=nxt[:, :, 1 : W - 1],
                        in0=h[:, :, 0 : W - 2],
                        in1=v[:, :, 2:W],
                        op=MIN,
                    )
                nc.vector.tensor_tensor(
                    out=nxt[:, :, 0:1], in0=h[:, :, 0:1], in1=v[:, :, 1:2],
                    op=MIN,
                )
                nc.vector.tensor_copy(
                    out=nxt[:, :, W - 1 : W], in_=h[:, :, W - 2 : W - 1]
                )
                img_t[i] = nxt

            for k in range(reps):
                for i in range(C + 1):
                    if i < C:
                        emit_bt(i)
                    if i >= 1:
                        emit_compute(i - 1)

            for i in range(C):
                nc.sync.dma_start(
                    o_d[i].rearrange("(p r) w -> p (r w)", p=128),
                    img_t[i].rearrange("p r w -> p (r w)"),
                )
    nc.compile()
    return nc


def _emit_pass_bf16b(nc, pools, x_d, o_d, C, H, W, cc, rpp, in_f32):
    """One erosion pass, pure-DVE bf16 compute, halo-row loads (f32eo2
    style), no ACT compute and no SBUF->SBUF boundary DMAs.

    Measured on HW (microbench): TT min bf16 = 0.372 ns/elem-per-partition
    when 4B-aligned and 0.415 when one operand starts at an odd column --
    both ~2.9x faster than fp32 (1.097). So compute everything in bf16 on
    the DVE: V pass = even/odd shared decomposition over the halo-loaded
    rows (all aligned), H pass = h(odd operand) then ob (aligned). The
    only ACT work is the fp32->bf16 convert on the first pass; stores
    ride the ACT queue as plain triggers.
    """
    from concourse.ap import AP as _AP

    ppi = H // rpp
    ips = max(1, 128 // ppi)
    t32p, tp, _bnd, pp, vt, _vsp, hp, obp = pools
    R = rpp
    Rh = R // 2
    E = R + 2
    PW = cc + 2
    assert R % 2 == 0 and cc % 2 == 0

    tiles = [(s0, c0) for s0 in range(0, C, ips) for c0 in range(0, W, cc)]
    front = {}

    def emit_front(i):
        s0, c0 = tiles[i]
        n_img = min(ips, C - s0)
        P = n_img * ppi
        last = c0 + cc == W
        t = tp.tile([128, E, PW], BF16, name="t")
        ld = t32p.tile([128, E, PW], F32, name="t32") if in_f32 else t
        wlo = max(c0 - 1, 0)
        whi = min(c0 + cc + 1, W)
        nw = whi - wlo
        dlo = 1 if c0 == 0 else 0
        for im in range(n_img):
            img = x_d[s0 + im]  # [H, W]
            p0 = im * ppi
            # interior partitions 1..ppi-2: rows rpp*p-1 .. rpp*p+rpp
            src = _AP(
                tensor=img.tensor,
                offset=img.offset + (rpp - 1) * W + wlo,
                ap=[[rpp * W, ppi - 2], [W, E], [1, nw]],
            )
            nc.sync.dma_start(ld[p0 + 1 : p0 + ppi - 1, :, dlo : dlo + nw], src)
            # top partition: rows 0..rpp into slots 1..E-1; slot 0 = row 0
            # again (min-idempotent duplicate stands in for the PAD row)
            nc.sync.dma_start(
                ld[p0 : p0 + 1, 1:E, dlo : dlo + nw],
                img[0 : rpp + 1, wlo:whi].rearrange("(p r) w -> p r w", p=1),
            )
            nc.sync.dma_start(
                ld[p0 : p0 + 1, 0:1, dlo : dlo + nw],
                img[0:1, wlo:whi].rearrange("(p r) w -> p r w", p=1),
            )
            # bottom partition: rows H-rpp-1..H-1 into slots 0..E-2;
            # slot E-1 = duplicate of the image's last row
            pe = p0 + ppi - 1
            nc.sync.dma_start(
                ld[pe : pe + 1, 0 : E - 1, dlo : dlo + nw],
                img[H - rpp - 1 : H, wlo:whi].rearrange("(p r) w -> p r w", p=1),
            )
            nc.sync.dma_start(
                ld[pe : pe + 1, E - 1 : E, dlo : dlo + nw],
                img[H - 1 : H, wlo:whi].rearrange("(p r) w -> p r w", p=1),
            )
        if c0 == 0:
            nc.gpsimd.memset(ld[0:P, :, 0:1], PAD)
        if last:
            nc.gpsimd.memset(ld[0:P, :, PW - 1 : PW], PAD)
        if in_f32:
            nc.scalar.copy(t[0:P], ld[0:P])
        front[i] = (t, n_img, P, s0, c0)

    def emit_back(i):
        t, n_img, P, s0, c0 = front.pop(i)
        # ---- V pass over halo rows (bf16, aligned -> fast) ----
        p = pp.tile([128, Rh + 1, PW], BF16, name="p")
        v = vt.tile([128, R, PW], BF16, name="v")
        nc.vector.tensor_tensor(
            out=p[0:P], in0=t[0:P, 0:E:2, :], in1=t[0:P, 1:E:2, :], op=MIN
        )
        nc.vector.tensor_tensor(
            out=v[0:P, 0:R:2, :],
            in0=p[0:P, 0:Rh, :],
            in1=t[0:P, 2:E:2, :],
            op=MIN,
        )
        nc.vector.tensor_tensor(
            out=v[0:P, 1:R:2, :],
            in0=t[0:P, 1 : E - 2 : 2, :],
            in1=p[0:P, 1 : Rh + 1, :],
            op=MIN,
        )
        # ---- H pass: h has one odd-offset operand (12% slower, fine) ----
        h = hp.tile([128, R, PW], BF16, name="h")
        nc.vector.tensor_tensor(
            out=h[0:P, :, 0 : cc + 1],
            in0=v[0:P, :, 0 : cc + 1],
            in1=v[0:P, :, 1 : cc + 2],
            op=MIN,
        )
        ob = obp.tile([128, R, cc], BF16, name="ob")
        nc.vector.tensor_tensor(
            out=ob[0:P], in0=h[0:P, :, 0:cc], in1=v[0:P, :, 2 : cc + 2], op=MIN
        )
        for im in range(n_img):
            dst = o_d[s0 + im, :, c0 : c0 + cc].rearrange(
                "(p r) w -> p r w", p=ppi
            )
            p0 = im * ppi
            nc.scalar.dma_start(dst, ob[p0 : p0 + ppi, :, :])

    skew = 2
    for i in range(len(tiles) + skew):
        if i < len(tiles):
            emit_front(i)
        if i >= skew:
            emit_back(i - skew)


def _emit_pass_bf16a(nc, pools, x_d, o_d, C, H, W, cc, rpp, in_f32):
    """One erosion pass, all-bf16 DVE compute with every tensor_tensor
    operand 4B-aligned so the DVE runs at the 2x_1p rate throughout.

    The 2x_1p perf mode needs 16-bit dtype, unit innermost stride AND a
    4-byte-aligned start address for every operand. The H pass combines
    columns of both parities, so one op would always have a 2-byte-offset
    (odd) operand and silently drop to 1x. Fix: the otherwise-idle ACT
    engine makes a one-column-shifted copy vs[c] = v[c+1] (ACT is 1x at
    1.2 GHz regardless of alignment), and the DVE computes
        g[c]  = min(v[c], v[c+2])   (both even offsets, 2x)
        ob[c] = min(g[c], vs[c])    (both aligned, 2x)
    V pass is the even/odd-shared decomposition (1.5 ops/elem, all row
    slices start at even column 0 -> aligned, 2x). Per-partition boundary
    rows come from a small side tile bt filled by partition-shifted
    SBUF->SBUF DMAs; image edges use own-row duplication (min-idempotent)
    so no memsets on partition starts other than 0 are needed.

    DVE ~1.75 cyc/elem (vs 3.5 at 1x fp32), ACT 1 copy/elem (+1 convert
    on the first pass), DMA loads R rows/partition (no halo rows).
    """
    ppi = H // rpp
    ips = max(1, 128 // ppi)
    t32p, tp, bnd, pp, vt, vsp, gp, obp = pools
    R = rpp
    Rh = R // 2
    PW = cc + 2
    assert R % 2 == 0 and cc % 2 == 0 and PW % 2 == 0

    tiles = [(s0, c0) for s0 in range(0, C, ips) for c0 in range(0, W, cc)]
    front = {}
    mid = {}

    def emit_front(i):
        s0, c0 = tiles[i]
        n_img = min(ips, C - s0)
        P = n_img * ppi
        last = c0 + cc == W
        t = tp.tile([128, R, PW], BF16, name="t")
        ld = t32p.tile([128, R, PW], F32, name="t32") if in_f32 else t
        wlo = max(c0 - 1, 0)
        whi = min(c0 + cc + 1, W)
        dlo = 1 if c0 == 0 else 0
        for im in range(n_img):
            src = x_d[s0 + im, :, wlo:whi].rearrange("(p r) w -> p r w", p=ppi)
            p0 = im * ppi
            nc.sync.dma_start(ld[p0 : p0 + ppi, :, dlo : dlo + (whi - wlo)], src)
        if c0 == 0:
            nc.gpsimd.memset(ld[0:P, :, 0:1], PAD)
        if last:
            nc.gpsimd.memset(ld[0:P, :, PW - 1 : PW], PAD)
        if in_f32:
            nc.scalar.copy(t[0:P], ld[0:P])
        front[i] = (t, n_img, P, s0, c0)

    def emit_mid(i):
        t, n_img, P, s0, c0 = front.pop(i)
        # bt[p,0] = row below the block (next partition's row 0),
        # bt[p,1] = row above (prev partition's row R-1); image edges
        # duplicate the block's own edge row (min-idempotent).
        bt = bnd.tile([128, 2, PW], BF16, name="bt")
        for im in range(n_img):
            p0 = im * ppi
            pe = p0 + ppi - 1
            nc.sync.dma_start(bt[p0:pe, 0:1, :], t[p0 + 1 : pe + 1, 0:1, :])
            nc.sync.dma_start(bt[pe : pe + 1, 0:1, :], t[pe : pe + 1, R - 1 : R, :])
            nc.sync.dma_start(bt[p0 + 1 : pe + 1, 1:2, :], t[p0:pe, R - 1 : R, :])
            nc.sync.dma_start(bt[p0 : p0 + 1, 1:2, :], t[p0 : p0 + 1, 0:1, :])

        # ---- V pass (bf16, all operands 4B-aligned -> 2x) ----
        p = pp.tile([128, Rh, PW], BF16, name="p")
        v = vt.tile([128, R, PW], BF16, name="v")
        nc.vector.tensor_tensor(
            out=p[0:P], in0=t[0:P, 0:R:2, :], in1=t[0:P, 1:R:2, :], op=MIN
        )
        if Rh > 1:
            nc.vector.tensor_tensor(
                out=v[0:P, 2:R:2, :],
                in0=t[0:P, 1 : R - 2 : 2, :],
                in1=p[0:P, 1:Rh, :],
                op=MIN,
            )
            nc.vector.tensor_tensor(
                out=v[0:P, 1 : R - 1 : 2, :],
                in0=p[0:P, 0 : Rh - 1, :],
                in1=t[0:P, 2:R:2, :],
                op=MIN,
            )
        nc.vector.tensor_tensor(
            out=v[0:P, 0:1, :], in0=p[0:P, 0:1, :], in1=bt[0:P, 1:2, :], op=MIN
        )
        nc.vector.tensor_tensor(
            out=v[0:P, R - 1 : R, :],
            in0=p[0:P, Rh - 1 : Rh, :],
            in1=bt[0:P, 0:1, :],
            op=MIN,
        )
        # ACT: parity-fixing shifted copy (1x, alignment-agnostic)
        vs = vsp.tile([128, R, cc], BF16, name="vs")
        nc.scalar.copy(vs[0:P], v[0:P, :, 1 : cc + 1])
        mid[i] = (v, vs, n_img, P, s0, c0)

    def emit_tail(i):
        v, vs, n_img, P, s0, c0 = mid.pop(i)
        g = gp.tile([128, R, cc], BF16, name="g")
        nc.vector.tensor_tensor(
            out=g[0:P], in0=v[0:P, :, 0:cc], in1=v[0:P, :, 2 : cc + 2], op=MIN
        )
        ob = obp.tile([128, R, cc], BF16, name="ob")
        nc.vector.tensor_tensor(out=ob[0:P], in0=g[0:P], in1=vs[0:P], op=MIN)
        for im in range(n_img):
            dst = o_d[s0 + im, :, c0 : c0 + cc].rearrange(
                "(p r) w -> p r w", p=ppi
            )
            p0 = im * ppi
            nc.scalar.dma_start(dst, ob[p0 : p0 + ppi, :, :])

    # 3-stage software pipeline: tail(i) is emitted after mid(i+1), so the
    # DVE's ob(i) lands behind V(i+1) in its in-order stream -- by then the
    # ACT's vs(i) is long done and the DVE never stalls on the ACT.
    n = len(tiles)
    for i in range(n + 2):
        if i < n:
            emit_front(i)
        if 1 <= i <= n:
            emit_mid(i - 1)
        if i >= 2:
            emit_tail(i - 2)


def _emit_pass(nc, pools, x_d, o_d, C, H, W, cc, rpp, in_f32):
    """Emit one full erosion pass x_d -> o_d into the open TileContext.

    in_f32: x_d is fp32 and must be converted to bf16 on ACT; otherwise
    x_d is bf16 and is used directly. o_d is always bf16.
    """
    ppi = H // rpp  # partitions per image
    ips = max(1, 128 // ppi)  # images per partition-stack
    inp, xbp, bnd, vtm, vt, htm, obp = pools
    R = rpp
    PW = cc + 2  # padded tile width

    tiles = [(s0, c0) for s0 in range(0, C, ips) for c0 in range(0, W, cc)]
    front = {}

    def emit_front(i):
        s0, c0 = tiles[i]
        n_img = min(ips, C - s0)
        P = n_img * ppi
        last = c0 + cc == W
        t = inp.tile([128, R, PW], F32, name="t") if in_f32 else None
        xb = xbp.tile([128, R, PW], BF16, name="xb")
        ld = t if in_f32 else xb
        wlo = max(c0 - 1, 0)
        whi = min(c0 + cc + 1, W)
        dlo = 1 if c0 == 0 else 0
        for im in range(n_img):
            src = x_d[s0 + im, :, wlo:whi].rearrange("(p r) w -> p r w", p=ppi)
            p0 = im * ppi
            nc.sync.dma_start(ld[p0 : p0 + ppi, :, dlo : dlo + (whi - wlo)], src)
        if c0 == 0:
            nc.gpsimd.memset(ld[0:P, :, 0:1], PAD)
        if last:
            nc.gpsimd.memset(ld[0:P, :, PW - 1 : PW], PAD)
        if in_f32:
            nc.scalar.copy(xb[0:P], t[0:P])
        front[i] = (xb, n_img, P, s0, c0)

    def emit_back(i):
        xb, n_img, P, s0, c0 = front.pop(i)
        # boundary-row side tile (bf16): bt[p,0] = first row of the block
        # below (xb[p+1] row 0), bt[p,1] = last row of the block above
        # (xb[p-1] row R-1); image-edge partitions: top -> PAD memset
        # (start partition p0 is 0 mod ppi>=32: legal), bottom -> own-row
        # duplication DMA (min-idempotent; DMA has no start-partition rule).
        bt = bnd.tile([128, 2, PW], BF16)
        for im in range(n_img):
            p0 = im * ppi
            pe = p0 + ppi - 1  # last partition of this image
            nc.sync.dma_start(bt[p0:pe, 0:1, :], xb[p0 + 1 : pe + 1, 0:1, :])
            nc.sync.dma_start(
                bt[p0 + 1 : pe + 1, 1:2, :], xb[p0:pe, R - 1 : R, :]
            )
            nc.sync.dma_start(
                bt[pe : pe + 1, 0:1, :], xb[pe : pe + 1, R - 1 : R, :]
            )
            nc.gpsimd.memset(bt[p0 : p0 + 1, 1:2, :], PAD)

        # ---- V pass (bf16, 2x DVE): v[r] = min(row r-1, r, r+1) ----
        v = vt.tile([128, R, PW], BF16)
        tmp = vtm.tile([128, R - 1, PW], BF16)
        nc.vector.tensor_tensor(
            out=tmp[0:P], in0=xb[0:P, 0 : R - 1, :], in1=xb[0:P, 1:R, :], op=MIN
        )
        nc.vector.tensor_tensor(
            out=v[0:P, 1 : R - 1, :],
            in0=tmp[0:P, 0 : R - 2, :],
            in1=xb[0:P, 2:R, :],
            op=MIN,
        )
        nc.vector.tensor_tensor(
            out=v[0:P, 0:1, :], in0=tmp[0:P, 0:1, :], in1=bt[0:P, 1:2, :], op=MIN
        )
        nc.vector.tensor_tensor(
            out=v[0:P, R - 1 : R, :],
            in0=tmp[0:P, R - 2 : R - 1, :],
            in1=bt[0:P, 0:1, :],
            op=MIN,
        )

        # ---- H pass (bf16, 2x DVE): o[c] = min(v[c], v[c+1], v[c+2]) ----
        h = htm.tile([128, R, cc + 1], BF16)
        nc.vector.tensor_tensor(
            out=h[0:P], in0=v[0:P, :, 0 : cc + 1], in1=v[0:P, :, 1 : cc + 2],
            op=MIN,
        )
        ob = obp.tile([128, R, cc], BF16)
        nc.vector.tensor_tensor(
            out=ob[0:P], in0=h[0:P, :, 0:cc], in1=v[0:P, :, 2 : cc + 2], op=MIN
        )

        # store bf16 from ACT's queue (host widens to fp32 after gather)
        for im in range(n_img):
            dst = o_d[s0 + im, :, c0 : c0 + cc].rearrange(
                "(p r) w -> p r w", p=ppi
            )
            p0 = im * ppi
            nc.scalar.dma_start(dst, ob[p0 : p0 + ppi, :, :])

    # software-pipelined emission: tile i+skew's load/convert lands in
    # every queue before tile i's compute/store, so ACT's in-order
    # sequencer never delays DVE's next tile.
    skew = 2
    for i in range(len(tiles) + skew):
        if i < len(tiles):
            emit_front(i)
        if i >= skew:
            emit_back(i - skew)


def build_erosion(C, H, W, cc=None, rpp=RPP, reps=1, bufs=None, mode="f32eo"):
    """Per-core Bass program: x [C,H,W] f32 -> o [C,H,W] bf16, erosion^reps."""
    if cc is None:
        # f32eo's fp32 tiles need cc=512 to amortize per-op overhead while
        # fitting SBUF; chained (reps>1) builds add a bf16 load pool, so
        # drop to 256 columns.
        cc = 512 if (mode.startswith("f32eo") and reps == 1) else 256
    assert H % rpp == 0
    ppi = H // rpp
    assert ppi <= 128 and W % cc == 0

    nc = bacc.Bacc("TRN2", target_bir_lowering=False, debug=False, num_devices=1)
    x_d = nc.dram_tensor("x", [C, H, W], F32, kind="ExternalInput").ap()
    o_d = nc.dram_tensor("o", [C, H, W], BF16, kind="ExternalOutput").ap()
    # ping-pong DRAM scratch (bf16) for chained passes
    s_d = [
        nc.dram_tensor(f"scratch{i}", [C, H, W], BF16, kind="Internal").ap()
        for i in range(min(2, max(0, reps - 1)))
    ]

    def stage(i):
        src = x_d if i == 0 else s_d[(i - 1) % 2]
        dst = o_d if i == reps - 1 else s_d[i % 2]
        return src, dst

    if mode.startswith("f32eo"):
        bf = {"inp": 2, "xb": 2, "bnd": 2, "vtm": 1, "vt": 1, "htm": 1, "ob": 2}
    else:
        bf = {"inp": 4, "xb": 3, "bnd": 2, "vtm": 1, "vt": 1, "htm": 1, "ob": 3}
    if bufs:
        bf.update(bufs)
    emit = {
        "f32eo": _emit_pass_f32eo,
        "f32eo2": _emit_pass_f32eo2,
        "bf16": _emit_pass,
    }[mode]
    with tile.TileContext(nc) as tc:
        with (
            tc.tile_pool(name="inp", bufs=bf["inp"]) as inp,
            tc.tile_pool(name="xb", bufs=bf["xb"]) as xbp,
            tc.tile_pool(name="bnd", bufs=bf["bnd"]) as bnd,
            tc.tile_pool(name="vtm", bufs=bf["vtm"]) as vtm,
            tc.tile_pool(name="vt", bufs=bf["vt"]) as vt,
            tc.tile_pool(name="htm", bufs=bf["htm"]) as htm,
            tc.tile_pool(name="ob", bufs=bf["ob"]) as obp,
        ):
            pools = (inp, xbp, bnd, vtm, vt, htm, obp)
            for i in range(reps):
                src, dst = stage(i)
                emit(nc, pools, src, dst, C, H, W, cc, rpp, in_f32=(i == 0))
    nc.compile()
    return nc


def build_erosion_bf16a(
    C, H, W, mode="bf16b", cc_first=256, cc_chain=None, rpp=RPP, reps=1,
    bufs=None,
):
    """bf16-mode program builder (modes bf16a / bf16b).

    bf16a: aligned bf16 DVE + ACT parity-fix copy, boundary side-tile.
    bf16b: pure-DVE bf16, halo-row loads, odd-offset H op (measured only
    12% slower than aligned on HW -- no ACT round trip).
    The fp32 first pass uses cc_first columns (the fp32 load tile is the
    SBUF hog); chained bf16 passes use cc_chain. Tile pools size slots to
    the max across passes and unused pools allocate nothing.
    """
    if cc_chain is None:
        cc_chain = 512 if mode == "bf16b" else 256
    if mode.startswith("cg"):
        assert H % 256 == 0
    else:
        assert H % rpp == 0
        ppi = H // rpp
        assert ppi <= 128 and W % cc_first == 0 and W % cc_chain == 0

    nc = bacc.Bacc("TRN2", target_bir_lowering=False, debug=False, num_devices=1)
    x_d = nc.dram_tensor("x", [C, H, W], F32, kind="ExternalInput").ap()
    o_d = nc.dram_tensor("o", [C, H, W], BF16, kind="ExternalOutput").ap()
    # 3-deep scratch rotation: pass k's stores WAR-conflict only with pass
    # k-3's loads, so coarse DRAM dep tracking can't stall the pipeline.
    ns = min(3, max(0, reps - 1))
    s_d = [
        nc.dram_tensor(f"scratch{i}", [C, H, W], BF16, kind="Internal").ap()
        for i in range(ns)
    ]

    def stage(i):
        src = x_d if i == 0 else s_d[(i - 1) % ns]
        dst = o_d if i == reps - 1 else s_d[i % ns]
        return src, dst

    if mode.startswith("cg"):
        bf = {"t32": 2, "t": 3, "bnd": 3, "p": 2, "v": 2, "vs": 1, "g": 1,
              "ob": 3}
    elif mode == "bf16b":
        bf = {"t32": 2, "t": 2, "bnd": 1, "p": 1, "v": 1, "vs": 1, "g": 1,
              "ob": 3}
    else:
        bf = {"t32": 3, "t": 2, "bnd": 2, "p": 1, "v": 2, "vs": 2, "g": 1,
              "ob": 3}
    if bufs:
        bf.update(bufs)
    emit = {"bf16a": _emit_pass_bf16a, "bf16b": _emit_pass_bf16b}.get(mode)
    with tile.TileContext(nc) as tc:
        with (
            tc.tile_pool(name="t32", bufs=bf["t32"]) as t32p,
            tc.tile_pool(name="t", bufs=bf["t"]) as tp,
            tc.tile_pool(name="bnd", bufs=bf["bnd"]) as bnd,
            tc.tile_pool(name="p", bufs=bf["p"]) as pp,
            tc.tile_pool(name="v", bufs=bf["v"]) as vt,
            tc.tile_pool(name="vs", bufs=bf["vs"]) as vsp,
            tc.tile_pool(name="g", bufs=bf["g"]) as gp,
            tc.tile_pool(name="ob", bufs=bf["ob"]) as obp,
        ):
            pools = (t32p, tp, bnd, pp, vt, vsp, gp, obp)
            for i in range(reps):
                src, dst = stage(i)
                if mode.startswith("cg"):
                    probe = mode[2:].lstrip("_") or "none"
                    _emit_pass_cg(
                        nc, pools, src, dst, C, H, W, in_f32=(i == 0),
                        probe=probe,
                    )
                else:
                    cc = cc_first if i == 0 else cc_chain
                    emit(nc, pools, src, dst, C, H, W, cc, rpp, in_f32=(i == 0))
    nc.compile()
    return nc


def _get_program(C, H, W, reps=1, mode="auto"):
    if mode == "auto":
        mode = "cg" if reps == 1 else "rsf"
    key = (C, H, W, reps, mode)
    if key not in _cache:
        if mode in ("rs", "rsf"):
            _cache[key] = build_erosion_rs(
                C, H, W, reps=reps, flat=(mode == "rsf")
            )
        elif mode in ("bf16a", "bf16b") or mode.startswith("cg"):
            _cache[key] = build_erosion_bf16a(C, H, W, mode=mode, reps=reps)
        else:
            _cache[key] = build_erosion(C, H, W, reps=reps, mode=mode)
    return _cache[key]


def kernel(x, m):
    from concourse.bass_utils import run_bass_kernel_spmd

    m = int(np.asarray(m))
    x = np.ascontiguousarray(np.asarray(x), dtype=np.float32)
    B, C, H, W = x.shape
    if m <= 0:
        return x.copy()
    # erosion by a (2m+1)-square = m chained 3x3 erosion passes in one NEFF
    nc = _get_program(C, H, W, reps=m)
    n_cores = 8
    assert B == n_cores, f"expected batch {n_cores}, got {B}"
    in_maps = [{"x": x[b]} for b in range(n_cores)]
    res = run_bass_kernel_spmd(nc, in_maps, core_ids=list(range(n_cores)))
    # device output is bf16; widen to fp32 on the host
    return np.stack(
        [np.asarray(r["o"]).astype(np.float32) for r in res.results], axis=0
    )


if __name__ == "__main__":
    # small-scale CoreSim correctness check (no hardware needed)
    import ml_dtypes

    from concourse.bass_interp import CoreSim

    rng = np.random.default_rng(0)
    C, H, W = 2, 256, 64
    x = rng.standard_normal((C, H, W)).astype(np.float32)

    def bf16r(a):
        return a.astype(ml_dtypes.bfloat16).astype(np.float32)

    for reps, mode in ((1, "cg"), (2, "cg"), (4, "cg")):
        if mode == "cg":
            nc = build_erosion_bf16a(C, H, W, mode=mode, reps=reps)
        elif mode in ("bf16a", "bf16b"):
            nc = build_erosion_bf16a(
                C, H, W, mode=mode, cc_first=16, cc_chain=32, rpp=4, reps=reps
            )
        else:
            nc = build_erosion(C, H, W, cc=32, rpp=4, reps=reps, mode=mode)
        sim = CoreSim(nc)
        sim.tensor("x")[:] = x
        sim.simulate(check_with_hw=False)
        got = np.asarray(sim.tensor("o")).astype(np.float32)

        # bf16a converts the input to bf16 before the first pass
        exp = bf16r(x) if mode in ("bf16", "bf16a") else x
        for _ in range(reps):
            xp = np.pad(exp, ((0, 0), (1, 1), (1, 1)), constant_values=PAD)
            nxt = np.empty_like(exp)
            for i in range(H):
                for j in range(W):
                    nxt[:, i, j] = xp[:, i : i + 3, j : j + 3].min(axis=(1, 2))
            exp = bf16r(nxt)  # device stores bf16 each pass
        ok = np.array_equal(got, exp)
        rel = np.max(np.abs(got - exp) / np.maximum(np.abs(exp), 1e-6))
        print(f"CoreSim reps={reps} mode={mode} exact: {ok} rel={rel:.2e}")

